# revision 1
# baseline (speedup 1.0000x reference)
"""MoE transformer block (QK-norm attention + top-8-of-16 MoE) on 8 trn2 cores.

v2: fp8 DoubleRow expert matmuls (4x modeled PE speedup), weights streamed
once per expert (token-resident activations), AllGather for attention heads
(instead of AllReduce on proj partials), ReduceScatter for the MoE combine
with feature-sharded per-core outputs assembled on host.

Sharding: attention head-parallel (core c owns head c), experts
expert-parallel (core c owns experts 2c, 2c+1), output feature-parallel
(core c returns features [64c, 64c+64) for all tokens).

Everything feature-major ("T layout": features on partitions, tokens on
free); x is transposed host-side so no on-device transposes are needed.
Scales (powers of two, exact): x_res*2^5, weights*2^11, h1*2^5, s*2^5,
o*2^7; descales folded into Act/DVE scale+bias immediates.
"""

import numpy as np
import ml_dtypes

import concourse.bass as bass
import concourse.mybir as mybir
from concourse.tile import TileContext
from concourse.masks import make_identity
from concourse.bass_utils import run_bass_kernel_spmd

BF16 = mybir.dt.bfloat16
F32 = mybir.dt.float32
F8 = mybir.dt.float8e4
AFT = mybir.ActivationFunctionType
MUL = mybir.AluOpType.mult
ADD = mybir.AluOpType.add
DR = mybir.MatmulPerfMode.DoubleRow

P = 128
D = 512          # embed dim
T = 1024         # tokens per batch
N = 2048         # total tokens
E = 16           # experts
EL = 2           # experts per core
HD = 2048        # expert hidden
HDIM = 64        # head dim
NCORES = 8

# fp8 scale exponents (see module docstring)
S_X = 2.0 ** 5
S_W = 2.0 ** 11
S_H = 2.0 ** 5
S_S = 2.0 ** 5
S_O = 2.0 ** 7
S_Y = 2.0 ** 5

_cache = {}


def build_program():
    nc = bass.Bass()
    dp_ = dict(isOutput=False)
    # layout/dtype-transformed inputs (host-side prep; values only, the
    # program is identical on all cores)
    xtb_d = nc.declare_dram_parameter("xtb", [P, 4, N], BF16, **dp_)
    xm_d = nc.declare_dram_parameter("xmask", [P, 4, N], BF16, **dp_)
    sm_d = nc.declare_dram_parameter("smalls", [P, 97], F32, **dp_)
    bqa_d = nc.declare_dram_parameter("bqalpha", [HDIM, 4], F32, **dp_)
    wbig_d = nc.declare_dram_parameter("wbig", [P, 4928], BF16, **dp_)
    sel_d = nc.declare_dram_parameter("selb", [E, EL, P], BF16, **dp_)
    win_d = nc.declare_dram_parameter("w_in8", [EL, P, 2, 16, 2, P], F8, **dp_)
    w1_d = nc.declare_dram_parameter("w18", [EL, 16, P, 2, 8, 2, P], F8, **dp_)
    w2_d = nc.declare_dram_parameter("w28", [EL, 4, P, 4, 8, 2, P], F8, **dp_)
    wout_d = nc.declare_dram_parameter("wout8", [EL, P, 8, 4, 2, P], F8, **dp_)
    eb_d = nc.declare_dram_parameter("ebias", [P, EL, 68], F32, **dp_)
    out_d = nc.declare_dram_parameter("out", [HDIM, N], BF16, isOutput=True)

    groups = [list(range(NCORES))]

    with TileContext(nc, num_cores=NCORES) as tc:
        with (
            tc.tile_pool(name="const", bufs=1) as cp,
            tc.tile_pool(name="pp", bufs=1) as pp,
            tc.tile_pool(name="psE", bufs=4, space="PSUM") as psE,
            tc.tile_pool(name="psB", bufs=2, space="PSUM") as psB,
            tc.tile_pool(name="psS", bufs=2, space="PSUM") as psS,
            tc.tile_pool(name="dram", bufs=1, space="DRAM") as dp,
        ):
            # ---- constants / persistent ----
            ones128b = cp.tile([P, 1], BF16, tag="ones128b")
            nc.vector.memset(ones128b, 1.0)
            ones64b = cp.tile([HDIM, 1], BF16, tag="ones64b")
            nc.vector.memset(ones64b, 1.0)
            ones1r = cp.tile([1, P], F32, tag="ones1r")
            nc.vector.memset(ones1r, 1.0)
            sm = cp.tile([P, 97], F32, tag="sm")
            nc.sync.dma_start(sm, sm_d[:, :])
            g_sb = sm[:, 0:4]
            b_sb = sm[:, 4:8]
            rb_sb = sm[:, 8:24]
            projb_sb = sm[:, 24:28]
            projc0_sb = sm[:, 28:32]
            c0_sb = sm[:, 32:33]
            vbias_sb = sm[:, 33:97]
            sel_sb = cp.tile([E, EL, P], BF16, tag="sel_sb")
            nc.sync.dma_start(sel_sb, sel_d[:, :, :])
            ones32r = cp.tile([1, HDIM], F32, tag="ones32r")
            nc.vector.memset(ones32r, S_Y)
            identb = cp.tile([P, P], BF16, tag="identb")
            make_identity(nc, identb)

            # persistent activations for the expert phase
            moeT = pp.tile([P, 4, N], BF16, tag="moeT")
            xrT8 = pp.tile([P, 4, N], F8, tag="xrT8")
            gatesT = pp.tile([E, N], BF16, tag="gatesT")
            h1T8 = pp.tile([P, 16, N], F8, tag="h1T8")

            ag_in = [dp.tile([HDIM, T], F8, name=f"ag_in{i}")
                     for i in range(2)]
            ag_out = [dp.tile([D, T], F8, addr_space="Shared",
                              name=f"ag_out{i}")
                      for i in range(2)]
            rs_in = dp.tile([D, N], BF16)
            rs_out = dp.tile([HDIM, N], BF16)

            ws = tc.alloc_tile_pool(name="wst", bufs=1)
            with tc.tile_pool(name="s1", bufs=1) as s1:
                xTb = s1.tile([P, 4, N], BF16, tag="xTb")
                nc.sync.dma_start(xTb, xtb_d[:, :, :])
                xnT = s1.tile([P, 4, N], BF16, tag="xnT")

                wbig = s1.tile([P, 4928], BF16, tag="wbig")
                nc.sync.dma_start(wbig, wbig_d[:, :])
                bqa = s1.tile([HDIM, 4], F32, tag="bqa")
                nc.sync.dma_start(bqa, bqa_d[:, :])
                bq_sb = bqa[:, 0:3]
                alpha_sb = bqa[0:1, 3:4]
                # prefetch: expert biases + e0 weights (consumed ~150us later)
                ebt = pp.tile([P, EL, 68], F32, tag="ebias")
                nc.sync.dma_start(ebt, eb_d[:, :, :])
                bias_cp = {"bin": ebt[:, :, 0:16], "b1a": ebt[:, :, 16:32],
                           "b1b": ebt[:, :, 32:48], "b2": ebt[:, :, 48:64],
                           "bout": ebt[:, :, 64:68]}
                pre_win = ws.tile([P, 2, 16, 2, P], F8, tag="win", bufs=1)
                nc.sync.dma_start(pre_win, win_d[0])
                pre_wout = ws.tile([P, 8, 4, 2, P], F8, tag="wot", bufs=1)
                nc.sync.dma_start(pre_wout, wout_d[0])
                pre_w1 = []
                for pr in range(3):
                    t = ws.tile([P, 2, 8, 2, P], F8, tag="w1s", bufs=3)
                    nc.sync.dma_start(t, w1_d[0, pr])
                    pre_w1.append(t)
                qT = s1.tile([HDIM, N], BF16, tag="qT")
                kT = s1.tile([HDIM, N], BF16, tag="kT")
                v_tm = s1.tile([P, 16, HDIM], BF16, tag="v_tm")

                # ---- RMSNorm: rrow = 1/sqrt(mean(x^2)+1e-6) as [1, N] ----
                with tc.tile_pool(name="s1a", bufs=4) as s1a:
                    rrow = s1a.tile([1, N], F32, tag="rrow", bufs=1)
                    for cc in range(4):
                        sl = slice(cc * D, (cc + 1) * D)
                        ps = psS.tile([1, D], F32, tag="ps_small")
                        for kc in range(4):
                            sq = s1a.tile([P, D], BF16, tag="sq_t", bufs=3)
                            nc.scalar.activation(sq, xTb[:, kc, sl], AFT.Square)
                            nc.tensor.matmul(ps, ones128b, sq,
                                             start=(kc == 0), stop=(kc == 3))
                        tmp = s1a.tile([1, D], F32, tag="r_t", bufs=2)
                        nc.vector.tensor_scalar(tmp, ps, 1.0 / D, 1e-6,
                                                op0=MUL, op1=ADD)
                        nc.scalar.activation(tmp, tmp, AFT.Sqrt)
                        nc.vector.reciprocal(rrow[0:1, sl], tmp)
                    # xnT = xTb * bcast(rrow) * g + b (bf16)
                    for cc in range(4):
                        sl = slice(cc * D, (cc + 1) * D)
                        pb = psB.tile([P, D], F32, tag="bc")
                        nc.tensor.matmul(pb, ones1r, rrow[0:1, sl],
                                         start=True, stop=True)
                        for kc in range(4):
                            t = s1a.tile([P, D], F32, tag="xn_t", bufs=3)
                            nc.vector.tensor_mul(t, xTb[:, kc, sl], pb)
                            nc.vector.tensor_scalar(
                                xnT[:, kc, sl], t,
                                g_sb[:, kc:kc + 1], b_sb[:, kc:kc + 1],
                                op0=MUL, op1=ADD)
                        # q/k/v for this chunk overlap the next chunk's norm
                        for wi, dst, bi in ((0, qT, 0), (1, kT, 1)):
                            ps = psS.tile([HDIM, D], F32, tag="ps_small")
                            for kc in range(4):
                                nc.tensor.matmul(
                                    ps,
                                    wbig[:, kc * 192 + wi * HDIM:
                                         kc * 192 + (wi + 1) * HDIM],
                                    xnT[:, kc, sl], start=(kc == 0),
                                    stop=(kc == 3))
                            nc.vector.tensor_scalar_add(dst[:, sl], ps,
                                                        bq_sb[:, bi:bi + 1])
                        for tk in range(cc * 4, cc * 4 + 4):
                            ps = psS.tile([P, HDIM], F32, tag="ps_small")
                            for kc in range(4):
                                nc.tensor.matmul(
                                    ps, xnT[:, kc, tk * P:(tk + 1) * P],
                                    wbig[:, kc * 192 + 128:kc * 192 + 192],
                                    start=(kc == 0), stop=(kc == 3))
                            nc.vector.tensor_add(v_tm[:, tk, :], ps, vbias_sb)

                # ---- attention (own head, both batches) ----
                with tc.tile_pool(name="att", bufs=1) as at, \
                     tc.tile_pool(name="atte", bufs=12) as ate:
                    # q_hat (alpha folded) / k_hat
                    qh = at.tile([HDIM, N], BF16, tag="qh")
                    kh = at.tile([HDIM, N], BF16, tag="kh")
                    for src, dst, use_alpha in ((qT, qh, True), (kT, kh, False)):
                        rn = at.tile([1, N], F32, tag="rn", bufs=2)
                        for cc in range(4):
                            sl = slice(cc * D, (cc + 1) * D)
                            sq = ate.tile([HDIM, D], BF16, tag="sqn", bufs=2)
                            nc.scalar.activation(sq, src[:, sl], AFT.Square)
                            ps = psS.tile([1, D], F32, tag="ps_small")
                            nc.tensor.matmul(ps, ones64b, sq, start=True, stop=True)
                            t = ate.tile([1, D], F32, tag="rn_t", bufs=2)
                            nc.scalar.activation(t, ps, AFT.Sqrt)
                            nc.vector.tensor_scalar_add(t, t, 1e-5)
                            nc.vector.reciprocal(rn[0:1, sl], t)
                        if use_alpha:
                            nc.vector.tensor_scalar_mul(rn, rn, alpha_sb[0:1, 0:1])
                        for cc in range(4):
                            sl = slice(cc * D, (cc + 1) * D)
                            pb = psB.tile([HDIM, D], F32, tag="bc")
                            nc.tensor.matmul(pb, ones1r[0:1, 0:HDIM], rn[0:1, sl],
                                             start=True, stop=True)
                            nc.vector.tensor_mul(dst[:, sl], src[:, sl], pb)
                    # scoresT -> exp (masked only on diagonal blocks) -> denom+AV
                    yhT = at.tile([HDIM, N], F8, tag="yhT")
                    for b in range(2):
                        for qc in range(2):
                            qsl = slice(b * T + qc * D, b * T + (qc + 1) * D)
                            nkc = 4 * qc + 4
                            ex_tiles = []
                            for kc in range(nkc):
                                ksl = slice(b * T + kc * P, b * T + (kc + 1) * P)
                                ps = psE.tile([P, D], F32, tag="mm")
                                nc.tensor.matmul(ps, kh[:, ksl], qh[:, qsl],
                                                 start=True, stop=True)
                                if kc >= 4 * qc:  # diagonal block: mask
                                    et = ate.tile([P, D], BF16, tag="exp_b", bufs=4)
                                    nc.scalar.activation(et, ps, AFT.Exp)
                                    eb = ate.tile([P, D], BF16, tag="exp_m", bufs=7)
                                    nc.vector.tensor_mul(
                                        eb, et,
                                        wbig[:, 768 + (kc - 4 * qc) * D:
                                             768 + (kc - 4 * qc + 1) * D])
                                else:
                                    eb = ate.tile([P, D], BF16, tag="exp_m", bufs=7)
                                    nc.scalar.activation(eb, ps, AFT.Exp)
                                ex_tiles.append(eb)
                            pd = psS.tile([1, D], F32, tag="ps_small")
                            py = psS.tile([HDIM, D], F32, tag="ps_small")
                            for kc in range(nkc):
                                nc.tensor.matmul(pd, ones128b, ex_tiles[kc],
                                                 start=(kc == 0), stop=(kc == nkc - 1))
                            for kc in range(nkc):
                                nc.tensor.matmul(py, v_tm[:, b * 8 + kc, :],
                                                 ex_tiles[kc],
                                                 start=(kc == 0), stop=(kc == nkc - 1))
                            dr = ate.tile([1, D], F32, tag="dr", bufs=2)
                            nc.vector.reciprocal(dr, pd)
                            pb2 = psB.tile([HDIM, D], F32, tag="bc")
                            nc.tensor.matmul(pb2, ones32r, dr,
                                             start=True, stop=True)
                            db = ate.tile([HDIM, D], BF16, tag="db", bufs=2)
                            nc.scalar.activation(db, pb2, AFT.Copy)
                            nc.vector.tensor_mul(yhT[:, qsl], py, db)
                            nc.sync.dma_start(
                                ag_in[b][:, qc * D:(qc + 1) * D], yhT[:, qsl])
                        nc.gpsimd.collective_compute(
                            "AllGather", mybir.AluOpType.bypass,
                            ins=[ag_in[b][:]], outs=[ag_out[b][:]],
                            replica_groups=groups)

                # ---- proj + x_res (feature-major) + router ----
                with tc.tile_pool(name="s2", bufs=1) as s2, \
                     tc.tile_pool(name="s2e", bufs=4) as s2e:
                    xm_sb = s2.tile([P, 4, N], BF16, tag="xm_sb")
                    nc.sync.dma_start(xm_sb, xm_d[:, :, :])
                    yT_sb = s2.tile([P, 4, N], F8, tag="yT_sb")
                    for ts in range(4):
                        nc.sync.dma_start(
                            yT_sb[:, :, ts * D:(ts + 1) * D],
                            ag_out[ts // 2][:, (ts % 2) * D:(ts % 2 + 1) * D]
                            .rearrange("(kc p) n -> p kc n", p=P))
                    xrTb = s2.tile([P, 4, N], BF16, tag="xrTb")
                    for ts in range(4):
                        if ts == 2:
                            # h1(e0) for batch-0 tokens: runs while AG_b1 is
                            # still in flight
                            for hts in range(2):
                                for hc in range(16):
                                    htsl = slice(hts * D, (hts + 1) * D)
                                    ps = psE.tile([P, D], F32, tag="mm")
                                    for kp in range(2):
                                        nc.tensor.matmul(
                                            ps, pre_win[:, kp, hc, :, :],
                                            xrT8[:, 2 * kp:2 * kp + 2, htsl],
                                            start=(kp == 0), stop=(kp == 1),
                                            perf_mode=DR)
                                    if (hc + hts) % 2 == 0:
                                        nc.scalar.activation(
                                            h1T8[:, hc, htsl], ps, AFT.Identity,
                                            scale=S_H / (S_X * S_W),
                                            bias=bias_cp["bin"][:, 0, hc:hc + 1])
                                    else:
                                        nc.vector.tensor_scalar(
                                            h1T8[:, hc, htsl], ps,
                                            S_H / (S_X * S_W),
                                            bias_cp["bin"][:, 0, hc:hc + 1],
                                            op0=MUL, op1=ADD)
                        for dc in range(4):
                            tsl = slice(ts * D, (ts + 1) * D)
                            ps = psE.tile([P, D], F32, tag="mm")
                            for kc in range(4):
                                nc.tensor.matmul(
                                    ps,
                                    wbig[:, 2816 + kc * D + dc * P:
                                         2816 + kc * D + (dc + 1) * P],
                                    yT_sb[:, kc, tsl],
                                    start=(kc == 0), stop=(kc == 3))
                            tmp = s2e.tile([P, D], F32, tag="yp_t", bufs=3)
                            nc.scalar.activation(tmp, ps, AFT.Identity,
                                                 scale=1.0 / S_Y,
                                                 bias=projb_sb[:, dc:dc + 1])
                            nc.vector.tensor_add(xrTb[:, dc, tsl], tmp,
                                                 xTb[:, dc, tsl])
                            nc.scalar.activation(xrT8[:, dc, tsl],
                                                 xrTb[:, dc, tsl], AFT.Copy,
                                                 scale=S_X)
                            # moeT init: yp*c0 + x feature slice (+projb*c0,
                            # folded into xmask host-side)
                            nc.vector.scalar_tensor_tensor(
                                moeT[:, dc, tsl], ps, c0_sb[:, 0:1],
                                xm_sb[:, dc, tsl], op0=MUL, op1=ADD)
                    # router: logits -> exp -> top-8 -> normalized gates
                    routes = s2.tile([P, 16, E], F32, tag="routes")
                    for tk in range(16):
                        ps = psS.tile([P, E], F32, tag="ps_small")
                        for kc in range(4):
                            nc.tensor.matmul(ps, xrTb[:, kc, tk * P:(tk + 1) * P],
                                             wbig[:, 4864 + kc * E:
                                                  4864 + (kc + 1) * E],
                                             start=(kc == 0), stop=(kc == 3))
                        nc.vector.tensor_add(routes[:, tk, :], ps, rb_sb)
                    nc.scalar.activation(routes, routes, AFT.Exp)
                    gates = s2.tile([P, 16, E], F32, tag="gates")
                    for g in range(16):
                        m8 = s2e.tile([P, 8], F32, tag="m8", bufs=2)
                        nc.vector.max(out=m8, in_=routes[:, g, :])
                        zap = s2e.tile([P, E], F32, tag="zap", bufs=2)
                        nc.vector.match_replace(out=zap, in_to_replace=m8,
                                                in_values=routes[:, g, :],
                                                imm_value=0)
                        nc.vector.tensor_sub(gates[:, g, :], routes[:, g, :], zap)
                    gsum = s2.tile([P, 16], F32, tag="gsum")
                    nc.vector.reduce_sum(gsum, gates, axis=mybir.AxisListType.X)
                    nc.vector.reciprocal(gsum, gsum)
                    gates_bf = pp.tile([P, 16, E], BF16, tag="gates_bf")
                    for g in range(16):
                        nc.vector.tensor_scalar_mul(gates_bf[:, g, :],
                                                    gates[:, g, :],
                                                    gsum[:, g:g + 1])

            # ---- experts: dense fp8 DoubleRow eval of 2 local experts ----
            if True:
                with tc.tile_pool(name="eact", bufs=1) as ac, \
                     tc.tile_pool(name="eev", bufs=6) as ev_:
                    for e in range(EL):
                        if e == 0:
                            win_t = pre_win
                            wout_t = pre_wout
                        else:
                            win_t = ws.tile([P, 2, 16, 2, P], F8, tag="win",
                                            bufs=1)
                            nc.sync.dma_start(win_t, win_d[e])
                            wout_t = ws.tile([P, 8, 4, 2, P], F8, tag="wot",
                                             bufs=1)
                            nc.sync.dma_start(wout_t, wout_d[e])
                        # h1 = x_res @ w_in  (fp8, S_H); e0's ts 0/1 were
                        # computed during the proj phase (fills the AG_b1 gap)
                        for ts in (range(2, 4) if e == 0 else range(4)):
                            for hc in range(16):
                                tsl = slice(ts * D, (ts + 1) * D)
                                ps = psE.tile([P, D], F32, tag="mm")
                                for kp in range(2):
                                    nc.tensor.matmul(
                                        ps, win_t[:, kp, hc, :, :],
                                        xrT8[:, 2 * kp:2 * kp + 2, tsl],
                                        start=(kp == 0), stop=(kp == 1),
                                        perf_mode=DR)
                                if (hc + ts) % 2 == 0:
                                    nc.scalar.activation(
                                        h1T8[:, hc, tsl], ps, AFT.Identity,
                                        scale=S_H / (S_X * S_W),
                                        bias=bias_cp["bin"][:, e, hc:hc + 1])
                                else:
                                    nc.vector.tensor_scalar(
                                        h1T8[:, hc, tsl], ps,
                                        S_H / (S_X * S_W),
                                        bias_cp["bin"][:, e, hc:hc + 1],
                                        op0=MUL, op1=ADD)
                        if e == 0:
                            # gatesT transposes, deferred here so the router's
                            # DVE chain never blocks the first h1 matmuls
                            for g in range(16):
                                pt = psS.tile([E, P], BF16, tag="ps_small")
                                nc.tensor.transpose(pt, gates_bf[:, g, :],
                                                    identb)
                                nc.scalar.activation(
                                    gatesT[:, g * P:(g + 1) * P], pt, AFT.Copy)
                        # SwiGLU: s = silu(h@w1b + b1b) * (h@w1a + b1a)
                        sT8 = ac.tile([P, 16, N], F8, tag="sT8", bufs=1)
                        for pr in range(16):
                            if e == 0 and pr < 3:
                                w1_t = pre_w1[pr]
                            else:
                                w1_t = ws.tile([P, 2, 8, 2, P], F8, tag="w1s",
                                               bufs=3)
                                nc.sync.dma_start(w1_t, w1_d[e, pr])
                            for ts in range(4):
                                tsl = slice(ts * D, (ts + 1) * D)
                                pa = psE.tile([P, D], F32, tag="mm")
                                pb = psE.tile([P, D], F32, tag="mm")
                                for kp in range(8):
                                    nc.tensor.matmul(
                                        pa, w1_t[:, 0, kp, :, :],
                                        h1T8[:, 2 * kp:2 * kp + 2, tsl],
                                        start=(kp == 0), stop=(kp == 7),
                                        perf_mode=DR)
                                for kp in range(8):
                                    nc.tensor.matmul(
                                        pb, w1_t[:, 1, kp, :, :],
                                        h1T8[:, 2 * kp:2 * kp + 2, tsl],
                                        start=(kp == 0), stop=(kp == 7),
                                        perf_mode=DR)
                                sil = ev_.tile([P, D], BF16, tag="sil", bufs=4)
                                nc.scalar.activation(
                                    sil, pb, AFT.Silu,
                                    scale=1.0 / (S_H * S_W),
                                    bias=bias_cp["b1b"][:, e, pr:pr + 1])
                                av8 = ev_.tile([P, D], F8, tag="av8", bufs=4)
                                nc.vector.tensor_scalar(
                                    av8, pa, S_S / (S_H * S_W),
                                    bias_cp["b1a"][:, e, pr:pr + 1],
                                    op0=MUL, op1=ADD)
                                nc.vector.tensor_mul(sT8[:, pr, tsl], av8, sil)
                        # o = s @ w2 + b2  (fp8, S_O)
                        oT8 = ac.tile([P, 16, N], F8, tag="oT8", bufs=1)
                        for og in range(4):
                            w2_t = ws.tile([P, 4, 8, 2, P], F8, tag="w2s", bufs=2)
                            nc.sync.dma_start(w2_t, w2_d[e, og])
                            for oc4 in range(4):
                                oc = og * 4 + oc4
                                for ts in range(4):
                                    tsl = slice(ts * D, (ts + 1) * D)
                                    ps = psE.tile([P, D], F32, tag="mm")
                                    for kp in range(8):
                                        nc.tensor.matmul(
                                            ps, w2_t[:, oc4, kp, :, :],
                                            sT8[:, 2 * kp:2 * kp + 2, tsl],
                                            start=(kp == 0), stop=(kp == 7),
                                            perf_mode=DR)
                                    nc.scalar.activation(
                                        oT8[:, oc, tsl], ps, AFT.Identity,
                                        scale=S_O / (S_S * S_W),
                                        bias=bias_cp["b2"][:, e, oc:oc + 1])
                        # gate broadcast [tokens] -> [P, D] per ts chunk
                        gb_tiles = []
                        for ts in range(4):
                            tsl = slice(ts * D, (ts + 1) * D)
                            pg = psB.tile([P, D], F32, tag="bc")
                            nc.tensor.matmul(pg, sel_sb[:, e, :], gatesT[:, tsl],
                                             start=True, stop=True)
                            gb = ev_.tile([P, D], BF16, tag="gb", bufs=4)
                            nc.scalar.activation(gb, pg, AFT.Copy)
                            gb_tiles.append(gb)
                        # eo = o @ w_out + b_out; moeT += gate * eo
                        for dc in range(4):
                            for ts in range(4):
                                tsl = slice(ts * D, (ts + 1) * D)
                                ps = psE.tile([P, D], F32, tag="mm")
                                for kp in range(8):
                                    nc.tensor.matmul(
                                        ps, wout_t[:, kp, dc, :, :],
                                        oT8[:, 2 * kp:2 * kp + 2, tsl],
                                        start=(kp == 0), stop=(kp == 7),
                                        perf_mode=DR)
                                eo = ev_.tile([P, D], F32, tag="eo", bufs=3)
                                nc.scalar.activation(
                                    eo, ps, AFT.Identity, scale=1.0 / (S_O * S_W),
                                    bias=bias_cp["bout"][:, e, dc:dc + 1])
                                t2 = ev_.tile([P, D], F32, tag="t2", bufs=3)
                                nc.vector.tensor_mul(t2, eo, gb_tiles[ts])
                                nc.vector.tensor_add(moeT[:, dc, tsl],
                                                     moeT[:, dc, tsl], t2)
                                if e == 1:
                                    nc.sync.dma_start(
                                        rs_in[dc * P:(dc + 1) * P, tsl],
                                        moeT[:, dc, tsl])

            ws.release()

            # ---- ReduceScatter(x + moe + yp*c0) -> out features ----
            nc.gpsimd.collective_compute(
                "ReduceScatter", mybir.AluOpType.add,
                ins=[rs_in[:]], outs=[rs_out[:]], replica_groups=groups)
            nc.sync.dma_start(out_d[:, :], rs_out[:, :])

    _split_matmul_waits(nc)
    return nc


def _split_matmul_waits(nc):
    """walrus allows only one sync-wait per engine-instruction sync slot; move
    extra waits onto standalone InstEventSemaphore waits inserted before."""
    import concourse.mybir as mybir
    k = 0
    for bb in nc.main_func.blocks:
        il = list(bb.instructions)
        out = []
        changed = False
        for ins in il:
            si = getattr(ins, "sync_info", None)
            if si is not None and len(si.on_wait) > 1 \
                    and type(ins).__name__ != "InstEventSemaphore":
                waits = list(si.on_wait)
                keep, move = waits[-1], waits[:-1]
                for w in move:
                    nop = mybir.InstEventSemaphore(name=f"I-wsplit-{k}",
                                                   ins=[], outs=[])
                    k += 1
                    nop.engine = ins.engine
                    nop.sync_info = type(si)(on_wait=[w], on_update=[])
                    out.append(nop)
                ins.sync_info = type(si)(on_wait=[keep],
                                         on_update=list(si.on_update))
                changed = True
            out.append(ins)
        if changed:
            bb.instructions = out


def _q8w(w):
    """host fp8 cast with fixed 2^11 scale (clipped to TRN e4m3 max)."""
    return np.clip(np.asarray(w, np.float32) * S_W, -240.0, 240.0).astype(
        ml_dtypes.float8_e4m3)


def _prep_inputs(inputs, core):
    bf = ml_dtypes.bfloat16
    f32 = np.float32
    h = core
    x = np.asarray(inputs["x"], f32).reshape(N, D)
    xT = np.ascontiguousarray(x.T)                      # [512, 2048]
    caw = np.asarray(inputs["c_attn_w"], f32)
    cab = np.asarray(inputs["c_attn_b"], f32)
    wqkv = np.concatenate([
        caw[:, h * 64:(h + 1) * 64],
        caw[:, 512 + h * 64:512 + (h + 1) * 64],
        caw[:, 1024 + h * 64:1024 + (h + 1) * 64]], axis=1)  # [512, 192]
    bqkv = np.stack([
        cab[h * 64:(h + 1) * 64],
        cab[512 + h * 64:512 + (h + 1) * 64],
        cab[1024 + h * 64:1024 + (h + 1) * 64]]).astype(f32)  # [3, 64]
    kk = np.arange(4)[None, :, None] * P + np.arange(P)[:, None, None]
    qq = np.arange(D)[None, None, :]
    maskd = (kk <= qq).astype(f32)                       # [128, 4, 512]
    projb = np.asarray(inputs["c_proj_b"], f32)
    projb_col = np.ascontiguousarray(projb.reshape(4, P).T)  # [p, dc]
    c0 = 1.0 if core == 0 else 0.0
    xmask = np.zeros((D, N), f32)
    xmask[64 * core:64 * core + 64, :] = xT[64 * core:64 * core + 64, :]
    xmask += (projb * c0)[:, None]
    xmask = np.ascontiguousarray(
        xmask.reshape(4, P, N).transpose(1, 0, 2)).astype(bf)
    selb = np.zeros((E, EL, P), f32)
    selb[2 * core, 0, :] = 1.0
    selb[2 * core + 1, 1, :] = 1.0

    sl = slice(2 * core, 2 * core + 2)
    w_in = np.asarray(inputs["w_in"], f32)[sl]           # [2, 512, 2048]
    w1 = np.asarray(inputs["w1"], f32)[sl]               # [2, 2048, 4096]
    w2 = np.asarray(inputs["w2"], f32)[sl]               # [2, 2048, 2048]
    w_out = np.asarray(inputs["w_out"], f32)[sl]         # [2, 2048, 512]

    # DoubleRow lhsT layouts (pair index j adjacent to the 128-wide m dim)
    w_in8 = np.ascontiguousarray(
        _q8w(w_in).reshape(EL, 2, 2, P, 16, P)
        .transpose(0, 3, 1, 4, 2, 5))                    # [EL, p, kp, hc, j, m]
    w1a = _q8w(w1[:, :, :HD]).reshape(EL, 8, 2, P, 16, P)
    w1b = _q8w(w1[:, :, HD:]).reshape(EL, 8, 2, P, 16, P)
    w18 = np.stack([w1a, w1b], axis=2)                   # [EL, kp, ab, j, p, pr, m]
    w18 = np.ascontiguousarray(
        w18.transpose(0, 5, 4, 2, 1, 3, 6))              # [EL, pr, p, ab, kp, j, m]
    w28 = _q8w(w2).reshape(EL, 8, 2, P, 16, P) \
        .transpose(0, 4, 3, 1, 2, 5)                     # [EL, oc, p, kp, j, m]
    w28 = np.ascontiguousarray(
        w28.reshape(EL, 4, 4, P, 8, 2, P)
        .transpose(0, 1, 3, 2, 4, 5, 6))                 # [EL, og, p, ocl, kp, j, m]
    wout8 = np.ascontiguousarray(
        _q8w(w_out).reshape(EL, 8, 2, P, 4, P)
        .transpose(0, 3, 1, 4, 2, 5))                    # [EL, p, kp, dc, j, m]

    def bias_t(key, scale, w):
        b = np.asarray(inputs[key], f32)[sl] * scale     # [2, w*128]
        return np.ascontiguousarray(b.reshape(EL, w, P).transpose(2, 0, 1))

    smalls = np.concatenate([
        np.asarray(inputs["g"], f32).reshape(4, P).T,
        np.asarray(inputs["b"], f32).reshape(4, P).T,
        np.broadcast_to(np.asarray(inputs["router_b"], f32), (P, E)),
        projb_col,
        projb_col * c0,
        np.full((P, 1), c0 / S_Y, f32),
        np.broadcast_to(bqkv[2], (P, HDIM)),
    ], axis=1).astype(f32)                               # [P, 97]
    bqa = np.concatenate([
        np.ascontiguousarray(bqkv.T),
        np.full((HDIM, 1), 0.0, f32)], axis=1)           # [64, 4]
    bqa[0, 3] = np.asarray(inputs["alpha"], f32)[h]
    wbig = np.concatenate([
        wqkv.reshape(4, P, 192).transpose(1, 0, 2).reshape(P, 768),
        maskd.reshape(P, 4 * D),
        np.asarray(inputs["c_proj_w"], f32)
        .reshape(4, P, D).transpose(1, 0, 2).reshape(P, 4 * D),
        np.asarray(inputs["router_w"], f32)
        .reshape(4, P, E).transpose(1, 0, 2).reshape(P, 4 * E),
    ], axis=1).astype(bf)                                # [P, 4928]
    b1 = bias_t("b1", 1.0, 32)
    ebias = np.concatenate([
        bias_t("b_in", S_H, 16),
        b1[:, :, :16] * S_S,
        b1[:, :, 16:],
        bias_t("b2", S_O, 16),
        bias_t("b_out", 1.0, 4),
    ], axis=2).astype(f32)                               # [P, 2, 68]
    return {
        "xtb": np.ascontiguousarray(
            xT.reshape(4, P, N).transpose(1, 0, 2)).astype(bf),
        "xmask": xmask,
        "smalls": smalls,
        "bqalpha": bqa,
        "wbig": wbig,
        "selb": selb.astype(bf),
        "w_in8": w_in8, "w18": w18, "w28": w28, "wout8": wout8,
        "ebias": ebias,
    }


last_result = [None]


def kernel(**inputs):
    if "nc" not in _cache:
        _cache["nc"] = build_program()
    nc = _cache["nc"]
    in_maps = [_prep_inputs(inputs, c) for c in range(NCORES)]
    res = run_bass_kernel_spmd(nc, in_maps, core_ids=list(range(NCORES)))
    last_result[0] = res
    outT = np.concatenate(
        [np.asarray(res.results[c]["out"]).astype(np.float32)
         for c in range(NCORES)], axis=0)                # [512, 2048]
    return np.ascontiguousarray(outT.T).reshape(2, 1024, 512)



# revision 24
# speedup vs baseline: 1.0776x; 1.0776x over previous
"""MoE transformer block (QK-norm attention + top-8-of-16 MoE) on 8 trn2 cores.

v4: dense fp8 DoubleRow experts (as v2 baseline) with restructured
scheduling:
- batch-split expert pipeline: each expert runs a b0-pass (tokens 0-1023)
  then a b1-pass, so expert-0's b0 compute starts right after AllGather-0
  and fully hides AllGather-1 + proj-b1;
- the ReduceScatter is split per batch: RS(b0)'s input is complete after
  the last expert's b0-pass and it runs hidden under ~95us of b1 compute,
  leaving only RS(b1) (~18us) in the tail;
- attention-phase cost cuts: softmax denominator folded into an augmented
  v row (kills 24 denominator matmuls), all broadcast matmuls in bf16
  (1 cyc/row instead of 4), RMSNorm's g folded into the broadcast and its
  additive b folded into the qkv biases (host-side);
- SwiGLU's a-path and the moe combine adds alternate DVE/ACT/GpSimd to
  keep the vector engine off the critical path;
- startup DMAs split so RMSNorm starts after the first 0.5MB chunk.

Sharding: attention head-parallel (core c owns head c), experts
expert-parallel (core c owns experts 2c, 2c+1), output feature-parallel
(core c returns features [64c, 64c+64) for all tokens).

Everything feature-major ("T layout": features on partitions, tokens on
free). Scales (powers of two, exact): x_res*2^5, weights*2^11, h1*2^5,
s*2^5, o*2^7; descales folded into Act/DVE scale+bias immediates.
"""

import numpy as np
import ml_dtypes

import concourse.bass as bass
import concourse.mybir as mybir
from concourse.tile import TileContext
from concourse.masks import make_identity
from concourse.bass_utils import run_bass_kernel_spmd

BF16 = mybir.dt.bfloat16
F32 = mybir.dt.float32
F8 = mybir.dt.float8e4
AFT = mybir.ActivationFunctionType
MUL = mybir.AluOpType.mult
ADD = mybir.AluOpType.add
DR = mybir.MatmulPerfMode.DoubleRow

P = 128
D = 512          # embed dim
T = 1024         # tokens per batch
N = 2048         # total tokens
E = 16           # experts
EL = 2           # experts per core
HD = 2048        # expert hidden
HDIM = 64        # head dim
NCORES = 8

S_X = 2.0 ** 5
S_W = 2.0 ** 11
S_H = 2.0 ** 5
S_S = 2.0 ** 5
S_O = 2.0 ** 7
S_Y = 2.0 ** 5

_cache = {}


def build_program():
    nc = bass.Bass()
    dp_ = dict(isOutput=False)
    xtb_d = nc.declare_dram_parameter("xtb", [P, 4, N], BF16, **dp_)
    xm_d = nc.declare_dram_parameter("xmask", [P, 4, N], BF16, **dp_)
    sm_d = nc.declare_dram_parameter("smalls", [P, 97], F32, **dp_)
    bqa_d = nc.declare_dram_parameter("bqalpha", [HDIM, 4], F32, **dp_)
    wbig_d = nc.declare_dram_parameter("wbig", [P, 4928], BF16, **dp_)
    grow_d = nc.declare_dram_parameter("grow", [1, D + HDIM], BF16,
                                       **dp_)
    sel_d = nc.declare_dram_parameter("selb", [E, EL, P], BF16, **dp_)
    win_d = nc.declare_dram_parameter("w_in8", [EL, P, 2, 16, 2, P], F8, **dp_)
    w1_d = nc.declare_dram_parameter("w18", [EL, 16, P, 2, 8, 2, P], F8, **dp_)
    w2_d = nc.declare_dram_parameter("w28", [EL, 4, P, 4, 8, 2, P], F8, **dp_)
    wout_d = nc.declare_dram_parameter("wout8", [EL, P, 8, 4, 2, P], F8, **dp_)
    eb_d = nc.declare_dram_parameter("ebias", [P, EL, 68], F32, **dp_)
    out_d = nc.declare_dram_parameter("out", [HDIM, N], BF16, isOutput=True)

    groups = [list(range(NCORES))]

    with TileContext(nc, num_cores=NCORES) as tc:
        with (
            tc.tile_pool(name="const", bufs=1) as cp,
            tc.tile_pool(name="pp", bufs=1) as pp,
            tc.tile_pool(name="psE", bufs=4, space="PSUM") as psE,
            tc.tile_pool(name="psB", bufs=2, space="PSUM") as psB,
            tc.tile_pool(name="psS", bufs=2, space="PSUM") as psS,
            tc.tile_pool(name="dram", bufs=1, space="DRAM") as dp,
        ):
            # ---- constants / persistent ----
            ws = tc.alloc_tile_pool(name="wst", bufs=1)
            s1pre = tc.alloc_tile_pool(name="s1pre", bufs=1)
            xTb = s1pre.tile([P, 4, N], BF16, tag="xTb")
            nc.sync.dma_start(xTb[:, :, 0:D], xtb_d[:, :, 0:D])
            wbig = s1pre.tile([P, 4928], BF16, tag="wbig")
            nc.sync.dma_start(wbig[:, 0:768], wbig_d[:, 0:768])
            for cc in range(1, 4):
                sl = slice(cc * D, (cc + 1) * D)
                nc.sync.dma_start(xTb[:, :, sl], xtb_d[:, :, sl])
            ones128b = cp.tile([P, 1], BF16, tag="ones128b")
            nc.vector.memset(ones128b, 1.0)
            ones64b = cp.tile([HDIM, 1], BF16, tag="ones64b")
            nc.vector.memset(ones64b, 1.0)
            ones1rb = cp.tile([1, P], BF16, tag="ones1rb")
            nc.vector.memset(ones1rb, 1.0)
            sm = cp.tile([P, 97], F32, tag="sm")
            nc.sync.dma_start(sm, sm_d[:, :])
            mfeat_sb = sm[:, 0:4]
            pbc0_sb = sm[:, 4:8]
            rb_sb = sm[:, 8:24]
            projb_sb = sm[:, 24:28]
            c0_sb = sm[:, 32:33]
            vbias_sb = sm[:, 33:97]
            sel_sb = cp.tile([E, EL, P], BF16, tag="sel_sb")
            nc.sync.dma_start(sel_sb, sel_d[:, :, :])
            grow = cp.tile([1, D + HDIM], BF16, tag="grow")
            nc.sync.dma_start(grow, grow_d[:, :])
            identb = cp.tile([P, P], BF16, tag="identb")
            make_identity(nc, identb)

            # persistent activations for the expert phase
            moeT = pp.tile([P, 4, N], BF16, tag="moeT")
            xrT8 = pp.tile([P, 4, N], F8, tag="xrT8")
            gatesT = pp.tile([E, N], BF16, tag="gatesT")
            h1T8 = pp.tile([P, 16, N], F8, tag="h1T8")

            ag_in = [dp.tile([HDIM, T], F8, name=f"ag_in{i}")
                     for i in range(2)]
            ag_out = [dp.tile([D, T], F8, addr_space="Shared",
                              name=f"ag_out{i}")
                      for i in range(2)]
            rs_in = [dp.tile([D, T], BF16, name=f"rs_in{i}")
                     for i in range(2)]
            rs_out = [dp.tile([HDIM, T], BF16, name=f"rs_out{i}")
                      for i in range(2)]

            with tc.tile_pool(name="s1", bufs=1) as s1:
                nc.sync.dma_start(wbig[:, 768:4928], wbig_d[:, 768:4928])
                bqa = s1.tile([HDIM, 4], F32, tag="bqa")
                nc.sync.dma_start(bqa, bqa_d[:, :])
                bq_sb = bqa[:, 0:3]
                alpha_sb = bqa[0:1, 3:4]
                # prefetch: expert biases + e0 weights (consumed ~90us later)
                ebt = pp.tile([P, EL, 68], F32, tag="ebias")
                nc.sync.dma_start(ebt, eb_d[:, :, :])
                bias_cp = {"bin": ebt[:, :, 0:16], "b1a": ebt[:, :, 16:32],
                           "b1b": ebt[:, :, 32:48], "b2": ebt[:, :, 48:64],
                           "bout": ebt[:, :, 64:68]}
                pre_win = ws.tile([P, 2, 16, 2, P], F8, tag="win", bufs=1)
                nc.sync.dma_start(pre_win, win_d[0])
                pre_wout = ws.tile([P, 8, 4, 2, P], F8, tag="wot", bufs=1)
                nc.sync.dma_start(pre_wout, wout_d[0])
                pre_w1 = []
                for pr in range(3):
                    t = ws.tile([P, 2, 8, 2, P], F8, tag="w1s", bufs=3)
                    nc.sync.dma_start(t, w1_d[0, pr])
                    pre_w1.append(t)
                # ---- RMSNorm + qkv + qk-norm + attention, batch-ordered:
                # all of batch b's chain runs before batch b+1 so AG(b)
                # issues early and b1 prep fills the AG0 window ----
                with tc.tile_pool(name="attp", bufs=1) as ap_, \
                     tc.tile_pool(name="ate", bufs=12) as ate:
                    xnT = ap_.tile([P, 4, N], BF16, tag="xnT")
                    qT = ap_.tile([HDIM, N], BF16, tag="qT")
                    kT = ap_.tile([HDIM, N], BF16, tag="kT")
                    v_aug = ap_.tile([P, 16, HDIM + 1], BF16, tag="v_aug")
                    nc.vector.memset(v_aug, 1.0)
                    qh = ap_.tile([HDIM, N], BF16, tag="qh")
                    kh = ap_.tile([HDIM, N], BF16, tag="kh")
                    yhT = ap_.tile([HDIM, N], F8, tag="yhT")
                    for b in range(2):
                        for cc in (2 * b, 2 * b + 1):
                            sl = slice(cc * D, (cc + 1) * D)
                            ps = psS.tile([1, D], F32, tag="ps_small")
                            for kc in range(4):
                                sq = ate.tile([P, D], BF16, tag="sq_t",
                                              bufs=3)
                                nc.scalar.activation(sq, xTb[:, kc, sl],
                                                     AFT.Square)
                                nc.tensor.matmul(ps, ones128b, sq,
                                                 start=(kc == 0),
                                                 stop=(kc == 3))
                            tmp = ate.tile([1, D], F32, tag="r_t", bufs=2)
                            nc.vector.tensor_scalar(tmp, ps, 1.0 / D, 1e-6,
                                                    op0=MUL, op1=ADD)
                            nc.scalar.activation(tmp, tmp, AFT.Sqrt)
                            rrow = ate.tile([1, D], BF16, tag="rrow",
                                            bufs=2)
                            with nc.allow_low_precision(
                                    reason="bf16 bcast row"):
                                nc.vector.reciprocal(rrow, tmp)
                            # xnT = xTb * bcast(rrow * g)
                            for kc in range(4):
                                pb = psB.tile([P, D], F32, tag="bc")
                                nc.tensor.matmul(
                                    pb, grow[0:1, kc * P:(kc + 1) * P],
                                    rrow[0:1, :], start=True, stop=True)
                                nc.vector.tensor_mul(xnT[:, kc, sl],
                                                     xTb[:, kc, sl], pb)
                            for wi, dst, bi in ((0, qT, 0), (1, kT, 1)):
                                ps2 = psS.tile([HDIM, D], F32,
                                               tag="ps_small")
                                for kc in range(4):
                                    nc.tensor.matmul(
                                        ps2,
                                        wbig[:, kc * 192 + wi * HDIM:
                                             kc * 192 + (wi + 1) * HDIM],
                                        xnT[:, kc, sl], start=(kc == 0),
                                        stop=(kc == 3))
                                nc.vector.tensor_scalar_add(
                                    dst[:, sl], ps2, bq_sb[:, bi:bi + 1])
                            for tk in range(cc * 4, cc * 4 + 4):
                                ps3 = psS.tile([P, HDIM], F32,
                                               tag="ps_small")
                                for kc in range(4):
                                    nc.tensor.matmul(
                                        ps3, xnT[:, kc, tk * P:(tk + 1) * P],
                                        wbig[:, kc * 192 + 128:
                                             kc * 192 + 192],
                                        start=(kc == 0), stop=(kc == 3))
                                nc.vector.tensor_add(v_aug[:, tk, 0:HDIM],
                                                     ps3, vbias_sb)
                            # qk-norm for this chunk (alpha folded in the
                            # q bcast row)
                            for src_, dst, brow in (
                                    (qT, qh, grow[0:1, D:D + HDIM]),
                                    (kT, kh, ones1rb[0:1, 0:HDIM])):
                                sq = ate.tile([HDIM, D], BF16, tag="sqn",
                                              bufs=2)
                                nc.scalar.activation(sq, src_[:, sl],
                                                     AFT.Square)
                                ps4 = psS.tile([1, D], F32, tag="ps_small")
                                nc.tensor.matmul(ps4, ones64b, sq,
                                                 start=True, stop=True)
                                t = ate.tile([1, D], F32, tag="rn_t",
                                             bufs=2)
                                nc.scalar.activation(t, ps4, AFT.Sqrt)
                                nc.vector.tensor_scalar_add(t, t, 1e-5)
                                rn = ate.tile([1, D], BF16, tag="rn",
                                              bufs=2)
                                with nc.allow_low_precision(
                                        reason="bf16 row"):
                                    nc.vector.reciprocal(rn, t)
                                pb = psB.tile([HDIM, D], F32, tag="bc")
                                nc.tensor.matmul(pb, brow, rn[0:1, :],
                                                 start=True, stop=True)
                                nc.vector.tensor_mul(dst[:, sl],
                                                     src_[:, sl], pb)
                        # scoresT -> exp (masked diag) -> AV (denominator
                        # folded into v_aug's ones row) -> yhT -> AG(b)
                        for qc in range(2):
                            qsl = slice(b * T + qc * D, b * T + (qc + 1) * D)
                            nkc = 4 * qc + 4
                            ex_tiles = []
                            for kc in range(nkc):
                                ksl = slice(b * T + kc * P,
                                            b * T + (kc + 1) * P)
                                ps = psE.tile([P, D], F32, tag="mm")
                                nc.tensor.matmul(ps, kh[:, ksl], qh[:, qsl],
                                                 start=True, stop=True)
                                if kc >= 4 * qc:  # diagonal block: mask
                                    et = ate.tile([P, D], BF16, tag="exp_b",
                                                  bufs=4)
                                    nc.scalar.activation(et, ps, AFT.Exp)
                                    eb2 = ate.tile([P, D], BF16,
                                                   tag="exp_m", bufs=7)
                                    nc.vector.tensor_mul(
                                        eb2, et,
                                        wbig[:, 768 + (kc - 4 * qc) * D:
                                             768 + (kc - 4 * qc + 1) * D])
                                else:
                                    eb2 = ate.tile([P, D], BF16,
                                                   tag="exp_m", bufs=7)
                                    nc.scalar.activation(eb2, ps, AFT.Exp)
                                ex_tiles.append(eb2)
                            py = psS.tile([HDIM + 1, D], F32,
                                          tag="ps_small")
                            for kc in range(nkc):
                                nc.tensor.matmul(py, v_aug[:, b * 8 + kc, :],
                                                 ex_tiles[kc],
                                                 start=(kc == 0),
                                                 stop=(kc == nkc - 1))
                            dr = ate.tile([1, D], BF16, tag="dr", bufs=2)
                            with nc.allow_low_precision(
                                    reason="bf16 softmax denom"):
                                nc.vector.reciprocal(dr,
                                                     py[HDIM:HDIM + 1, :])
                            pb2 = psB.tile([HDIM, D], F32, tag="bc")
                            nc.tensor.matmul(pb2, ones1rb[0:1, 0:HDIM], dr,
                                             start=True, stop=True)
                            db = ate.tile([HDIM, D], BF16, tag="db", bufs=2)
                            nc.scalar.activation(db, pb2, AFT.Copy,
                                                 scale=S_Y)
                            nc.vector.tensor_mul(yhT[:, qsl],
                                                 py[0:HDIM, :], db)
                            nc.sync.dma_start(
                                ag_in[b][:, qc * D:(qc + 1) * D],
                                yhT[:, qsl])
                        nc.gpsimd.collective_compute(
                            "AllGather", mybir.AluOpType.bypass,
                            ins=[ag_in[b][:]], outs=[ag_out[b][:]],
                            replica_groups=groups)

                # ---- proj + x_res + router (per batch), interleaved with
                # expert passes so weight DMAs never queue behind the
                # AG1-blocked b1 input DMAs ----
                pass
            # (s1 stays open: xTb / qT / kT / v_aug / wbig / bqa)
            with tc.tile_pool(name="s2", bufs=1) as s2, \
                 tc.tile_pool(name="s2e", bufs=4) as s2e, \
                 tc.tile_pool(name="eact", bufs=1) as ac, \
                 tc.tile_pool(name="eev", bufs=6) as ev_:
                yT_sb = s2.tile([P, 4, N], F8, tag="yT_sb")
                routes = s2.tile([P, 16, E], F32, tag="routes")
                gates = s2.tile([P, 16, E], F32, tag="gates")
                gsum = s2.tile([P, 16], F32, tag="gsum")
                gates_bf = s2.tile([P, 16, E], BF16, tag="gates_bf")

                def emit_batch(b):
                    for ts in (2 * b, 2 * b + 1):
                        nc.sync.dma_start(
                            yT_sb[:, :, ts * D:(ts + 1) * D],
                            ag_out[b][:, (ts % 2) * D:(ts % 2 + 1) * D]
                            .rearrange("(kc p) n -> p kc n", p=P))
                    for ts in (2 * b, 2 * b + 1):
                        for dc in range(4):
                            tsl = slice(ts * D, (ts + 1) * D)
                            ps = psE.tile([P, D], F32, tag="mm")
                            for kc in range(4):
                                nc.tensor.matmul(
                                    ps,
                                    wbig[:, 2816 + kc * D + dc * P:
                                         2816 + kc * D + (dc + 1) * P],
                                    yT_sb[:, kc, tsl],
                                    start=(kc == 0), stop=(kc == 3))
                            # xmb = x*featmask + projb*c0 (replaces the
                            # host xmask tensor)
                            xmb = s2e.tile([P, D], BF16, tag="xmb", bufs=3)
                            nc.vector.tensor_scalar(
                                xmb, xTb[:, dc, tsl],
                                mfeat_sb[:, dc:dc + 1],
                                pbc0_sb[:, dc:dc + 1], op0=MUL, op1=ADD)
                            tmp = s2e.tile([P, D], F32, tag="yp_t", bufs=3)
                            nc.scalar.activation(
                                tmp, ps, AFT.Identity, scale=1.0 / S_Y,
                                bias=projb_sb[:, dc:dc + 1])
                            # x_res written in place over xTb
                            nc.vector.tensor_add(xTb[:, dc, tsl], tmp,
                                                 xTb[:, dc, tsl])
                            nc.scalar.activation(xrT8[:, dc, tsl],
                                                 xTb[:, dc, tsl],
                                                 AFT.Copy, scale=S_X)
                            # moeT init: yp*c0 + x feature slice
                            nc.vector.scalar_tensor_tensor(
                                moeT[:, dc, tsl], ps, c0_sb[:, 0:1],
                                xmb, op0=MUL, op1=ADD)
                def emit_gates(b):
                    # router for this batch -> normalized top-8 gates
                    for tk in range(8 * b, 8 * b + 8):
                        ps = psS.tile([P, E], F32, tag="ps_small")
                        for kc in range(4):
                            nc.tensor.matmul(
                                ps, xTb[:, kc, tk * P:(tk + 1) * P],
                                wbig[:, 4864 + kc * E:4864 + (kc + 1) * E],
                                start=(kc == 0), stop=(kc == 3))
                        nc.vector.tensor_add(routes[:, tk, :], ps, rb_sb)
                    nc.scalar.activation(routes[:, 8 * b:8 * b + 8, :],
                                         routes[:, 8 * b:8 * b + 8, :],
                                         AFT.Exp)
                    for g in range(8 * b, 8 * b + 8):
                        m8 = s2e.tile([P, 8], F32, tag="m8", bufs=2)
                        nc.vector.max(out=m8, in_=routes[:, g, :])
                        zap = s2e.tile([P, E], F32, tag="zap", bufs=2)
                        nc.vector.match_replace(out=zap, in_to_replace=m8,
                                                in_values=routes[:, g, :],
                                                imm_value=0)
                        nc.vector.tensor_sub(gates[:, g, :],
                                             routes[:, g, :], zap)
                    nc.vector.reduce_sum(gsum[:, 8 * b:8 * b + 8],
                                         gates[:, 8 * b:8 * b + 8, :],
                                         axis=mybir.AxisListType.X)
                    nc.vector.reciprocal(gsum[:, 8 * b:8 * b + 8],
                                         gsum[:, 8 * b:8 * b + 8])
                    for g in range(8 * b, 8 * b + 8):
                        nc.vector.tensor_scalar_mul(gates_bf[:, g, :],
                                                    gates[:, g, :],
                                                    gsum[:, g:g + 1])
                    for g in range(8 * b, 8 * b + 8):
                        pt = psS.tile([E, P], BF16, tag="ps_small")
                        nc.tensor.transpose(pt, gates_bf[:, g, :], identb)
                        nc.scalar.activation(
                            gatesT[:, g * P:(g + 1) * P], pt, AFT.Copy)

                held = {}

                def emit_pass(e, half, post_h1=None):
                    ts_range = (2 * half, 2 * half + 1)
                    if half == 0:
                        if e == 0:
                            win_t = pre_win
                            wout_t = pre_wout
                        else:
                            win_t = ws.tile([P, 2, 16, 2, P], F8,
                                            tag="win", bufs=1, name="win_t")
                            nc.sync.dma_start(win_t, win_d[e])
                            wout_t = ws.tile([P, 8, 4, 2, P], F8,
                                             tag="wot", bufs=1,
                                             name="wout_t")
                            nc.sync.dma_start(wout_t, wout_d[e])
                        held[e] = (win_t, wout_t)
                    else:
                        win_t, wout_t = held[e]
                    # h1 = x_res @ w_in  (fp8, S_H)
                    for ts in ts_range:
                        for hc in range(16):
                            tsl = slice(ts * D, (ts + 1) * D)
                            ps = psE.tile([P, D], F32, tag="mm")
                            for kp in range(2):
                                nc.tensor.matmul(
                                    ps, win_t[:, kp, hc, :, :],
                                    xrT8[:, 2 * kp:2 * kp + 2, tsl],
                                    start=(kp == 0), stop=(kp == 1),
                                    perf_mode=DR)
                            if (hc + ts) % 2 == 0:
                                nc.scalar.activation(
                                    h1T8[:, hc, tsl], ps, AFT.Identity,
                                    scale=S_H / (S_X * S_W),
                                    bias=bias_cp["bin"][:, e, hc:hc + 1])
                            else:
                                nc.vector.tensor_scalar(
                                    h1T8[:, hc, tsl], ps,
                                    S_H / (S_X * S_W),
                                    bias_cp["bin"][:, e, hc:hc + 1],
                                    op0=MUL, op1=ADD)
                    if post_h1 is not None:
                        post_h1()
                    # SwiGLU: s = silu(h@w1b + b1b) * (h@w1a + b1a)
                    sT8 = ac.tile([P, 16, T], F8, tag="sT8", bufs=1,
                                  name="sT8")
                    for pr in range(16):
                        if e == 0 and half == 0 and pr < 3:
                            w1_t = pre_w1[pr]
                        else:
                            w1_t = ws.tile([P, 2, 8, 2, P], F8, tag="w1s",
                                           bufs=3, name="w1_t")
                            nc.sync.dma_start(w1_t, w1_d[e, pr])
                        for ts in ts_range:
                            tsl = slice(ts * D, (ts + 1) * D)
                            pa = psE.tile([P, D], F32, tag="mm")
                            pb = psE.tile([P, D], F32, tag="mm")
                            for kp in range(8):
                                nc.tensor.matmul(
                                    pa, w1_t[:, 0, kp, :, :],
                                    h1T8[:, 2 * kp:2 * kp + 2, tsl],
                                    start=(kp == 0), stop=(kp == 7),
                                    perf_mode=DR)
                            for kp in range(8):
                                nc.tensor.matmul(
                                    pb, w1_t[:, 1, kp, :, :],
                                    h1T8[:, 2 * kp:2 * kp + 2, tsl],
                                    start=(kp == 0), stop=(kp == 7),
                                    perf_mode=DR)
                            sil = ev_.tile([P, D], BF16, tag="sil", bufs=4)
                            nc.scalar.activation(
                                sil, pb, AFT.Silu,
                                scale=1.0 / (S_H * S_W),
                                bias=bias_cp["b1b"][:, e, pr:pr + 1])
                            av8 = ev_.tile([P, D], F8, tag="av8", bufs=4)
                            if (pr + ts) % 2 == 0:
                                nc.scalar.activation(
                                    av8, pa, AFT.Identity,
                                    scale=S_S / (S_H * S_W),
                                    bias=bias_cp["b1a"][:, e, pr:pr + 1])
                            else:
                                nc.vector.tensor_scalar(
                                    av8, pa, S_S / (S_H * S_W),
                                    bias_cp["b1a"][:, e, pr:pr + 1],
                                    op0=MUL, op1=ADD)
                            ltsl = slice((ts - 2 * half) * D,
                                         (ts - 2 * half + 1) * D)
                            nc.vector.tensor_mul(sT8[:, pr, ltsl], av8, sil)
                    # o = s @ w2 + b2  (fp8, S_O)
                    oT8 = ac.tile([P, 16, T], F8, tag="oT8", bufs=1,
                                  name="oT8")
                    for og in range(4):
                        w2_t = ws.tile([P, 4, 8, 2, P], F8, tag="w2s",
                                       bufs=2, name="w2_t")
                        nc.sync.dma_start(w2_t, w2_d[e, og])
                        for oc4 in range(4):
                            oc = og * 4 + oc4
                            for ts in ts_range:
                                ltsl = slice((ts - 2 * half) * D,
                                             (ts - 2 * half + 1) * D)
                                ps = psE.tile([P, D], F32, tag="mm")
                                for kp in range(8):
                                    nc.tensor.matmul(
                                        ps, w2_t[:, oc4, kp, :, :],
                                        sT8[:, 2 * kp:2 * kp + 2, ltsl],
                                        start=(kp == 0), stop=(kp == 7),
                                        perf_mode=DR)
                                if (oc + ts) % 2 == 0:
                                    nc.scalar.activation(
                                        oT8[:, oc, ltsl], ps, AFT.Identity,
                                        scale=S_O / (S_S * S_W),
                                        bias=bias_cp["b2"][:, e, oc:oc + 1])
                                else:
                                    nc.vector.tensor_scalar(
                                        oT8[:, oc, ltsl], ps,
                                        S_O / (S_S * S_W),
                                        bias_cp["b2"][:, e, oc:oc + 1],
                                        op0=MUL, op1=ADD)
                    # gate broadcast [tokens] -> [P, D] per ts chunk
                    gb_tiles = {}
                    for ts in ts_range:
                        tsl = slice(ts * D, (ts + 1) * D)
                        pg = psB.tile([P, D], F32, tag="bc")
                        nc.tensor.matmul(pg, sel_sb[:, e, :], gatesT[:, tsl],
                                         start=True, stop=True)
                        gb = ev_.tile([P, D], BF16, tag="gb", bufs=4)
                        nc.scalar.activation(gb, pg, AFT.Copy)
                        gb_tiles[ts] = gb
                    # eo = o @ w_out + b_out; moeT += gate * eo
                    for dc in range(4):
                        for ts in ts_range:
                            tsl = slice(ts * D, (ts + 1) * D)
                            ltsl = slice((ts - 2 * half) * D,
                                         (ts - 2 * half + 1) * D)
                            ps = psE.tile([P, D], F32, tag="mm")
                            for kp in range(8):
                                nc.tensor.matmul(
                                    ps, wout_t[:, kp, dc, :, :],
                                    oT8[:, 2 * kp:2 * kp + 2, ltsl],
                                    start=(kp == 0), stop=(kp == 7),
                                    perf_mode=DR)
                            eo = ev_.tile([P, D], F32, tag="eo", bufs=3)
                            nc.scalar.activation(
                                eo, ps, AFT.Identity,
                                scale=1.0 / (S_O * S_W),
                                bias=bias_cp["bout"][:, e, dc:dc + 1])
                            t2 = ev_.tile([P, D], F32, tag="t2", bufs=3)
                            nc.vector.tensor_mul(t2, eo, gb_tiles[ts])
                            if (dc + ts) % 2 == 0:
                                nc.vector.tensor_add(moeT[:, dc, tsl],
                                                     moeT[:, dc, tsl], t2)
                            else:
                                nc.gpsimd.tensor_add(moeT[:, dc, tsl],
                                                     moeT[:, dc, tsl], t2)
                            if e == 1:
                                nc.sync.dma_start(
                                    rs_in[half][dc * P:(dc + 1) * P,
                                                (ts - 2 * half) * D:
                                                (ts - 2 * half + 1) * D],
                                    moeT[:, dc, tsl])
                    if e == 1:
                        nc.gpsimd.collective_compute(
                            "ReduceScatter", mybir.AluOpType.add,
                            ins=[rs_in[half][:]], outs=[rs_out[half][:]],
                            replica_groups=groups)
                        nc.sync.dma_start(
                            out_d[:, half * T:(half + 1) * T],
                            rs_out[half][:, :])

                emit_batch(0)
                emit_pass(0, 0, post_h1=lambda: emit_gates(0))
                emit_batch(1)
                emit_pass(0, 1, post_h1=lambda: emit_gates(1))
                emit_pass(1, 0)
                emit_pass(1, 1)
            s1pre.release()

            ws.release()

    _split_matmul_waits(nc)
    return nc


def _split_matmul_waits(nc):
    """walrus allows only one sync-wait per engine-instruction sync slot; move
    extra waits onto standalone InstEventSemaphore waits inserted before."""
    import concourse.mybir as mybir
    k = 0
    for bb in nc.main_func.blocks:
        il = list(bb.instructions)
        out = []
        changed = False
        for ins in il:
            si = getattr(ins, "sync_info", None)
            if si is not None and len(si.on_wait) > 1 \
                    and type(ins).__name__ != "InstEventSemaphore":
                waits = list(si.on_wait)
                keep, move = waits[-1], waits[:-1]
                for w in move:
                    nop = mybir.InstEventSemaphore(name=f"I-wsplit-{k}",
                                                   ins=[], outs=[])
                    k += 1
                    nop.engine = ins.engine
                    nop.sync_info = type(si)(on_wait=[w], on_update=[])
                    out.append(nop)
                ins.sync_info = type(si)(on_wait=[keep],
                                         on_update=list(si.on_update))
                changed = True
            out.append(ins)
        if changed:
            bb.instructions = out


def _q8w(w):
    """host fp8 cast with fixed 2^11 scale (clipped to TRN e4m3 max)."""
    return np.clip(np.asarray(w, np.float32) * S_W, -240.0, 240.0).astype(
        ml_dtypes.float8_e4m3)


def _prep_inputs(inputs, core):
    bf = ml_dtypes.bfloat16
    f32 = np.float32
    h = core
    x = np.asarray(inputs["x"], f32).reshape(N, D)
    xT = np.ascontiguousarray(x.T)                      # [512, 2048]
    g = np.asarray(inputs["g"], f32)
    bvec = np.asarray(inputs["b"], f32)
    caw = np.asarray(inputs["c_attn_w"], f32)
    cab = np.asarray(inputs["c_attn_b"], f32)
    wq = caw[:, h * 64:(h + 1) * 64]
    wk = caw[:, 512 + h * 64:512 + (h + 1) * 64]
    wv = caw[:, 1024 + h * 64:1024 + (h + 1) * 64]
    wqkv = np.concatenate([wq, wk, wv], axis=1)          # [512, 192]
    # RMSNorm additive b folded into qkv biases
    bq = bvec @ wq + cab[h * 64:(h + 1) * 64]
    bk = bvec @ wk + cab[512 + h * 64:512 + (h + 1) * 64]
    bv = bvec @ wv + cab[1024 + h * 64:1024 + (h + 1) * 64]
    kk = np.arange(4)[None, :, None] * P + np.arange(P)[:, None, None]
    qq = np.arange(D)[None, None, :]
    maskd = (kk <= qq).astype(f32)                       # [128, 4, 512]
    projb = np.asarray(inputs["c_proj_b"], f32)
    projb_col = np.ascontiguousarray(projb.reshape(4, P).T)  # [p, dc]
    c0 = 1.0 if core == 0 else 0.0
    xmask = np.zeros((D, N), f32)
    xmask[64 * core:64 * core + 64, :] = xT[64 * core:64 * core + 64, :]
    xmask += (projb * c0)[:, None]
    xmask = np.ascontiguousarray(
        xmask.reshape(4, P, N).transpose(1, 0, 2)).astype(bf)
    selb = np.zeros((E, EL, P), f32)
    selb[2 * core, 0, :] = 1.0
    selb[2 * core + 1, 1, :] = 1.0

    sl = slice(2 * core, 2 * core + 2)
    w_in = np.asarray(inputs["w_in"], f32)[sl]           # [2, 512, 2048]
    w1 = np.asarray(inputs["w1"], f32)[sl]               # [2, 2048, 4096]
    w2 = np.asarray(inputs["w2"], f32)[sl]               # [2, 2048, 2048]
    w_out = np.asarray(inputs["w_out"], f32)[sl]         # [2, 2048, 512]

    # DoubleRow lhsT layouts (pair index j adjacent to the 128-wide m dim)
    w_in8 = np.ascontiguousarray(
        _q8w(w_in).reshape(EL, 2, 2, P, 16, P)
        .transpose(0, 3, 1, 4, 2, 5))                    # [EL, p, kp, hc, j, m]
    w1a = _q8w(w1[:, :, :HD]).reshape(EL, 8, 2, P, 16, P)
    w1b = _q8w(w1[:, :, HD:]).reshape(EL, 8, 2, P, 16, P)
    w18 = np.stack([w1a, w1b], axis=2)                   # [EL, kp, ab, j, p, pr, m]
    w18 = np.ascontiguousarray(
        w18.transpose(0, 5, 4, 2, 1, 3, 6))              # [EL, pr, p, ab, kp, j, m]
    w28 = _q8w(w2).reshape(EL, 8, 2, P, 16, P) \
        .transpose(0, 4, 3, 1, 2, 5)                     # [EL, oc, p, kp, j, m]
    w28 = np.ascontiguousarray(
        w28.reshape(EL, 4, 4, P, 8, 2, P)
        .transpose(0, 1, 3, 2, 4, 5, 6))                 # [EL, og, p, ocl, kp, j, m]
    wout8 = np.ascontiguousarray(
        _q8w(w_out).reshape(EL, 8, 2, P, 4, P)
        .transpose(0, 3, 1, 4, 2, 5))                    # [EL, p, kp, dc, j, m]

    def bias_t(key, scale, w):
        b = np.asarray(inputs[key], f32)[sl] * scale     # [2, w*128]
        return np.ascontiguousarray(b.reshape(EL, w, P).transpose(2, 0, 1))

    mfeat = np.zeros((D,), f32)
    mfeat[64 * core:64 * core + 64] = 1.0
    smalls = np.concatenate([
        np.ascontiguousarray(mfeat.reshape(4, P).T),
        np.ascontiguousarray((projb * c0).reshape(4, P).T),
        np.broadcast_to(np.asarray(inputs["router_b"], f32), (P, E)),
        projb_col,
        projb_col * c0,
        np.full((P, 1), c0 / S_Y, f32),
        np.broadcast_to(bv, (P, HDIM)),
    ], axis=1).astype(f32)                               # [P, 97]
    bqa = np.zeros((HDIM, 4), f32)
    bqa[:, 0] = bq
    bqa[:, 1] = bk
    bqa[0, 3] = np.asarray(inputs["alpha"], f32)[h]
    wbig = np.concatenate([
        wqkv.reshape(4, P, 192).transpose(1, 0, 2).reshape(P, 768),
        maskd.reshape(P, 4 * D),
        np.asarray(inputs["c_proj_w"], f32)
        .reshape(4, P, D).transpose(1, 0, 2).reshape(P, 4 * D),
        np.asarray(inputs["router_w"], f32)
        .reshape(4, P, E).transpose(1, 0, 2).reshape(P, 4 * E),
    ], axis=1).astype(bf)                                # [P, 4928]
    b1 = bias_t("b1", 1.0, 32)
    ebias = np.concatenate([
        bias_t("b_in", S_H, 16),
        b1[:, :, :16] * S_S,
        b1[:, :, 16:],
        bias_t("b2", S_O, 16),
        bias_t("b_out", 1.0, 4),
    ], axis=2).astype(f32)                               # [P, 2, 68]
    return {
        "xtb": np.ascontiguousarray(
            xT.reshape(4, P, N).transpose(1, 0, 2)).astype(bf),
        "xmask": xmask,
        "smalls": smalls,
        "bqalpha": bqa,
        "wbig": wbig,
        "grow": np.concatenate(
            [g, np.full((HDIM,), np.asarray(inputs["alpha"], f32)[h])]
        ).reshape(1, D + HDIM).astype(bf),
        "selb": selb.astype(bf),
        "w_in8": w_in8, "w18": w18, "w28": w28, "wout8": wout8,
        "ebias": ebias,
    }


last_result = [None]


def kernel(**inputs):
    if "nc" not in _cache:
        _cache["nc"] = build_program()
    nc = _cache["nc"]
    in_maps = [_prep_inputs(inputs, c) for c in range(NCORES)]
    res = run_bass_kernel_spmd(nc, in_maps, core_ids=list(range(NCORES)))
    last_result[0] = res
    outT = np.concatenate(
        [np.asarray(res.results[c]["out"]).astype(np.float32)
         for c in range(NCORES)], axis=0)                # [512, 2048]
    return np.ascontiguousarray(outT.T).reshape(2, 1024, 512)


# revision 26
# speedup vs baseline: 1.0881x; 1.0098x over previous
"""MoE transformer block (QK-norm attention + top-8-of-16 MoE) on 8 trn2 cores.

v4: dense fp8 DoubleRow experts (as v2 baseline) with restructured
scheduling:
- batch-split expert pipeline: each expert runs a b0-pass (tokens 0-1023)
  then a b1-pass, so expert-0's b0 compute starts right after AllGather-0
  and fully hides AllGather-1 + proj-b1;
- the ReduceScatter is split per batch: RS(b0)'s input is complete after
  the last expert's b0-pass and it runs hidden under ~95us of b1 compute,
  leaving only RS(b1) (~18us) in the tail;
- attention-phase cost cuts: softmax denominator folded into an augmented
  v row (kills 24 denominator matmuls), all broadcast matmuls in bf16
  (1 cyc/row instead of 4), RMSNorm's g folded into the broadcast and its
  additive b folded into the qkv biases (host-side);
- SwiGLU's a-path and the moe combine adds alternate DVE/ACT/GpSimd to
  keep the vector engine off the critical path;
- startup DMAs split so RMSNorm starts after the first 0.5MB chunk.

Sharding: attention head-parallel (core c owns head c), experts
expert-parallel (core c owns experts 2c, 2c+1), output feature-parallel
(core c returns features [64c, 64c+64) for all tokens).

Everything feature-major ("T layout": features on partitions, tokens on
free). Scales (powers of two, exact): x_res*2^5, weights*2^11, h1*2^5,
s*2^5, o*2^7; descales folded into Act/DVE scale+bias immediates.
"""

import numpy as np
import ml_dtypes

import concourse.bass as bass
import concourse.mybir as mybir
from concourse.tile import TileContext
from concourse.masks import make_identity
from concourse.bass_utils import run_bass_kernel_spmd

BF16 = mybir.dt.bfloat16
F32 = mybir.dt.float32
F8 = mybir.dt.float8e4
AFT = mybir.ActivationFunctionType
MUL = mybir.AluOpType.mult
ADD = mybir.AluOpType.add
DR = mybir.MatmulPerfMode.DoubleRow

P = 128
D = 512          # embed dim
T = 1024         # tokens per batch
N = 2048         # total tokens
E = 16           # experts
EL = 2           # experts per core
HD = 2048        # expert hidden
HDIM = 64        # head dim
NCORES = 8

S_X = 2.0 ** 5
S_W = 2.0 ** 11
S_H = 2.0 ** 5
S_S = 2.0 ** 5
S_O = 2.0 ** 7
S_Y = 2.0 ** 5

_cache = {}


def build_program():
    nc = bass.Bass()
    dp_ = dict(isOutput=False)
    xtb_d = nc.declare_dram_parameter("xtb", [P, 4, N], BF16, **dp_)
    xm_d = nc.declare_dram_parameter("xmask", [P, 4, N], BF16, **dp_)
    sm_d = nc.declare_dram_parameter("smalls", [P, 97], F32, **dp_)
    bqa_d = nc.declare_dram_parameter("bqalpha", [HDIM, 4], F32, **dp_)
    wbig_d = nc.declare_dram_parameter("wbig", [P, 4928], BF16, **dp_)
    grow_d = nc.declare_dram_parameter("grow", [1, D + HDIM], BF16,
                                       **dp_)
    sel_d = nc.declare_dram_parameter("selb", [E, EL, P], BF16, **dp_)
    win_d = nc.declare_dram_parameter("w_in8", [EL, P, 2, 16, 2, P], F8, **dp_)
    w1_d = nc.declare_dram_parameter("w18", [EL, 16, P, 2, 8, 2, P], F8, **dp_)
    w2_d = nc.declare_dram_parameter("w28", [EL, 4, P, 4, 8, 2, P], F8, **dp_)
    wout_d = nc.declare_dram_parameter("wout8", [EL, P, 8, 4, 2, P], F8, **dp_)
    eb_d = nc.declare_dram_parameter("ebias", [P, EL, 68], F32, **dp_)
    out_d = nc.declare_dram_parameter("out", [HDIM, N], BF16, isOutput=True)

    groups = [list(range(NCORES))]

    with TileContext(nc, num_cores=NCORES) as tc:
        with (
            tc.tile_pool(name="const", bufs=1) as cp,
            tc.tile_pool(name="pp", bufs=1) as pp,
            tc.tile_pool(name="psE", bufs=4, space="PSUM") as psE,
            tc.tile_pool(name="psB", bufs=2, space="PSUM") as psB,
            tc.tile_pool(name="psS", bufs=2, space="PSUM") as psS,
            tc.tile_pool(name="dram", bufs=1, space="DRAM") as dp,
        ):
            # ---- constants / persistent ----
            ws = tc.alloc_tile_pool(name="wst", bufs=1)
            s1pre = tc.alloc_tile_pool(name="s1pre", bufs=1)
            xTb = s1pre.tile([P, 4, N], BF16, tag="xTb")
            nc.sync.dma_start(xTb[:, :, 0:D], xtb_d[:, :, 0:D])
            wbig = s1pre.tile([P, 4928], BF16, tag="wbig")
            nc.sync.dma_start(wbig[:, 0:768], wbig_d[:, 0:768])
            for cc in range(1, 4):
                sl = slice(cc * D, (cc + 1) * D)
                nc.sync.dma_start(xTb[:, :, sl], xtb_d[:, :, sl])
            ones128b = cp.tile([P, 1], BF16, tag="ones128b")
            nc.vector.memset(ones128b, 1.0)
            ones64b = cp.tile([HDIM, 1], BF16, tag="ones64b")
            nc.vector.memset(ones64b, 1.0)
            ones1rb = cp.tile([1, P], BF16, tag="ones1rb")
            nc.vector.memset(ones1rb, 1.0)
            sm = cp.tile([P, 97], F32, tag="sm")
            nc.sync.dma_start(sm, sm_d[:, :])
            mfeat_sb = sm[:, 0:4]
            pbc0_sb = sm[:, 4:8]
            rb_sb = sm[:, 8:24]
            projb_sb = sm[:, 24:28]
            c0_sb = sm[:, 32:33]
            vbias_sb = sm[:, 33:97]
            sel_sb = cp.tile([E, EL, P], BF16, tag="sel_sb")
            nc.sync.dma_start(sel_sb, sel_d[:, :, :])
            eps6 = cp.tile([1, 1], F32, tag="eps6")
            nc.vector.memset(eps6, 1e-6)
            grow = cp.tile([1, D + HDIM], BF16, tag="grow")
            nc.sync.dma_start(grow, grow_d[:, :])
            identb = cp.tile([P, P], BF16, tag="identb")
            make_identity(nc, identb)

            # persistent activations for the expert phase
            moeT = pp.tile([P, 4, N], BF16, tag="moeT")
            xrT8 = pp.tile([P, 4, N], F8, tag="xrT8")
            gatesT = pp.tile([E, N], BF16, tag="gatesT")
            h1T8 = pp.tile([P, 16, N], F8, tag="h1T8")

            ag_in = [dp.tile([HDIM, T], F8, name=f"ag_in{i}")
                     for i in range(2)]
            ag_out = [dp.tile([D, T], F8, addr_space="Shared",
                              name=f"ag_out{i}")
                      for i in range(2)]
            rs_in = [dp.tile([D, T], BF16, name=f"rs_in{i}")
                     for i in range(2)]
            rs_out = [dp.tile([HDIM, T], BF16, name=f"rs_out{i}")
                      for i in range(2)]

            with tc.tile_pool(name="s1", bufs=1) as s1:
                nc.sync.dma_start(wbig[:, 768:4928], wbig_d[:, 768:4928])
                bqa = s1.tile([HDIM, 4], F32, tag="bqa")
                nc.sync.dma_start(bqa, bqa_d[:, :])
                bq_sb = bqa[:, 0:3]
                alpha_sb = bqa[0:1, 3:4]
                # prefetch: expert biases + e0 weights (consumed ~90us later)
                ebt = pp.tile([P, EL, 68], F32, tag="ebias")
                nc.sync.dma_start(ebt, eb_d[:, :, :])
                bias_cp = {"bin": ebt[:, :, 0:16], "b1a": ebt[:, :, 16:32],
                           "b1b": ebt[:, :, 32:48], "b2": ebt[:, :, 48:64],
                           "bout": ebt[:, :, 64:68]}
                pre_win = ws.tile([P, 2, 16, 2, P], F8, tag="win", bufs=1)
                nc.sync.dma_start(pre_win, win_d[0])
                pre_wout = ws.tile([P, 8, 4, 2, P], F8, tag="wot", bufs=1)
                nc.sync.dma_start(pre_wout, wout_d[0])
                pre_w1 = []
                for pr in range(3):
                    t = ws.tile([P, 2, 8, 2, P], F8, tag="w1s", bufs=3)
                    nc.sync.dma_start(t, w1_d[0, pr])
                    pre_w1.append(t)
                # ---- RMSNorm + qkv + qk-norm + attention, batch-ordered:
                # all of batch b's chain runs before batch b+1 so AG(b)
                # issues early and b1 prep fills the AG0 window ----
                with tc.tile_pool(name="attp", bufs=1) as ap_, \
                     tc.tile_pool(name="ate", bufs=12) as ate:
                    xnT = ap_.tile([P, 4, N], BF16, tag="xnT")
                    qT = ap_.tile([HDIM, N], BF16, tag="qT")
                    kT = ap_.tile([HDIM, N], BF16, tag="kT")
                    v_aug = ap_.tile([P, 16, HDIM + 1], BF16, tag="v_aug")
                    nc.vector.memset(v_aug, 1.0)
                    qh = ap_.tile([HDIM, N], BF16, tag="qh")
                    kh = ap_.tile([HDIM, N], BF16, tag="kh")
                    yhT = ap_.tile([HDIM, N], F8, tag="yhT")
                    for b in range(2):
                        for cc in (2 * b, 2 * b + 1):
                            sl = slice(cc * D, (cc + 1) * D)
                            ps = psS.tile([1, D], F32, tag="ps_small")
                            for kc in range(4):
                                sq = ate.tile([P, D], BF16, tag="sq_t",
                                              bufs=3)
                                nc.scalar.activation(sq, xTb[:, kc, sl],
                                                     AFT.Square)
                                nc.tensor.matmul(ps, ones128b, sq,
                                                 start=(kc == 0),
                                                 stop=(kc == 3))
                            tmp = ate.tile([1, D], F32, tag="r_t", bufs=2)
                            nc.scalar.activation(tmp, ps, AFT.Sqrt,
                                                 scale=1.0 / D,
                                                 bias=eps6[0:1, 0:1])
                            rrow = ate.tile([1, D], BF16, tag="rrow",
                                            bufs=2)
                            with nc.allow_low_precision(
                                    reason="bf16 bcast row"):
                                nc.vector.reciprocal(rrow, tmp)
                            # xnT = xTb * bcast(rrow * g)
                            for kc in range(4):
                                pb = psB.tile([P, D], F32, tag="bc")
                                nc.tensor.matmul(
                                    pb, grow[0:1, kc * P:(kc + 1) * P],
                                    rrow[0:1, :], start=True, stop=True)
                                nc.vector.tensor_mul(xnT[:, kc, sl],
                                                     xTb[:, kc, sl], pb)
                            for wi, dst, bi in ((0, qT, 0), (1, kT, 1)):
                                ps2 = psS.tile([HDIM, D], F32,
                                               tag="ps_small")
                                for kc in range(4):
                                    nc.tensor.matmul(
                                        ps2,
                                        wbig[:, kc * 192 + wi * HDIM:
                                             kc * 192 + (wi + 1) * HDIM],
                                        xnT[:, kc, sl], start=(kc == 0),
                                        stop=(kc == 3))
                                nc.scalar.activation(
                                    dst[:, sl], ps2, AFT.Identity,
                                    bias=bq_sb[:, bi:bi + 1])
                            for tk in range(cc * 4, cc * 4 + 4):
                                ps3 = psS.tile([P, HDIM], F32,
                                               tag="ps_small")
                                for kc in range(4):
                                    nc.tensor.matmul(
                                        ps3, xnT[:, kc, tk * P:(tk + 1) * P],
                                        wbig[:, kc * 192 + 128:
                                             kc * 192 + 192],
                                        start=(kc == 0), stop=(kc == 3))
                                nc.vector.tensor_add(v_aug[:, tk, 0:HDIM],
                                                     ps3, vbias_sb)
                            # qk-norm for this chunk (alpha folded in the
                            # q bcast row)
                            for src_, dst, brow in (
                                    (qT, qh, grow[0:1, D:D + HDIM]),
                                    (kT, kh, ones1rb[0:1, 0:HDIM])):
                                sq = ate.tile([HDIM, D], BF16, tag="sqn",
                                              bufs=2)
                                nc.scalar.activation(sq, src_[:, sl],
                                                     AFT.Square)
                                ps4 = psS.tile([1, D], F32, tag="ps_small")
                                nc.tensor.matmul(ps4, ones64b, sq,
                                                 start=True, stop=True)
                                t = ate.tile([1, D], F32, tag="rn_t",
                                             bufs=2)
                                nc.scalar.activation(t, ps4, AFT.Sqrt)
                                nc.vector.tensor_scalar_add(t, t, 1e-5)
                                rn = ate.tile([1, D], BF16, tag="rn",
                                              bufs=2)
                                with nc.allow_low_precision(
                                        reason="bf16 row"):
                                    nc.vector.reciprocal(rn, t)
                                pb = psB.tile([HDIM, D], F32, tag="bc")
                                nc.tensor.matmul(pb, brow, rn[0:1, :],
                                                 start=True, stop=True)
                                nc.vector.tensor_mul(dst[:, sl],
                                                     src_[:, sl], pb)
                        # scoresT -> exp (masked diag) -> AV (denominator
                        # folded into v_aug's ones row) -> yhT -> AG(b)
                        for qc in range(2):
                            qsl = slice(b * T + qc * D, b * T + (qc + 1) * D)
                            nkc = 4 * qc + 4
                            ex_tiles = []
                            for kc in range(nkc):
                                ksl = slice(b * T + kc * P,
                                            b * T + (kc + 1) * P)
                                ps = psE.tile([P, D], F32, tag="mm")
                                nc.tensor.matmul(ps, kh[:, ksl], qh[:, qsl],
                                                 start=True, stop=True)
                                if kc >= 4 * qc:  # diagonal block: mask
                                    et = ate.tile([P, D], BF16, tag="exp_b",
                                                  bufs=4)
                                    nc.scalar.activation(et, ps, AFT.Exp)
                                    eb2 = ate.tile([P, D], BF16,
                                                   tag="exp_m", bufs=7)
                                    nc.vector.tensor_mul(
                                        eb2, et,
                                        wbig[:, 768 + (kc - 4 * qc) * D:
                                             768 + (kc - 4 * qc + 1) * D])
                                else:
                                    eb2 = ate.tile([P, D], BF16,
                                                   tag="exp_m", bufs=7)
                                    nc.scalar.activation(eb2, ps, AFT.Exp)
                                ex_tiles.append(eb2)
                            py = psS.tile([HDIM + 1, D], F32,
                                          tag="ps_small")
                            for kc in range(nkc):
                                nc.tensor.matmul(py, v_aug[:, b * 8 + kc, :],
                                                 ex_tiles[kc],
                                                 start=(kc == 0),
                                                 stop=(kc == nkc - 1))
                            dr = ate.tile([1, D], BF16, tag="dr", bufs=2)
                            with nc.allow_low_precision(
                                    reason="bf16 softmax denom"):
                                nc.vector.reciprocal(dr,
                                                     py[HDIM:HDIM + 1, :])
                            pb2 = psB.tile([HDIM, D], F32, tag="bc")
                            nc.tensor.matmul(pb2, ones1rb[0:1, 0:HDIM], dr,
                                             start=True, stop=True)
                            db = ate.tile([HDIM, D], BF16, tag="db", bufs=2)
                            nc.scalar.activation(db, pb2, AFT.Copy,
                                                 scale=S_Y)
                            nc.vector.tensor_mul(yhT[:, qsl],
                                                 py[0:HDIM, :], db)
                            nc.sync.dma_start(
                                ag_in[b][:, qc * D:(qc + 1) * D],
                                yhT[:, qsl])
                        nc.gpsimd.collective_compute(
                            "AllGather", mybir.AluOpType.bypass,
                            ins=[ag_in[b][:]], outs=[ag_out[b][:]],
                            replica_groups=groups)

                # ---- proj + x_res + router (per batch), interleaved with
                # expert passes so weight DMAs never queue behind the
                # AG1-blocked b1 input DMAs ----
                pass
            # (s1 stays open: xTb / qT / kT / v_aug / wbig / bqa)
            with tc.tile_pool(name="s2", bufs=1) as s2, \
                 tc.tile_pool(name="s2e", bufs=4) as s2e, \
                 tc.tile_pool(name="eact", bufs=1) as ac, \
                 tc.tile_pool(name="eev", bufs=6) as ev_:
                yT_sb = s2.tile([P, 4, N], F8, tag="yT_sb")
                routes = s2.tile([P, 16, E], F32, tag="routes")
                gates = s2.tile([P, 16, E], F32, tag="gates")
                gsum = s2.tile([P, 16], F32, tag="gsum")
                gates_bf = s2.tile([P, 16, E], BF16, tag="gates_bf")

                def emit_batch(b):
                    for ts in (2 * b, 2 * b + 1):
                        nc.sync.dma_start(
                            yT_sb[:, :, ts * D:(ts + 1) * D],
                            ag_out[b][:, (ts % 2) * D:(ts % 2 + 1) * D]
                            .rearrange("(kc p) n -> p kc n", p=P))
                    for ts in (2 * b, 2 * b + 1):
                        for dc in range(4):
                            tsl = slice(ts * D, (ts + 1) * D)
                            ps = psE.tile([P, D], F32, tag="mm")
                            for kc in range(4):
                                nc.tensor.matmul(
                                    ps,
                                    wbig[:, 2816 + kc * D + dc * P:
                                         2816 + kc * D + (dc + 1) * P],
                                    yT_sb[:, kc, tsl],
                                    start=(kc == 0), stop=(kc == 3))
                            # xmb = x*featmask + projb*c0 (replaces the
                            # host xmask tensor)
                            xmb = s2e.tile([P, D], BF16, tag="xmb", bufs=3)
                            nc.vector.tensor_scalar(
                                xmb, xTb[:, dc, tsl],
                                mfeat_sb[:, dc:dc + 1],
                                pbc0_sb[:, dc:dc + 1], op0=MUL, op1=ADD)
                            tmp = s2e.tile([P, D], F32, tag="yp_t", bufs=3)
                            nc.scalar.activation(
                                tmp, ps, AFT.Identity, scale=1.0 / S_Y,
                                bias=projb_sb[:, dc:dc + 1])
                            # x_res written in place over xTb
                            nc.vector.tensor_add(xTb[:, dc, tsl], tmp,
                                                 xTb[:, dc, tsl])
                            nc.scalar.activation(xrT8[:, dc, tsl],
                                                 xTb[:, dc, tsl],
                                                 AFT.Copy, scale=S_X)
                            # moeT init: yp*c0 + x feature slice
                            nc.vector.scalar_tensor_tensor(
                                moeT[:, dc, tsl], ps, c0_sb[:, 0:1],
                                xmb, op0=MUL, op1=ADD)
                def emit_gates(b):
                    # router for this batch -> normalized top-8 gates
                    for tk in range(8 * b, 8 * b + 8):
                        ps = psS.tile([P, E], F32, tag="ps_small")
                        for kc in range(4):
                            nc.tensor.matmul(
                                ps, xTb[:, kc, tk * P:(tk + 1) * P],
                                wbig[:, 4864 + kc * E:4864 + (kc + 1) * E],
                                start=(kc == 0), stop=(kc == 3))
                        nc.vector.tensor_add(routes[:, tk, :], ps, rb_sb)
                    nc.scalar.activation(routes[:, 8 * b:8 * b + 8, :],
                                         routes[:, 8 * b:8 * b + 8, :],
                                         AFT.Exp)
                    for g in range(8 * b, 8 * b + 8):
                        m8 = s2e.tile([P, 8], F32, tag="m8", bufs=2)
                        nc.vector.max(out=m8, in_=routes[:, g, :])
                        zap = s2e.tile([P, E], F32, tag="zap", bufs=2)
                        nc.vector.match_replace(out=zap, in_to_replace=m8,
                                                in_values=routes[:, g, :],
                                                imm_value=0)
                        nc.vector.tensor_sub(gates[:, g, :],
                                             routes[:, g, :], zap)
                    nc.vector.reduce_sum(gsum[:, 8 * b:8 * b + 8],
                                         gates[:, 8 * b:8 * b + 8, :],
                                         axis=mybir.AxisListType.X)
                    nc.vector.reciprocal(gsum[:, 8 * b:8 * b + 8],
                                         gsum[:, 8 * b:8 * b + 8])
                    for g in range(8 * b, 8 * b + 8):
                        nc.vector.tensor_scalar_mul(gates_bf[:, g, :],
                                                    gates[:, g, :],
                                                    gsum[:, g:g + 1])
                    for g in range(8 * b, 8 * b + 8):
                        pt = psS.tile([E, P], BF16, tag="ps_small")
                        nc.tensor.transpose(pt, gates_bf[:, g, :], identb)
                        nc.scalar.activation(
                            gatesT[:, g * P:(g + 1) * P], pt, AFT.Copy)

                held = {}

                def emit_pass(e, half, post_h1=None):
                    ts_range = (2 * half, 2 * half + 1)
                    if half == 0:
                        if e == 0:
                            win_t = pre_win
                            wout_t = pre_wout
                        else:
                            win_t = ws.tile([P, 2, 16, 2, P], F8,
                                            tag="win", bufs=1, name="win_t")
                            nc.sync.dma_start(win_t, win_d[e])
                            wout_t = ws.tile([P, 8, 4, 2, P], F8,
                                             tag="wot", bufs=1,
                                             name="wout_t")
                            nc.sync.dma_start(wout_t, wout_d[e])
                        held[e] = (win_t, wout_t)
                    else:
                        win_t, wout_t = held[e]
                    # h1 = x_res @ w_in  (fp8, S_H)
                    for ts in ts_range:
                        for hc in range(16):
                            tsl = slice(ts * D, (ts + 1) * D)
                            ps = psE.tile([P, D], F32, tag="mm")
                            for kp in range(2):
                                nc.tensor.matmul(
                                    ps, win_t[:, kp, hc, :, :],
                                    xrT8[:, 2 * kp:2 * kp + 2, tsl],
                                    start=(kp == 0), stop=(kp == 1),
                                    perf_mode=DR)
                            if (hc + ts) % 2 == 0:
                                nc.scalar.activation(
                                    h1T8[:, hc, tsl], ps, AFT.Identity,
                                    scale=S_H / (S_X * S_W),
                                    bias=bias_cp["bin"][:, e, hc:hc + 1])
                            else:
                                nc.vector.tensor_scalar(
                                    h1T8[:, hc, tsl], ps,
                                    S_H / (S_X * S_W),
                                    bias_cp["bin"][:, e, hc:hc + 1],
                                    op0=MUL, op1=ADD)
                    if post_h1 is not None:
                        post_h1()
                    # SwiGLU: s = silu(h@w1b + b1b) * (h@w1a + b1a)
                    sT8 = ac.tile([P, 16, T], F8, tag="sT8", bufs=1,
                                  name="sT8")
                    for pr in range(16):
                        if e == 0 and half == 0 and pr < 3:
                            w1_t = pre_w1[pr]
                        else:
                            w1_t = ws.tile([P, 2, 8, 2, P], F8, tag="w1s",
                                           bufs=3, name="w1_t")
                            nc.sync.dma_start(w1_t, w1_d[e, pr])
                        for ts in ts_range:
                            tsl = slice(ts * D, (ts + 1) * D)
                            pa = psE.tile([P, D], F32, tag="mm")
                            pb = psE.tile([P, D], F32, tag="mm")
                            for kp in range(8):
                                nc.tensor.matmul(
                                    pa, w1_t[:, 0, kp, :, :],
                                    h1T8[:, 2 * kp:2 * kp + 2, tsl],
                                    start=(kp == 0), stop=(kp == 7),
                                    perf_mode=DR)
                            for kp in range(8):
                                nc.tensor.matmul(
                                    pb, w1_t[:, 1, kp, :, :],
                                    h1T8[:, 2 * kp:2 * kp + 2, tsl],
                                    start=(kp == 0), stop=(kp == 7),
                                    perf_mode=DR)
                            sil = ev_.tile([P, D], BF16, tag="sil", bufs=4)
                            nc.scalar.activation(
                                sil, pb, AFT.Silu,
                                scale=1.0 / (S_H * S_W),
                                bias=bias_cp["b1b"][:, e, pr:pr + 1])
                            av8 = ev_.tile([P, D], F8, tag="av8", bufs=4)
                            if (pr + ts) % 2 == 0:
                                nc.scalar.activation(
                                    av8, pa, AFT.Identity,
                                    scale=S_S / (S_H * S_W),
                                    bias=bias_cp["b1a"][:, e, pr:pr + 1])
                            else:
                                nc.vector.tensor_scalar(
                                    av8, pa, S_S / (S_H * S_W),
                                    bias_cp["b1a"][:, e, pr:pr + 1],
                                    op0=MUL, op1=ADD)
                            ltsl = slice((ts - 2 * half) * D,
                                         (ts - 2 * half + 1) * D)
                            nc.vector.tensor_mul(sT8[:, pr, ltsl], av8, sil)
                    # o = s @ w2 + b2  (fp8, S_O)
                    oT8 = ac.tile([P, 16, T], F8, tag="oT8", bufs=1,
                                  name="oT8")
                    for og in range(4):
                        w2_t = ws.tile([P, 4, 8, 2, P], F8, tag="w2s",
                                       bufs=2, name="w2_t")
                        nc.sync.dma_start(w2_t, w2_d[e, og])
                        for oc4 in range(4):
                            oc = og * 4 + oc4
                            for ts in ts_range:
                                ltsl = slice((ts - 2 * half) * D,
                                             (ts - 2 * half + 1) * D)
                                ps = psE.tile([P, D], F32, tag="mm")
                                for kp in range(8):
                                    nc.tensor.matmul(
                                        ps, w2_t[:, oc4, kp, :, :],
                                        sT8[:, 2 * kp:2 * kp + 2, ltsl],
                                        start=(kp == 0), stop=(kp == 7),
                                        perf_mode=DR)
                                if (oc + ts) % 2 == 0:
                                    nc.scalar.activation(
                                        oT8[:, oc, ltsl], ps, AFT.Identity,
                                        scale=S_O / (S_S * S_W),
                                        bias=bias_cp["b2"][:, e, oc:oc + 1])
                                else:
                                    nc.vector.tensor_scalar(
                                        oT8[:, oc, ltsl], ps,
                                        S_O / (S_S * S_W),
                                        bias_cp["b2"][:, e, oc:oc + 1],
                                        op0=MUL, op1=ADD)
                    # gate broadcast [tokens] -> [P, D] per ts chunk
                    gb_tiles = {}
                    for ts in ts_range:
                        tsl = slice(ts * D, (ts + 1) * D)
                        pg = psB.tile([P, D], F32, tag="bc")
                        nc.tensor.matmul(pg, sel_sb[:, e, :], gatesT[:, tsl],
                                         start=True, stop=True)
                        gb = ev_.tile([P, D], BF16, tag="gb", bufs=4)
                        nc.scalar.activation(gb, pg, AFT.Copy)
                        gb_tiles[ts] = gb
                    # eo = o @ w_out + b_out; moeT += gate * eo
                    for dc in range(4):
                        for ts in ts_range:
                            tsl = slice(ts * D, (ts + 1) * D)
                            ltsl = slice((ts - 2 * half) * D,
                                         (ts - 2 * half + 1) * D)
                            ps = psE.tile([P, D], F32, tag="mm")
                            for kp in range(8):
                                nc.tensor.matmul(
                                    ps, wout_t[:, kp, dc, :, :],
                                    oT8[:, 2 * kp:2 * kp + 2, ltsl],
                                    start=(kp == 0), stop=(kp == 7),
                                    perf_mode=DR)
                            eo = ev_.tile([P, D], F32, tag="eo", bufs=3)
                            nc.scalar.activation(
                                eo, ps, AFT.Identity,
                                scale=1.0 / (S_O * S_W),
                                bias=bias_cp["bout"][:, e, dc:dc + 1])
                            t2 = ev_.tile([P, D], F32, tag="t2", bufs=3)
                            nc.vector.tensor_mul(t2, eo, gb_tiles[ts])
                            if (dc + ts) % 2 == 0:
                                nc.vector.tensor_add(moeT[:, dc, tsl],
                                                     moeT[:, dc, tsl], t2)
                            else:
                                nc.gpsimd.tensor_add(moeT[:, dc, tsl],
                                                     moeT[:, dc, tsl], t2)
                            if e == 1:
                                nc.sync.dma_start(
                                    rs_in[half][dc * P:(dc + 1) * P,
                                                (ts - 2 * half) * D:
                                                (ts - 2 * half + 1) * D],
                                    moeT[:, dc, tsl])
                    if e == 1:
                        nc.gpsimd.collective_compute(
                            "ReduceScatter", mybir.AluOpType.add,
                            ins=[rs_in[half][:]], outs=[rs_out[half][:]],
                            replica_groups=groups)
                        nc.sync.dma_start(
                            out_d[:, half * T:(half + 1) * T],
                            rs_out[half][:, :])

                emit_batch(0)
                emit_pass(0, 0, post_h1=lambda: emit_gates(0))
                emit_batch(1)
                emit_pass(0, 1, post_h1=lambda: emit_gates(1))
                emit_pass(1, 0)
                emit_pass(1, 1)
            s1pre.release()

            ws.release()

    _split_matmul_waits(nc)
    return nc


def _split_matmul_waits(nc):
    """walrus allows only one sync-wait per engine-instruction sync slot; move
    extra waits onto standalone InstEventSemaphore waits inserted before."""
    import concourse.mybir as mybir
    k = 0
    for bb in nc.main_func.blocks:
        il = list(bb.instructions)
        out = []
        changed = False
        for ins in il:
            si = getattr(ins, "sync_info", None)
            if si is not None and len(si.on_wait) > 1 \
                    and type(ins).__name__ != "InstEventSemaphore":
                waits = list(si.on_wait)
                keep, move = waits[-1], waits[:-1]
                for w in move:
                    nop = mybir.InstEventSemaphore(name=f"I-wsplit-{k}",
                                                   ins=[], outs=[])
                    k += 1
                    nop.engine = ins.engine
                    nop.sync_info = type(si)(on_wait=[w], on_update=[])
                    out.append(nop)
                ins.sync_info = type(si)(on_wait=[keep],
                                         on_update=list(si.on_update))
                changed = True
            out.append(ins)
        if changed:
            bb.instructions = out


def _q8w(w):
    """host fp8 cast with fixed 2^11 scale (clipped to TRN e4m3 max)."""
    return np.clip(np.asarray(w, np.float32) * S_W, -240.0, 240.0).astype(
        ml_dtypes.float8_e4m3)


def _prep_inputs(inputs, core):
    bf = ml_dtypes.bfloat16
    f32 = np.float32
    h = core
    x = np.asarray(inputs["x"], f32).reshape(N, D)
    xT = np.ascontiguousarray(x.T)                      # [512, 2048]
    g = np.asarray(inputs["g"], f32)
    bvec = np.asarray(inputs["b"], f32)
    caw = np.asarray(inputs["c_attn_w"], f32)
    cab = np.asarray(inputs["c_attn_b"], f32)
    wq = caw[:, h * 64:(h + 1) * 64]
    wk = caw[:, 512 + h * 64:512 + (h + 1) * 64]
    wv = caw[:, 1024 + h * 64:1024 + (h + 1) * 64]
    wqkv = np.concatenate([wq, wk, wv], axis=1)          # [512, 192]
    # RMSNorm additive b folded into qkv biases
    bq = bvec @ wq + cab[h * 64:(h + 1) * 64]
    bk = bvec @ wk + cab[512 + h * 64:512 + (h + 1) * 64]
    bv = bvec @ wv + cab[1024 + h * 64:1024 + (h + 1) * 64]
    kk = np.arange(4)[None, :, None] * P + np.arange(P)[:, None, None]
    qq = np.arange(D)[None, None, :]
    maskd = (kk <= qq).astype(f32)                       # [128, 4, 512]
    projb = np.asarray(inputs["c_proj_b"], f32)
    projb_col = np.ascontiguousarray(projb.reshape(4, P).T)  # [p, dc]
    c0 = 1.0 if core == 0 else 0.0
    xmask = np.zeros((D, N), f32)
    xmask[64 * core:64 * core + 64, :] = xT[64 * core:64 * core + 64, :]
    xmask += (projb * c0)[:, None]
    xmask = np.ascontiguousarray(
        xmask.reshape(4, P, N).transpose(1, 0, 2)).astype(bf)
    selb = np.zeros((E, EL, P), f32)
    selb[2 * core, 0, :] = 1.0
    selb[2 * core + 1, 1, :] = 1.0

    sl = slice(2 * core, 2 * core + 2)
    w_in = np.asarray(inputs["w_in"], f32)[sl]           # [2, 512, 2048]
    w1 = np.asarray(inputs["w1"], f32)[sl]               # [2, 2048, 4096]
    w2 = np.asarray(inputs["w2"], f32)[sl]               # [2, 2048, 2048]
    w_out = np.asarray(inputs["w_out"], f32)[sl]         # [2, 2048, 512]

    # DoubleRow lhsT layouts (pair index j adjacent to the 128-wide m dim)
    w_in8 = np.ascontiguousarray(
        _q8w(w_in).reshape(EL, 2, 2, P, 16, P)
        .transpose(0, 3, 1, 4, 2, 5))                    # [EL, p, kp, hc, j, m]
    w1a = _q8w(w1[:, :, :HD]).reshape(EL, 8, 2, P, 16, P)
    w1b = _q8w(w1[:, :, HD:]).reshape(EL, 8, 2, P, 16, P)
    w18 = np.stack([w1a, w1b], axis=2)                   # [EL, kp, ab, j, p, pr, m]
    w18 = np.ascontiguousarray(
        w18.transpose(0, 5, 4, 2, 1, 3, 6))              # [EL, pr, p, ab, kp, j, m]
    w28 = _q8w(w2).reshape(EL, 8, 2, P, 16, P) \
        .transpose(0, 4, 3, 1, 2, 5)                     # [EL, oc, p, kp, j, m]
    w28 = np.ascontiguousarray(
        w28.reshape(EL, 4, 4, P, 8, 2, P)
        .transpose(0, 1, 3, 2, 4, 5, 6))                 # [EL, og, p, ocl, kp, j, m]
    wout8 = np.ascontiguousarray(
        _q8w(w_out).reshape(EL, 8, 2, P, 4, P)
        .transpose(0, 3, 1, 4, 2, 5))                    # [EL, p, kp, dc, j, m]

    def bias_t(key, scale, w):
        b = np.asarray(inputs[key], f32)[sl] * scale     # [2, w*128]
        return np.ascontiguousarray(b.reshape(EL, w, P).transpose(2, 0, 1))

    mfeat = np.zeros((D,), f32)
    mfeat[64 * core:64 * core + 64] = 1.0
    smalls = np.concatenate([
        np.ascontiguousarray(mfeat.reshape(4, P).T),
        np.ascontiguousarray((projb * c0).reshape(4, P).T),
        np.broadcast_to(np.asarray(inputs["router_b"], f32), (P, E)),
        projb_col,
        projb_col * c0,
        np.full((P, 1), c0 / S_Y, f32),
        np.broadcast_to(bv, (P, HDIM)),
    ], axis=1).astype(f32)                               # [P, 97]
    bqa = np.zeros((HDIM, 4), f32)
    bqa[:, 0] = bq
    bqa[:, 1] = bk
    bqa[0, 3] = np.asarray(inputs["alpha"], f32)[h]
    wbig = np.concatenate([
        wqkv.reshape(4, P, 192).transpose(1, 0, 2).reshape(P, 768),
        maskd.reshape(P, 4 * D),
        np.asarray(inputs["c_proj_w"], f32)
        .reshape(4, P, D).transpose(1, 0, 2).reshape(P, 4 * D),
        np.asarray(inputs["router_w"], f32)
        .reshape(4, P, E).transpose(1, 0, 2).reshape(P, 4 * E),
    ], axis=1).astype(bf)                                # [P, 4928]
    b1 = bias_t("b1", 1.0, 32)
    ebias = np.concatenate([
        bias_t("b_in", S_H, 16),
        b1[:, :, :16] * S_S,
        b1[:, :, 16:],
        bias_t("b2", S_O, 16),
        bias_t("b_out", 1.0, 4),
    ], axis=2).astype(f32)                               # [P, 2, 68]
    return {
        "xtb": np.ascontiguousarray(
            xT.reshape(4, P, N).transpose(1, 0, 2)).astype(bf),
        "xmask": xmask,
        "smalls": smalls,
        "bqalpha": bqa,
        "wbig": wbig,
        "grow": np.concatenate(
            [g, np.full((HDIM,), np.asarray(inputs["alpha"], f32)[h])]
        ).reshape(1, D + HDIM).astype(bf),
        "selb": selb.astype(bf),
        "w_in8": w_in8, "w18": w18, "w28": w28, "wout8": wout8,
        "ebias": ebias,
    }


last_result = [None]


def kernel(**inputs):
    if "nc" not in _cache:
        _cache["nc"] = build_program()
    nc = _cache["nc"]
    in_maps = [_prep_inputs(inputs, c) for c in range(NCORES)]
    res = run_bass_kernel_spmd(nc, in_maps, core_ids=list(range(NCORES)))
    last_result[0] = res
    outT = np.concatenate(
        [np.asarray(res.results[c]["out"]).astype(np.float32)
         for c in range(NCORES)], axis=0)                # [512, 2048]
    return np.ascontiguousarray(outT.T).reshape(2, 1024, 512)


# revision 31
# speedup vs baseline: 1.0921x; 1.0036x over previous
"""MoE transformer block (QK-norm attention + top-8-of-16 MoE) on 8 trn2 cores.

v4: dense fp8 DoubleRow experts (as v2 baseline) with restructured
scheduling:
- batch-split expert pipeline: each expert runs a b0-pass (tokens 0-1023)
  then a b1-pass, so expert-0's b0 compute starts right after AllGather-0
  and fully hides AllGather-1 + proj-b1;
- the ReduceScatter is split per batch: RS(b0)'s input is complete after
  the last expert's b0-pass and it runs hidden under ~95us of b1 compute,
  leaving only RS(b1) (~18us) in the tail;
- attention-phase cost cuts: softmax denominator folded into an augmented
  v row (kills 24 denominator matmuls), all broadcast matmuls in bf16
  (1 cyc/row instead of 4), RMSNorm's g folded into the broadcast and its
  additive b folded into the qkv biases (host-side);
- SwiGLU's a-path and the moe combine adds alternate DVE/ACT/GpSimd to
  keep the vector engine off the critical path;
- startup DMAs split so RMSNorm starts after the first 0.5MB chunk.

Sharding: attention head-parallel (core c owns head c), experts
expert-parallel (core c owns experts 2c, 2c+1), output feature-parallel
(core c returns features [64c, 64c+64) for all tokens).

Everything feature-major ("T layout": features on partitions, tokens on
free). Scales (powers of two, exact): x_res*2^5, weights*2^11, h1*2^5,
s*2^5, o*2^7; descales folded into Act/DVE scale+bias immediates.
"""

import numpy as np
import ml_dtypes

import concourse.bass as bass
import concourse.mybir as mybir
from concourse.tile import TileContext
from concourse.masks import make_identity
from concourse.bass_utils import run_bass_kernel_spmd

BF16 = mybir.dt.bfloat16
F32 = mybir.dt.float32
F8 = mybir.dt.float8e4
AFT = mybir.ActivationFunctionType
MUL = mybir.AluOpType.mult
ADD = mybir.AluOpType.add
DR = mybir.MatmulPerfMode.DoubleRow

P = 128
D = 512          # embed dim
T = 1024         # tokens per batch
N = 2048         # total tokens
E = 16           # experts
EL = 2           # experts per core
HD = 2048        # expert hidden
HDIM = 64        # head dim
NCORES = 8

S_X = 2.0 ** 5
S_W = 2.0 ** 11
S_H = 2.0 ** 5
S_S = 2.0 ** 5
S_O = 2.0 ** 7
S_Y = 2.0 ** 5

_cache = {}


def build_program():
    nc = bass.Bass()
    dp_ = dict(isOutput=False)
    xtb_d = nc.declare_dram_parameter("xtb", [P, 4, N], BF16, **dp_)
    xm_d = nc.declare_dram_parameter("xmask", [P, 4, N], BF16, **dp_)
    sm_d = nc.declare_dram_parameter("smalls", [P, 97], F32, **dp_)
    bqa_d = nc.declare_dram_parameter("bqalpha", [HDIM, 4], F32, **dp_)
    wbig_d = nc.declare_dram_parameter("wbig", [P, 2880], BF16, **dp_)
    grow_d = nc.declare_dram_parameter("grow", [1, D + HDIM], BF16,
                                       **dp_)
    sel_d = nc.declare_dram_parameter("selb", [E, EL, P], BF16, **dp_)
    win_d = nc.declare_dram_parameter("w_in8", [EL, P, 2, 16, 2, P], F8, **dp_)
    w1_d = nc.declare_dram_parameter("w18", [EL, 16, P, 2, 8, 2, P], F8, **dp_)
    w2_d = nc.declare_dram_parameter("w28", [EL, 4, P, 4, 8, 2, P], F8, **dp_)
    wout_d = nc.declare_dram_parameter("wout8", [EL, P, 8, 4, 2, P], F8, **dp_)
    eb_d = nc.declare_dram_parameter("ebias", [P, EL, 68], F32, **dp_)
    out_d = nc.declare_dram_parameter("out", [HDIM, N], BF16, isOutput=True)

    groups = [list(range(NCORES))]

    with TileContext(nc, num_cores=NCORES) as tc:
        with (
            tc.tile_pool(name="const", bufs=1) as cp,
            tc.tile_pool(name="pp", bufs=1) as pp,
            tc.tile_pool(name="psE", bufs=4, space="PSUM") as psE,
            tc.tile_pool(name="psB", bufs=2, space="PSUM") as psB,
            tc.tile_pool(name="psS", bufs=2, space="PSUM") as psS,
            tc.tile_pool(name="dram", bufs=1, space="DRAM") as dp,
        ):
            # ---- constants / persistent ----
            ws = tc.alloc_tile_pool(name="wst", bufs=1)
            s1pre = tc.alloc_tile_pool(name="s1pre", bufs=1)
            xTb = s1pre.tile([P, 4, N], BF16, tag="xTb")
            nc.sync.dma_start(xTb[:, :, 0:D], xtb_d[:, :, 0:D])
            wbig = s1pre.tile([P, 2880], BF16, tag="wbig")
            nc.sync.dma_start(wbig[:, 0:768], wbig_d[:, 0:768])
            for cc in range(1, 4):
                sl = slice(cc * D, (cc + 1) * D)
                nc.sync.dma_start(xTb[:, :, sl], xtb_d[:, :, sl])
            ones128b = cp.tile([P, 1], BF16, tag="ones128b")
            nc.vector.memset(ones128b, 1.0)
            ones64b = cp.tile([HDIM, 1], BF16, tag="ones64b")
            nc.vector.memset(ones64b, 1.0)
            ones1rb = cp.tile([1, P], BF16, tag="ones1rb")
            nc.vector.memset(ones1rb, 1.0)
            sm = cp.tile([P, 97], F32, tag="sm")
            nc.sync.dma_start(sm, sm_d[:, :])
            mfeat_sb = sm[:, 0:4]
            pbc0_sb = sm[:, 4:8]
            rb_sb = sm[:, 8:24]
            projb_sb = sm[:, 24:28]
            c0_sb = sm[:, 32:33]
            vbias_sb = sm[:, 33:97]
            sel_sb = cp.tile([E, EL, P], BF16, tag="sel_sb")
            nc.sync.dma_start(sel_sb, sel_d[:, :, :])
            eps6 = cp.tile([1, 1], F32, tag="eps6")
            nc.vector.memset(eps6, 1e-6)
            grow = cp.tile([1, D + HDIM], BF16, tag="grow")
            nc.sync.dma_start(grow, grow_d[:, :])
            identb = cp.tile([P, P], BF16, tag="identb")
            make_identity(nc, identb)

            # persistent activations for the expert phase
            moeT = pp.tile([P, 4, N], BF16, tag="moeT")
            xrT8 = pp.tile([P, 4, N], F8, tag="xrT8")
            gatesT = pp.tile([E, N], BF16, tag="gatesT")
            h1T8 = pp.tile([P, 16, N], F8, tag="h1T8")

            ag_in = [dp.tile([HDIM, T], F8, name=f"ag_in{i}")
                     for i in range(2)]
            ag_out = [dp.tile([D, T], F8, addr_space="Shared",
                              name=f"ag_out{i}")
                      for i in range(2)]
            rs_in = [dp.tile([D, T], BF16, name=f"rs_in{i}")
                     for i in range(2)]
            rs_out = [dp.tile([HDIM, T], BF16, name=f"rs_out{i}")
                      for i in range(2)]

            with tc.tile_pool(name="s1", bufs=1) as s1:
                nc.sync.dma_start(wbig[:, 768:2880], wbig_d[:, 768:2880])
                bqa = s1.tile([HDIM, 4], F32, tag="bqa")
                nc.sync.dma_start(bqa, bqa_d[:, :])
                bq_sb = bqa[:, 0:3]
                alpha_sb = bqa[0:1, 3:4]
                # prefetch: expert biases + e0 weights (consumed ~90us later)
                ebt = pp.tile([P, EL, 68], F32, tag="ebias")
                nc.sync.dma_start(ebt, eb_d[:, :, :])
                bias_cp = {"bin": ebt[:, :, 0:16], "b1a": ebt[:, :, 16:32],
                           "b1b": ebt[:, :, 32:48], "b2": ebt[:, :, 48:64],
                           "bout": ebt[:, :, 64:68]}
                pre_win = ws.tile([P, 2, 16, 2, P], F8, tag="win", bufs=1)
                nc.sync.dma_start(pre_win, win_d[0])
                pre_wout = ws.tile([P, 8, 4, 2, P], F8, tag="wot", bufs=1)
                nc.sync.dma_start(pre_wout, wout_d[0])
                pre_w1 = []
                for pr in range(3):
                    t = ws.tile([P, 2, 8, 2, P], F8, tag="w1s", bufs=3)
                    nc.sync.dma_start(t, w1_d[0, pr])
                    pre_w1.append(t)
                # ---- RMSNorm + qkv + qk-norm + attention, batch-ordered:
                # all of batch b's chain runs before batch b+1 so AG(b)
                # issues early and b1 prep fills the AG0 window ----
                with tc.tile_pool(name="attp", bufs=1) as ap_, \
                     tc.tile_pool(name="ate", bufs=12) as ate:
                    xnT = ap_.tile([P, 4, N], BF16, tag="xnT")
                    qT = ap_.tile([HDIM, N], BF16, tag="qT")
                    kT = ap_.tile([HDIM, N], BF16, tag="kT")
                    v_aug = ap_.tile([P, 16, HDIM + 1], BF16, tag="v_aug")
                    nc.vector.memset(v_aug, 1.0)
                    qh = ap_.tile([HDIM, N], BF16, tag="qh")
                    kh = ap_.tile([HDIM, N], BF16, tag="kh")
                    yhT = ap_.tile([HDIM, N], F8, tag="yhT")
                    for b in range(2):
                        for cc in (2 * b, 2 * b + 1):
                            sl = slice(cc * D, (cc + 1) * D)
                            ps = psS.tile([1, D], F32, tag="ps_small")
                            for kc in range(4):
                                sq = ate.tile([P, D], BF16, tag="sq_t",
                                              bufs=3)
                                nc.scalar.activation(sq, xTb[:, kc, sl],
                                                     AFT.Square)
                                nc.tensor.matmul(ps, ones128b, sq,
                                                 start=(kc == 0),
                                                 stop=(kc == 3))
                            tmp = ate.tile([1, D], F32, tag="r_t", bufs=2)
                            nc.scalar.activation(tmp, ps, AFT.Sqrt,
                                                 scale=1.0 / D,
                                                 bias=eps6[0:1, 0:1])
                            rrow = ate.tile([1, D], BF16, tag="rrow",
                                            bufs=2)
                            with nc.allow_low_precision(
                                    reason="bf16 bcast row"):
                                nc.vector.reciprocal(rrow, tmp)
                            # xnT = xTb * bcast(rrow * g)
                            for kc in range(4):
                                pb = psB.tile([P, D], F32, tag="bc")
                                nc.tensor.matmul(
                                    pb, grow[0:1, kc * P:(kc + 1) * P],
                                    rrow[0:1, :], start=True, stop=True)
                                nc.vector.tensor_mul(xnT[:, kc, sl],
                                                     xTb[:, kc, sl], pb)
                            for wi, dst, bi in ((0, qT, 0), (1, kT, 1)):
                                ps2 = psS.tile([HDIM, D], F32,
                                               tag="ps_small")
                                for kc in range(4):
                                    nc.tensor.matmul(
                                        ps2,
                                        wbig[:, kc * 192 + wi * HDIM:
                                             kc * 192 + (wi + 1) * HDIM],
                                        xnT[:, kc, sl], start=(kc == 0),
                                        stop=(kc == 3))
                                nc.scalar.activation(
                                    dst[:, sl], ps2, AFT.Identity,
                                    bias=bq_sb[:, bi:bi + 1])
                            for tk in range(cc * 4, cc * 4 + 4):
                                ps3 = psS.tile([P, HDIM], F32,
                                               tag="ps_small")
                                for kc in range(4):
                                    nc.tensor.matmul(
                                        ps3, xnT[:, kc, tk * P:(tk + 1) * P],
                                        wbig[:, kc * 192 + 128:
                                             kc * 192 + 192],
                                        start=(kc == 0), stop=(kc == 3))
                                nc.vector.tensor_add(v_aug[:, tk, 0:HDIM],
                                                     ps3, vbias_sb)
                            # qk-norm for this chunk (alpha folded in the
                            # q bcast row)
                            for src_, dst, brow in (
                                    (qT, qh, grow[0:1, D:D + HDIM]),
                                    (kT, kh, ones1rb[0:1, 0:HDIM])):
                                sq = ate.tile([HDIM, D], BF16, tag="sqn",
                                              bufs=2)
                                nc.scalar.activation(sq, src_[:, sl],
                                                     AFT.Square)
                                ps4 = psS.tile([1, D], F32, tag="ps_small")
                                nc.tensor.matmul(ps4, ones64b, sq,
                                                 start=True, stop=True)
                                t = ate.tile([1, D], F32, tag="rn_t",
                                             bufs=2)
                                nc.scalar.activation(t, ps4, AFT.Sqrt)
                                nc.vector.tensor_scalar_add(t, t, 1e-5)
                                rn = ate.tile([1, D], BF16, tag="rn",
                                              bufs=2)
                                with nc.allow_low_precision(
                                        reason="bf16 row"):
                                    nc.vector.reciprocal(rn, t)
                                pb = psB.tile([HDIM, D], F32, tag="bc")
                                nc.tensor.matmul(pb, brow, rn[0:1, :],
                                                 start=True, stop=True)
                                nc.vector.tensor_mul(dst[:, sl],
                                                     src_[:, sl], pb)
                        # scoresT -> exp (masked diag) -> AV (denominator
                        # folded into v_aug's ones row) -> yhT -> AG(b)
                        for qc in range(2):
                            qsl = slice(b * T + qc * D, b * T + (qc + 1) * D)
                            nkc = 4 * qc + 4
                            ex_tiles = []
                            for kc in range(nkc):
                                ksl = slice(b * T + kc * P,
                                            b * T + (kc + 1) * P)
                                ps = psE.tile([P, D], F32, tag="mm")
                                nc.tensor.matmul(ps, kh[:, ksl], qh[:, qsl],
                                                 start=True, stop=True)
                                if kc >= 4 * qc:  # diagonal block: mask
                                    et = ate.tile([P, D], BF16, tag="exp_b",
                                                  bufs=4)
                                    nc.scalar.activation(et, ps, AFT.Exp)
                                    eb2 = ate.tile([P, D], BF16,
                                                   tag="exp_m", bufs=7)
                                    nc.vector.tensor_mul(
                                        eb2, et,
                                        wbig[:, 768 + (kc - 4 * qc) * D:
                                             768 + (kc - 4 * qc + 1) * D])
                                else:
                                    eb2 = ate.tile([P, D], BF16,
                                                   tag="exp_m", bufs=7)
                                    nc.scalar.activation(eb2, ps, AFT.Exp)
                                ex_tiles.append(eb2)
                            py = psS.tile([HDIM + 1, D], F32,
                                          tag="ps_small")
                            for kc in range(nkc):
                                nc.tensor.matmul(py, v_aug[:, b * 8 + kc, :],
                                                 ex_tiles[kc],
                                                 start=(kc == 0),
                                                 stop=(kc == nkc - 1))
                            dr = ate.tile([1, D], BF16, tag="dr", bufs=2)
                            with nc.allow_low_precision(
                                    reason="bf16 softmax denom"):
                                nc.vector.reciprocal(dr,
                                                     py[HDIM:HDIM + 1, :])
                            pb2 = psB.tile([HDIM, D], F32, tag="bc")
                            nc.tensor.matmul(pb2, ones1rb[0:1, 0:HDIM], dr,
                                             start=True, stop=True)
                            db = ate.tile([HDIM, D], BF16, tag="db", bufs=2)
                            nc.scalar.activation(db, pb2, AFT.Copy,
                                                 scale=S_Y)
                            nc.vector.tensor_mul(yhT[:, qsl],
                                                 py[0:HDIM, :], db)
                            nc.sync.dma_start(
                                ag_in[b][:, qc * D:(qc + 1) * D],
                                yhT[:, qsl])
                        nc.gpsimd.collective_compute(
                            "AllGather", mybir.AluOpType.bypass,
                            ins=[ag_in[b][:]], outs=[ag_out[b][:]],
                            replica_groups=groups)

                # ---- proj + x_res + router (per batch), interleaved with
                # expert passes so weight DMAs never queue behind the
                # AG1-blocked b1 input DMAs ----
                pass
            # (s1 stays open: xTb / qT / kT / v_aug / wbig / bqa)
            with tc.tile_pool(name="s2", bufs=1) as s2, \
                 tc.tile_pool(name="s2e", bufs=4) as s2e, \
                 tc.tile_pool(name="eact", bufs=1) as ac, \
                 tc.tile_pool(name="eev", bufs=6) as ev_:
                yT_sb = s2.tile([P, 4, N], F8, tag="yT_sb")
                routes = s2.tile([P, 16, E], F32, tag="routes")
                gates = s2.tile([P, 16, E], F32, tag="gates")
                gsum = s2.tile([P, 16], F32, tag="gsum")
                gates_bf = s2.tile([P, 16, E], BF16, tag="gates_bf")

                def emit_batch(b):
                    for ts in (2 * b, 2 * b + 1):
                        nc.sync.dma_start(
                            yT_sb[:, :, ts * D:(ts + 1) * D],
                            ag_out[b][:, (ts % 2) * D:(ts % 2 + 1) * D]
                            .rearrange("(kc p) n -> p kc n", p=P))
                    for ts in (2 * b, 2 * b + 1):
                        for dc in range(4):
                            tsl = slice(ts * D, (ts + 1) * D)
                            ps = psE.tile([P, D], F32, tag="mm")
                            for kc in range(4):
                                nc.tensor.matmul(
                                    ps,
                                    wbig[:, 2816 + kc * D + dc * P:
                                         2816 + kc * D + (dc + 1) * P],
                                    yT_sb[:, kc, tsl],
                                    start=(kc == 0), stop=(kc == 3))
                            # xmb = x*featmask + projb*c0 (replaces the
                            # host xmask tensor)
                            xmb = s2e.tile([P, D], BF16, tag="xmb", bufs=3)
                            nc.vector.tensor_scalar(
                                xmb, xTb[:, dc, tsl],
                                mfeat_sb[:, dc:dc + 1],
                                pbc0_sb[:, dc:dc + 1], op0=MUL, op1=ADD)
                            tmp = s2e.tile([P, D], F32, tag="yp_t", bufs=3)
                            nc.scalar.activation(
                                tmp, ps, AFT.Identity, scale=1.0 / S_Y,
                                bias=projb_sb[:, dc:dc + 1])
                            # x_res written in place over xTb
                            nc.vector.tensor_add(xTb[:, dc, tsl], tmp,
                                                 xTb[:, dc, tsl])
                            nc.scalar.activation(xrT8[:, dc, tsl],
                                                 xTb[:, dc, tsl],
                                                 AFT.Copy, scale=S_X)
                            # moeT init: yp*c0 + x feature slice
                            nc.vector.scalar_tensor_tensor(
                                moeT[:, dc, tsl], ps, c0_sb[:, 0:1],
                                xmb, op0=MUL, op1=ADD)
                def emit_gates(b):
                    # router for this batch -> normalized top-8 gates
                    for tk in range(8 * b, 8 * b + 8):
                        ps = psS.tile([P, E], F32, tag="ps_small")
                        for kc in range(4):
                            nc.tensor.matmul(
                                ps, xTb[:, kc, tk * P:(tk + 1) * P],
                                wbig[:, 2816 + kc * E:2816 + (kc + 1) * E],
                                start=(kc == 0), stop=(kc == 3))
                        nc.vector.tensor_add(routes[:, tk, :], ps, rb_sb)
                    nc.scalar.activation(routes[:, 8 * b:8 * b + 8, :],
                                         routes[:, 8 * b:8 * b + 8, :],
                                         AFT.Exp)
                    for g in range(8 * b, 8 * b + 8):
                        m8 = s2e.tile([P, 8], F32, tag="m8", bufs=2)
                        nc.vector.max(out=m8, in_=routes[:, g, :])
                        zap = s2e.tile([P, E], F32, tag="zap", bufs=2)
                        nc.vector.match_replace(out=zap, in_to_replace=m8,
                                                in_values=routes[:, g, :],
                                                imm_value=0)
                        nc.vector.tensor_sub(gates[:, g, :],
                                             routes[:, g, :], zap)
                    nc.vector.reduce_sum(gsum[:, 8 * b:8 * b + 8],
                                         gates[:, 8 * b:8 * b + 8, :],
                                         axis=mybir.AxisListType.X)
                    nc.vector.reciprocal(gsum[:, 8 * b:8 * b + 8],
                                         gsum[:, 8 * b:8 * b + 8])
                    for g in range(8 * b, 8 * b + 8):
                        nc.vector.tensor_scalar_mul(gates_bf[:, g, :],
                                                    gates[:, g, :],
                                                    gsum[:, g:g + 1])
                    for g in range(8 * b, 8 * b + 8):
                        pt = psS.tile([E, P], BF16, tag="ps_small")
                        nc.tensor.transpose(pt, gates_bf[:, g, :], identb)
                        nc.scalar.activation(
                            gatesT[:, g * P:(g + 1) * P], pt, AFT.Copy)

                held = {}

                def emit_pass(e, half, post_h1=None):
                    ts_range = (2 * half, 2 * half + 1)
                    if half == 0:
                        if e == 0:
                            win_t = pre_win
                            wout_t = pre_wout
                        else:
                            win_t = ws.tile([P, 2, 16, 2, P], F8,
                                            tag="win", bufs=1, name="win_t")
                            nc.sync.dma_start(win_t, win_d[e])
                            wout_t = ws.tile([P, 8, 4, 2, P], F8,
                                             tag="wot", bufs=1,
                                             name="wout_t")
                            nc.sync.dma_start(wout_t, wout_d[e])
                        held[e] = (win_t, wout_t)
                    else:
                        win_t, wout_t = held[e]
                    # h1 = x_res @ w_in  (fp8, S_H)
                    for ts in ts_range:
                        for hc in range(16):
                            tsl = slice(ts * D, (ts + 1) * D)
                            ps = psE.tile([P, D], F32, tag="mm")
                            for kp in range(2):
                                nc.tensor.matmul(
                                    ps, win_t[:, kp, hc, :, :],
                                    xrT8[:, 2 * kp:2 * kp + 2, tsl],
                                    start=(kp == 0), stop=(kp == 1),
                                    perf_mode=DR)
                            if (hc + ts) % 2 == 0:
                                nc.scalar.activation(
                                    h1T8[:, hc, tsl], ps, AFT.Identity,
                                    scale=S_H / (S_X * S_W),
                                    bias=bias_cp["bin"][:, e, hc:hc + 1])
                            else:
                                nc.vector.tensor_scalar(
                                    h1T8[:, hc, tsl], ps,
                                    S_H / (S_X * S_W),
                                    bias_cp["bin"][:, e, hc:hc + 1],
                                    op0=MUL, op1=ADD)
                    if post_h1 is not None:
                        post_h1()
                    # SwiGLU: s = silu(h@w1b + b1b) * (h@w1a + b1a)
                    sT8 = ac.tile([P, 16, T], F8, tag="sT8", bufs=1,
                                  name="sT8")
                    for pr in range(16):
                        if e == 0 and half == 0 and pr < 3:
                            w1_t = pre_w1[pr]
                        else:
                            w1_t = ws.tile([P, 2, 8, 2, P], F8, tag="w1s",
                                           bufs=3, name="w1_t")
                            nc.sync.dma_start(w1_t, w1_d[e, pr])
                        for ts in ts_range:
                            tsl = slice(ts * D, (ts + 1) * D)
                            pa = psE.tile([P, D], F32, tag="mm")
                            pb = psE.tile([P, D], F32, tag="mm")
                            for kp in range(8):
                                nc.tensor.matmul(
                                    pa, w1_t[:, 0, kp, :, :],
                                    h1T8[:, 2 * kp:2 * kp + 2, tsl],
                                    start=(kp == 0), stop=(kp == 7),
                                    perf_mode=DR)
                            for kp in range(8):
                                nc.tensor.matmul(
                                    pb, w1_t[:, 1, kp, :, :],
                                    h1T8[:, 2 * kp:2 * kp + 2, tsl],
                                    start=(kp == 0), stop=(kp == 7),
                                    perf_mode=DR)
                            sil = ev_.tile([P, D], BF16, tag="sil", bufs=4)
                            nc.scalar.activation(
                                sil, pb, AFT.Silu,
                                scale=1.0 / (S_H * S_W),
                                bias=bias_cp["b1b"][:, e, pr:pr + 1])
                            av8 = ev_.tile([P, D], F8, tag="av8", bufs=4)
                            if (pr + ts) % 2 == 0:
                                nc.scalar.activation(
                                    av8, pa, AFT.Identity,
                                    scale=S_S / (S_H * S_W),
                                    bias=bias_cp["b1a"][:, e, pr:pr + 1])
                            else:
                                nc.vector.tensor_scalar(
                                    av8, pa, S_S / (S_H * S_W),
                                    bias_cp["b1a"][:, e, pr:pr + 1],
                                    op0=MUL, op1=ADD)
                            ltsl = slice((ts - 2 * half) * D,
                                         (ts - 2 * half + 1) * D)
                            nc.vector.tensor_mul(sT8[:, pr, ltsl], av8, sil)
                    # o = s @ w2 + b2  (fp8, S_O)
                    oT8 = ac.tile([P, 16, T], F8, tag="oT8", bufs=1,
                                  name="oT8")
                    for og in range(4):
                        w2_t = ws.tile([P, 4, 8, 2, P], F8, tag="w2s",
                                       bufs=2, name="w2_t")
                        nc.sync.dma_start(w2_t, w2_d[e, og])
                        for oc4 in range(4):
                            oc = og * 4 + oc4
                            for ts in ts_range:
                                ltsl = slice((ts - 2 * half) * D,
                                             (ts - 2 * half + 1) * D)
                                ps = psE.tile([P, D], F32, tag="mm")
                                for kp in range(8):
                                    nc.tensor.matmul(
                                        ps, w2_t[:, oc4, kp, :, :],
                                        sT8[:, 2 * kp:2 * kp + 2, ltsl],
                                        start=(kp == 0), stop=(kp == 7),
                                        perf_mode=DR)
                                if (oc + ts) % 2 == 0:
                                    nc.scalar.activation(
                                        oT8[:, oc, ltsl], ps, AFT.Identity,
                                        scale=S_O / (S_S * S_W),
                                        bias=bias_cp["b2"][:, e, oc:oc + 1])
                                else:
                                    nc.vector.tensor_scalar(
                                        oT8[:, oc, ltsl], ps,
                                        S_O / (S_S * S_W),
                                        bias_cp["b2"][:, e, oc:oc + 1],
                                        op0=MUL, op1=ADD)
                    # gate broadcast [tokens] -> [P, D] per ts chunk
                    gb_tiles = {}
                    for ts in ts_range:
                        tsl = slice(ts * D, (ts + 1) * D)
                        pg = psB.tile([P, D], F32, tag="bc")
                        nc.tensor.matmul(pg, sel_sb[:, e, :], gatesT[:, tsl],
                                         start=True, stop=True)
                        gb = ev_.tile([P, D], BF16, tag="gb", bufs=4)
                        nc.scalar.activation(gb, pg, AFT.Copy)
                        gb_tiles[ts] = gb
                    # eo = o @ w_out + b_out; moeT += gate * eo
                    for dc in range(4):
                        for ts in ts_range:
                            tsl = slice(ts * D, (ts + 1) * D)
                            ltsl = slice((ts - 2 * half) * D,
                                         (ts - 2 * half + 1) * D)
                            ps = psE.tile([P, D], F32, tag="mm")
                            for kp in range(8):
                                nc.tensor.matmul(
                                    ps, wout_t[:, kp, dc, :, :],
                                    oT8[:, 2 * kp:2 * kp + 2, ltsl],
                                    start=(kp == 0), stop=(kp == 7),
                                    perf_mode=DR)
                            eo = ev_.tile([P, D], F32, tag="eo", bufs=3)
                            nc.scalar.activation(
                                eo, ps, AFT.Identity,
                                scale=1.0 / (S_O * S_W),
                                bias=bias_cp["bout"][:, e, dc:dc + 1])
                            t2 = ev_.tile([P, D], F32, tag="t2", bufs=3)
                            nc.vector.tensor_mul(t2, eo, gb_tiles[ts])
                            if (dc + ts) % 2 == 0:
                                nc.vector.tensor_add(moeT[:, dc, tsl],
                                                     moeT[:, dc, tsl], t2)
                            else:
                                nc.gpsimd.tensor_add(moeT[:, dc, tsl],
                                                     moeT[:, dc, tsl], t2)
                            if e == 1:
                                nc.sync.dma_start(
                                    rs_in[half][dc * P:(dc + 1) * P,
                                                (ts - 2 * half) * D:
                                                (ts - 2 * half + 1) * D],
                                    moeT[:, dc, tsl])
                    if e == 1:
                        nc.gpsimd.collective_compute(
                            "ReduceScatter", mybir.AluOpType.add,
                            ins=[rs_in[half][:]], outs=[rs_out[half][:]],
                            replica_groups=groups)
                        nc.sync.dma_start(
                            out_d[:, half * T:(half + 1) * T],
                            rs_out[half][:, :])

                emit_batch(0)
                emit_pass(0, 0, post_h1=lambda: emit_gates(0))
                emit_batch(1)
                emit_pass(0, 1, post_h1=lambda: emit_gates(1))
                emit_pass(1, 0)
                emit_pass(1, 1)
            s1pre.release()

            ws.release()

    _split_matmul_waits(nc)
    return nc


def _split_matmul_waits(nc):
    """walrus allows only one sync-wait per engine-instruction sync slot; move
    extra waits onto standalone InstEventSemaphore waits inserted before."""
    import concourse.mybir as mybir
    k = 0
    for bb in nc.main_func.blocks:
        il = list(bb.instructions)
        out = []
        changed = False
        for ins in il:
            si = getattr(ins, "sync_info", None)
            if si is not None and len(si.on_wait) > 1 \
                    and type(ins).__name__ != "InstEventSemaphore":
                waits = list(si.on_wait)
                keep, move = waits[-1], waits[:-1]
                for w in move:
                    nop = mybir.InstEventSemaphore(name=f"I-wsplit-{k}",
                                                   ins=[], outs=[])
                    k += 1
                    nop.engine = ins.engine
                    nop.sync_info = type(si)(on_wait=[w], on_update=[])
                    out.append(nop)
                ins.sync_info = type(si)(on_wait=[keep],
                                         on_update=list(si.on_update))
                changed = True
            out.append(ins)
        if changed:
            bb.instructions = out


def _q8w(w):
    """host fp8 cast with fixed 2^11 scale (clipped to TRN e4m3 max)."""
    return np.clip(np.asarray(w, np.float32) * S_W, -240.0, 240.0).astype(
        ml_dtypes.float8_e4m3)


def _prep_inputs(inputs, core):
    bf = ml_dtypes.bfloat16
    f32 = np.float32
    h = core
    x = np.asarray(inputs["x"], f32).reshape(N, D)
    xT = np.ascontiguousarray(x.T)                      # [512, 2048]
    g = np.asarray(inputs["g"], f32)
    bvec = np.asarray(inputs["b"], f32)
    caw = np.asarray(inputs["c_attn_w"], f32)
    cab = np.asarray(inputs["c_attn_b"], f32)
    wq = caw[:, h * 64:(h + 1) * 64]
    wk = caw[:, 512 + h * 64:512 + (h + 1) * 64]
    wv = caw[:, 1024 + h * 64:1024 + (h + 1) * 64]
    wqkv = np.concatenate([wq, wk, wv], axis=1)          # [512, 192]
    # RMSNorm additive b folded into qkv biases
    bq = bvec @ wq + cab[h * 64:(h + 1) * 64]
    bk = bvec @ wk + cab[512 + h * 64:512 + (h + 1) * 64]
    bv = bvec @ wv + cab[1024 + h * 64:1024 + (h + 1) * 64]
    kk = np.arange(4)[None, :, None] * P + np.arange(P)[:, None, None]
    qq = np.arange(D)[None, None, :]
    maskd = (kk <= qq).astype(f32)                       # [128, 4, 512]
    projb = np.asarray(inputs["c_proj_b"], f32)
    projb_col = np.ascontiguousarray(projb.reshape(4, P).T)  # [p, dc]
    c0 = 1.0 if core == 0 else 0.0
    xmask = np.zeros((D, N), f32)
    xmask[64 * core:64 * core + 64, :] = xT[64 * core:64 * core + 64, :]
    xmask += (projb * c0)[:, None]
    xmask = np.ascontiguousarray(
        xmask.reshape(4, P, N).transpose(1, 0, 2)).astype(bf)
    selb = np.zeros((E, EL, P), f32)
    selb[2 * core, 0, :] = 1.0
    selb[2 * core + 1, 1, :] = 1.0

    sl = slice(2 * core, 2 * core + 2)
    w_in = np.asarray(inputs["w_in"], f32)[sl]           # [2, 512, 2048]
    w1 = np.asarray(inputs["w1"], f32)[sl]               # [2, 2048, 4096]
    w2 = np.asarray(inputs["w2"], f32)[sl]               # [2, 2048, 2048]
    w_out = np.asarray(inputs["w_out"], f32)[sl]         # [2, 2048, 512]

    # DoubleRow lhsT layouts (pair index j adjacent to the 128-wide m dim)
    w_in8 = np.ascontiguousarray(
        _q8w(w_in).reshape(EL, 2, 2, P, 16, P)
        .transpose(0, 3, 1, 4, 2, 5))                    # [EL, p, kp, hc, j, m]
    w1a = _q8w(w1[:, :, :HD]).reshape(EL, 8, 2, P, 16, P)
    w1b = _q8w(w1[:, :, HD:]).reshape(EL, 8, 2, P, 16, P)
    w18 = np.stack([w1a, w1b], axis=2)                   # [EL, kp, ab, j, p, pr, m]
    w18 = np.ascontiguousarray(
        w18.transpose(0, 5, 4, 2, 1, 3, 6))              # [EL, pr, p, ab, kp, j, m]
    w28 = _q8w(w2).reshape(EL, 8, 2, P, 16, P) \
        .transpose(0, 4, 3, 1, 2, 5)                     # [EL, oc, p, kp, j, m]
    w28 = np.ascontiguousarray(
        w28.reshape(EL, 4, 4, P, 8, 2, P)
        .transpose(0, 1, 3, 2, 4, 5, 6))                 # [EL, og, p, ocl, kp, j, m]
    wout8 = np.ascontiguousarray(
        _q8w(w_out).reshape(EL, 8, 2, P, 4, P)
        .transpose(0, 3, 1, 4, 2, 5))                    # [EL, p, kp, dc, j, m]

    def bias_t(key, scale, w):
        b = np.asarray(inputs[key], f32)[sl] * scale     # [2, w*128]
        return np.ascontiguousarray(b.reshape(EL, w, P).transpose(2, 0, 1))

    mfeat = np.zeros((D,), f32)
    mfeat[64 * core:64 * core + 64] = 1.0
    smalls = np.concatenate([
        np.ascontiguousarray(mfeat.reshape(4, P).T),
        np.ascontiguousarray((projb * c0).reshape(4, P).T),
        np.broadcast_to(np.asarray(inputs["router_b"], f32), (P, E)),
        projb_col,
        projb_col * c0,
        np.full((P, 1), c0 / S_Y, f32),
        np.broadcast_to(bv, (P, HDIM)),
    ], axis=1).astype(f32)                               # [P, 97]
    bqa = np.zeros((HDIM, 4), f32)
    bqa[:, 0] = bq
    bqa[:, 1] = bk
    bqa[0, 3] = np.asarray(inputs["alpha"], f32)[h]
    wbig = np.concatenate([
        wqkv.reshape(4, P, 192).transpose(1, 0, 2).reshape(P, 768),
        maskd.reshape(P, 4 * D),
        np.asarray(inputs["router_w"], f32)
        .reshape(4, P, E).transpose(1, 0, 2).reshape(P, 4 * E),
    ], axis=1).astype(bf)                                # [P, 2880]
    b1 = bias_t("b1", 1.0, 32)
    ebias = np.concatenate([
        bias_t("b_in", S_H, 16),
        b1[:, :, :16] * S_S,
        b1[:, :, 16:],
        bias_t("b2", S_O, 16),
        bias_t("b_out", 1.0, 4),
    ], axis=2).astype(f32)                               # [P, 2, 68]
    return {
        "xtb": np.ascontiguousarray(
            xT.reshape(4, P, N).transpose(1, 0, 2)).astype(bf),
        "xmask": xmask,
        "smalls": smalls,
        "bqalpha": bqa,
        "wbig": wbig,
        "grow": np.concatenate(
            [g, np.full((HDIM,), np.asarray(inputs["alpha"], f32)[h])]
        ).reshape(1, D + HDIM).astype(bf),
        "selb": selb.astype(bf),
        "w_in8": w_in8, "w18": w18, "w28": w28, "wout8": wout8,
        "ebias": ebias,
    }


last_result = [None]


def kernel(**inputs):
    if "nc" not in _cache:
        _cache["nc"] = build_program()
    nc = _cache["nc"]
    in_maps = [_prep_inputs(inputs, c) for c in range(NCORES)]
    res = run_bass_kernel_spmd(nc, in_maps, core_ids=list(range(NCORES)))
    last_result[0] = res
    outT = np.concatenate(
        [np.asarray(res.results[c]["out"]).astype(np.float32)
         for c in range(NCORES)], axis=0)                # [512, 2048]
    return np.ascontiguousarray(outT.T).reshape(2, 1024, 512)


# revision 42
# speedup vs baseline: 1.0926x; 1.0004x over previous
"""MoE transformer block (QK-norm attention + top-8-of-16 MoE) on 8 trn2 cores.

v4: dense fp8 DoubleRow experts (as v2 baseline) with restructured
scheduling:
- batch-split expert pipeline: each expert runs a b0-pass (tokens 0-1023)
  then a b1-pass, so expert-0's b0 compute starts right after AllGather-0
  and fully hides AllGather-1 + proj-b1;
- the ReduceScatter is split per batch: RS(b0)'s input is complete after
  the last expert's b0-pass and it runs hidden under ~95us of b1 compute,
  leaving only RS(b1) (~18us) in the tail;
- attention-phase cost cuts: softmax denominator folded into an augmented
  v row (kills 24 denominator matmuls), all broadcast matmuls in bf16
  (1 cyc/row instead of 4), RMSNorm's g folded into the broadcast and its
  additive b folded into the qkv biases (host-side);
- SwiGLU's a-path and the moe combine adds alternate DVE/ACT/GpSimd to
  keep the vector engine off the critical path;
- startup DMAs split so RMSNorm starts after the first 0.5MB chunk.

Sharding: attention head-parallel (core c owns head c), experts
expert-parallel (core c owns experts 2c, 2c+1), output feature-parallel
(core c returns features [64c, 64c+64) for all tokens).

Everything feature-major ("T layout": features on partitions, tokens on
free). Scales (powers of two, exact): x_res*2^5, weights*2^11, h1*2^5,
s*2^5, o*2^7; descales folded into Act/DVE scale+bias immediates.
"""

import numpy as np
import ml_dtypes

import concourse.bass as bass
import concourse.mybir as mybir
from concourse.tile import TileContext
from concourse.masks import make_identity
from concourse.bass_utils import run_bass_kernel_spmd

BF16 = mybir.dt.bfloat16
F32 = mybir.dt.float32
F8 = mybir.dt.float8e4
AFT = mybir.ActivationFunctionType
MUL = mybir.AluOpType.mult
ADD = mybir.AluOpType.add
DR = mybir.MatmulPerfMode.DoubleRow

P = 128
D = 512          # embed dim
T = 1024         # tokens per batch
N = 2048         # total tokens
E = 16           # experts
EL = 2           # experts per core
HD = 2048        # expert hidden
HDIM = 64        # head dim
NCORES = 8

S_X = 2.0 ** 5
S_W = 2.0 ** 11
S_H = 2.0 ** 5
S_S = 2.0 ** 5
S_O = 2.0 ** 7
S_Y = 2.0 ** 5

_cache = {}


def build_program():
    nc = bass.Bass()
    dp_ = dict(isOutput=False)
    xtb_d = nc.declare_dram_parameter("xtb", [P, 4, N], BF16, **dp_)
    xm_d = nc.declare_dram_parameter("xmask", [P, 4, N], BF16, **dp_)
    sm_d = nc.declare_dram_parameter("smalls", [P, 97], F32, **dp_)
    bqa_d = nc.declare_dram_parameter("bqalpha", [HDIM, 4], F32, **dp_)
    wbig_d = nc.declare_dram_parameter("wbig", [P, 2880], BF16, **dp_)
    grow_d = nc.declare_dram_parameter("grow", [1, D + HDIM], BF16,
                                       **dp_)
    sel_d = nc.declare_dram_parameter("selb", [E, EL, P], BF16, **dp_)
    win_d = nc.declare_dram_parameter("w_in8", [EL, P, 2, 16, 2, P], F8, **dp_)
    w1_d = nc.declare_dram_parameter("w18", [EL, 16, P, 2, 8, 2, P], F8, **dp_)
    w2_d = nc.declare_dram_parameter("w28", [EL, 4, P, 4, 8, 2, P], F8, **dp_)
    wout_d = nc.declare_dram_parameter("wout8", [EL, P, 8, 4, 2, P], F8, **dp_)
    eb_d = nc.declare_dram_parameter("ebias", [P, EL, 68], F32, **dp_)
    out_d = nc.declare_dram_parameter("out", [HDIM, N], BF16, isOutput=True)

    groups = [list(range(NCORES))]

    with TileContext(nc, num_cores=NCORES) as tc:
        with (
            tc.tile_pool(name="const", bufs=1) as cp,
            tc.tile_pool(name="pp", bufs=1) as pp,
            tc.tile_pool(name="psE", bufs=4, space="PSUM") as psE,
            tc.tile_pool(name="psB", bufs=2, space="PSUM") as psB,
            tc.tile_pool(name="psS", bufs=2, space="PSUM") as psS,
            tc.tile_pool(name="dram", bufs=1, space="DRAM") as dp,
        ):
            # ---- constants / persistent ----
            ws = tc.alloc_tile_pool(name="wst", bufs=1)
            s1pre = tc.alloc_tile_pool(name="s1pre", bufs=1)
            xTb = s1pre.tile([P, 4, N], BF16, tag="xTb")
            nc.sync.dma_start(xTb[:, :, 0:D], xtb_d[:, :, 0:D])
            wbig = s1pre.tile([P, 2880], BF16, tag="wbig")
            nc.sync.dma_start(wbig[:, 0:768], wbig_d[:, 0:768])
            for cc in range(1, 4):
                sl = slice(cc * D, (cc + 1) * D)
                nc.sync.dma_start(xTb[:, :, sl], xtb_d[:, :, sl])
            ones128b = cp.tile([P, 1], BF16, tag="ones128b")
            nc.vector.memset(ones128b, 1.0)
            ones64b = cp.tile([HDIM, 1], BF16, tag="ones64b")
            nc.vector.memset(ones64b, 1.0)
            ones1rb = cp.tile([1, P], BF16, tag="ones1rb")
            nc.vector.memset(ones1rb, 1.0)
            sm = cp.tile([P, 97], F32, tag="sm")
            nc.sync.dma_start(sm, sm_d[:, :])
            mfeat_sb = sm[:, 0:4]
            pbc0_sb = sm[:, 4:8]
            rb_sb = sm[:, 8:24]
            projb_sb = sm[:, 24:28]
            c0_sb = sm[:, 32:33]
            vbias_sb = sm[:, 33:97]
            sel_sb = cp.tile([E, EL, P], BF16, tag="sel_sb")
            nc.sync.dma_start(sel_sb, sel_d[:, :, :])
            eps6 = cp.tile([1, 1], F32, tag="eps6")
            nc.vector.memset(eps6, 1e-6)
            grow = cp.tile([1, D + HDIM], BF16, tag="grow")
            nc.sync.dma_start(grow, grow_d[:, :])
            identb = cp.tile([P, P], BF16, tag="identb")
            make_identity(nc, identb)

            # persistent activations for the expert phase
            moeT = pp.tile([P, 4, N], BF16, tag="moeT")
            xrT8 = pp.tile([P, 4, N], F8, tag="xrT8")
            gatesT = pp.tile([E, N], BF16, tag="gatesT")
            h1T8 = pp.tile([P, 16, N], F8, tag="h1T8")

            ag_in = [dp.tile([HDIM, T], F8, name=f"ag_in{i}")
                     for i in range(2)]
            ag_out = [dp.tile([D, T], F8, addr_space="Shared",
                              name=f"ag_out{i}")
                      for i in range(2)]
            rs_in = [dp.tile([D, T], BF16, name=f"rs_in{i}")
                     for i in range(2)]
            rs_out = [dp.tile([HDIM, T], BF16, name=f"rs_out{i}")
                      for i in range(2)]

            with tc.tile_pool(name="s1", bufs=1) as s1:
                nc.sync.dma_start(wbig[:, 768:2880], wbig_d[:, 768:2880])
                bqa = s1.tile([HDIM, 4], F32, tag="bqa")
                nc.sync.dma_start(bqa, bqa_d[:, :])
                bq_sb = bqa[:, 0:3]
                alpha_sb = bqa[0:1, 3:4]
                # prefetch: expert biases + e0 weights (consumed ~90us later)
                ebt = pp.tile([P, EL, 68], F32, tag="ebias")
                nc.sync.dma_start(ebt, eb_d[:, :, :])
                bias_cp = {"bin": ebt[:, :, 0:16], "b1a": ebt[:, :, 16:32],
                           "b1b": ebt[:, :, 32:48], "b2": ebt[:, :, 48:64],
                           "bout": ebt[:, :, 64:68]}
                pre_win = ws.tile([P, 2, 16, 2, P], F8, tag="win", bufs=1)
                nc.sync.dma_start(pre_win, win_d[0])
                pre_wout = ws.tile([P, 8, 4, 2, P], F8, tag="wot", bufs=1)
                nc.sync.dma_start(pre_wout, wout_d[0])
                pre_w1 = []
                for pr in range(3):
                    t = ws.tile([P, 2, 8, 2, P], F8, tag="w1s", bufs=4)
                    nc.sync.dma_start(t, w1_d[0, pr])
                    pre_w1.append(t)
                # ---- RMSNorm + qkv + qk-norm + attention, batch-ordered:
                # all of batch b's chain runs before batch b+1 so AG(b)
                # issues early and b1 prep fills the AG0 window ----
                with tc.tile_pool(name="attp", bufs=1) as ap_, \
                     tc.tile_pool(name="ate", bufs=12) as ate:
                    xnT = ap_.tile([P, 4, N], BF16, tag="xnT")
                    qT = ap_.tile([HDIM, N], BF16, tag="qT")
                    kT = ap_.tile([HDIM, N], BF16, tag="kT")
                    v_aug = ap_.tile([P, 16, HDIM + 1], BF16, tag="v_aug")
                    nc.vector.memset(v_aug, 1.0)
                    qh = ap_.tile([HDIM, N], BF16, tag="qh")
                    kh = ap_.tile([HDIM, N], BF16, tag="kh")
                    yhT = ap_.tile([HDIM, N], F8, tag="yhT")
                    for b in range(2):
                        for cc in (2 * b, 2 * b + 1):
                            sl = slice(cc * D, (cc + 1) * D)
                            ps = psS.tile([1, D], F32, tag="ps_small")
                            for kc in range(4):
                                sq = ate.tile([P, D], BF16, tag="sq_t",
                                              bufs=3)
                                nc.scalar.activation(sq, xTb[:, kc, sl],
                                                     AFT.Square)
                                nc.tensor.matmul(ps, ones128b, sq,
                                                 start=(kc == 0),
                                                 stop=(kc == 3))
                            tmp = ate.tile([1, D], F32, tag="r_t", bufs=2)
                            nc.scalar.activation(tmp, ps, AFT.Sqrt,
                                                 scale=1.0 / D,
                                                 bias=eps6[0:1, 0:1])
                            rrow = ate.tile([1, D], BF16, tag="rrow",
                                            bufs=2)
                            with nc.allow_low_precision(
                                    reason="bf16 bcast row"):
                                nc.vector.reciprocal(rrow, tmp)
                            # xnT = xTb * bcast(rrow * g)
                            for kc in range(4):
                                pb = psB.tile([P, D], F32, tag="bc")
                                nc.tensor.matmul(
                                    pb, grow[0:1, kc * P:(kc + 1) * P],
                                    rrow[0:1, :], start=True, stop=True)
                                nc.vector.tensor_mul(xnT[:, kc, sl],
                                                     xTb[:, kc, sl], pb)
                            for wi, dst, bi in ((0, qT, 0), (1, kT, 1)):
                                ps2 = psS.tile([HDIM, D], F32,
                                               tag="ps_small")
                                for kc in range(4):
                                    nc.tensor.matmul(
                                        ps2,
                                        wbig[:, kc * 192 + wi * HDIM:
                                             kc * 192 + (wi + 1) * HDIM],
                                        xnT[:, kc, sl], start=(kc == 0),
                                        stop=(kc == 3))
                                nc.scalar.activation(
                                    dst[:, sl], ps2, AFT.Identity,
                                    bias=bq_sb[:, bi:bi + 1])
                            for tk in range(cc * 4, cc * 4 + 4):
                                ps3 = psS.tile([P, HDIM], F32,
                                               tag="ps_small")
                                for kc in range(4):
                                    nc.tensor.matmul(
                                        ps3, xnT[:, kc, tk * P:(tk + 1) * P],
                                        wbig[:, kc * 192 + 128:
                                             kc * 192 + 192],
                                        start=(kc == 0), stop=(kc == 3))
                                nc.vector.tensor_add(v_aug[:, tk, 0:HDIM],
                                                     ps3, vbias_sb)
                            # qk-norm for this chunk (alpha folded in the
                            # q bcast row)
                            for src_, dst, brow in (
                                    (qT, qh, grow[0:1, D:D + HDIM]),
                                    (kT, kh, ones1rb[0:1, 0:HDIM])):
                                sq = ate.tile([HDIM, D], BF16, tag="sqn",
                                              bufs=2)
                                nc.scalar.activation(sq, src_[:, sl],
                                                     AFT.Square)
                                ps4 = psS.tile([1, D], F32, tag="ps_small")
                                nc.tensor.matmul(ps4, ones64b, sq,
                                                 start=True, stop=True)
                                t = ate.tile([1, D], F32, tag="rn_t",
                                             bufs=2)
                                nc.scalar.activation(t, ps4, AFT.Sqrt)
                                nc.vector.tensor_scalar_add(t, t, 1e-5)
                                rn = ate.tile([1, D], BF16, tag="rn",
                                              bufs=2)
                                with nc.allow_low_precision(
                                        reason="bf16 row"):
                                    nc.vector.reciprocal(rn, t)
                                pb = psB.tile([HDIM, D], F32, tag="bc")
                                nc.tensor.matmul(pb, brow, rn[0:1, :],
                                                 start=True, stop=True)
                                nc.vector.tensor_mul(dst[:, sl],
                                                     src_[:, sl], pb)
                        # scoresT -> exp (masked diag) -> AV (denominator
                        # folded into v_aug's ones row) -> yhT -> AG(b)
                        for qc in range(2):
                            qsl = slice(b * T + qc * D, b * T + (qc + 1) * D)
                            nkc = 4 * qc + 4
                            ex_tiles = []
                            for kc in range(nkc):
                                ksl = slice(b * T + kc * P,
                                            b * T + (kc + 1) * P)
                                ps = psE.tile([P, D], F32, tag="mm")
                                nc.tensor.matmul(ps, kh[:, ksl], qh[:, qsl],
                                                 start=True, stop=True)
                                if kc >= 4 * qc:  # diagonal block: mask
                                    et = ate.tile([P, D], BF16, tag="exp_b",
                                                  bufs=4)
                                    nc.scalar.activation(et, ps, AFT.Exp)
                                    eb2 = ate.tile([P, D], BF16,
                                                   tag="exp_m", bufs=7)
                                    nc.vector.tensor_mul(
                                        eb2, et,
                                        wbig[:, 768 + (kc - 4 * qc) * D:
                                             768 + (kc - 4 * qc + 1) * D])
                                else:
                                    eb2 = ate.tile([P, D], BF16,
                                                   tag="exp_m", bufs=7)
                                    nc.scalar.activation(eb2, ps, AFT.Exp)
                                ex_tiles.append(eb2)
                            py = psS.tile([HDIM + 1, D], F32,
                                          tag="ps_small")
                            for kc in range(nkc):
                                nc.tensor.matmul(py, v_aug[:, b * 8 + kc, :],
                                                 ex_tiles[kc],
                                                 start=(kc == 0),
                                                 stop=(kc == nkc - 1))
                            dr = ate.tile([1, D], BF16, tag="dr", bufs=2)
                            with nc.allow_low_precision(
                                    reason="bf16 softmax denom"):
                                nc.vector.reciprocal(dr,
                                                     py[HDIM:HDIM + 1, :])
                            pb2 = psB.tile([HDIM, D], F32, tag="bc")
                            nc.tensor.matmul(pb2, ones1rb[0:1, 0:HDIM], dr,
                                             start=True, stop=True)
                            db = ate.tile([HDIM, D], BF16, tag="db", bufs=2)
                            nc.scalar.activation(db, pb2, AFT.Copy,
                                                 scale=S_Y)
                            nc.vector.tensor_mul(yhT[:, qsl],
                                                 py[0:HDIM, :], db)
                            nc.sync.dma_start(
                                ag_in[b][:, qc * D:(qc + 1) * D],
                                yhT[:, qsl])
                        nc.gpsimd.collective_compute(
                            "AllGather", mybir.AluOpType.bypass,
                            ins=[ag_in[b][:]], outs=[ag_out[b][:]],
                            replica_groups=groups)

                # ---- proj + x_res + router (per batch), interleaved with
                # expert passes so weight DMAs never queue behind the
                # AG1-blocked b1 input DMAs ----
                pass
            # (s1 stays open: xTb / qT / kT / v_aug / wbig / bqa)
            with tc.tile_pool(name="s2", bufs=1) as s2, \
                 tc.tile_pool(name="s2e", bufs=4) as s2e, \
                 tc.tile_pool(name="eact", bufs=1) as ac, \
                 tc.tile_pool(name="eev", bufs=6) as ev_:
                yT_sb = s2.tile([P, 4, N], F8, tag="yT_sb")
                routes = s2.tile([P, 16, E], F32, tag="routes")
                gates = routes
                gsum = s2.tile([P, 16], F32, tag="gsum")
                gates_bf = s2.tile([P, 16, E], BF16, tag="gates_bf")

                def emit_batch(b):
                    for ts in (2 * b, 2 * b + 1):
                        nc.sync.dma_start(
                            yT_sb[:, :, ts * D:(ts + 1) * D],
                            ag_out[b][:, (ts % 2) * D:(ts % 2 + 1) * D]
                            .rearrange("(kc p) n -> p kc n", p=P))
                    for ts in (2 * b, 2 * b + 1):
                        for dc in range(4):
                            tsl = slice(ts * D, (ts + 1) * D)
                            ps = psE.tile([P, D], F32, tag="mm")
                            for kc in range(4):
                                nc.tensor.matmul(
                                    ps,
                                    wbig[:, 2816 + kc * D + dc * P:
                                         2816 + kc * D + (dc + 1) * P],
                                    yT_sb[:, kc, tsl],
                                    start=(kc == 0), stop=(kc == 3))
                            # xmb = x*featmask + projb*c0 (replaces the
                            # host xmask tensor)
                            xmb = s2e.tile([P, D], BF16, tag="xmb", bufs=3)
                            nc.vector.tensor_scalar(
                                xmb, xTb[:, dc, tsl],
                                mfeat_sb[:, dc:dc + 1],
                                pbc0_sb[:, dc:dc + 1], op0=MUL, op1=ADD)
                            tmp = s2e.tile([P, D], F32, tag="yp_t", bufs=3)
                            nc.scalar.activation(
                                tmp, ps, AFT.Identity, scale=1.0 / S_Y,
                                bias=projb_sb[:, dc:dc + 1])
                            # x_res written in place over xTb
                            nc.vector.tensor_add(xTb[:, dc, tsl], tmp,
                                                 xTb[:, dc, tsl])
                            nc.scalar.activation(xrT8[:, dc, tsl],
                                                 xTb[:, dc, tsl],
                                                 AFT.Copy, scale=S_X)
                            # moeT init: yp*c0 + x feature slice
                            nc.vector.scalar_tensor_tensor(
                                moeT[:, dc, tsl], ps, c0_sb[:, 0:1],
                                xmb, op0=MUL, op1=ADD)
                def emit_gates(b):
                    # router for this batch -> normalized top-8 gates
                    for tk in range(8 * b, 8 * b + 8):
                        ps = psS.tile([P, E], F32, tag="ps_small")
                        for kc in range(4):
                            nc.tensor.matmul(
                                ps, xTb[:, kc, tk * P:(tk + 1) * P],
                                wbig[:, 2816 + kc * E:2816 + (kc + 1) * E],
                                start=(kc == 0), stop=(kc == 3))
                        nc.vector.tensor_add(routes[:, tk, :], ps, rb_sb)
                    nc.scalar.activation(routes[:, 8 * b:8 * b + 8, :],
                                         routes[:, 8 * b:8 * b + 8, :],
                                         AFT.Exp)
                    for g in range(8 * b, 8 * b + 8):
                        m8 = s2e.tile([P, 8], F32, tag="m8", bufs=2)
                        nc.vector.max(out=m8, in_=routes[:, g, :])
                        zap = s2e.tile([P, E], F32, tag="zap", bufs=2)
                        nc.vector.match_replace(out=zap, in_to_replace=m8,
                                                in_values=routes[:, g, :],
                                                imm_value=0)
                        nc.vector.tensor_sub(gates[:, g, :],
                                             routes[:, g, :], zap)
                    nc.vector.reduce_sum(gsum[:, 8 * b:8 * b + 8],
                                         gates[:, 8 * b:8 * b + 8, :],
                                         axis=mybir.AxisListType.X)
                    nc.vector.reciprocal(gsum[:, 8 * b:8 * b + 8],
                                         gsum[:, 8 * b:8 * b + 8])
                    for g in range(8 * b, 8 * b + 8):
                        nc.vector.tensor_scalar_mul(gates_bf[:, g, :],
                                                    gates[:, g, :],
                                                    gsum[:, g:g + 1])
                    for g in range(8 * b, 8 * b + 8):
                        pt = psS.tile([E, P], BF16, tag="ps_small")
                        nc.tensor.transpose(pt, gates_bf[:, g, :], identb)
                        nc.scalar.activation(
                            gatesT[:, g * P:(g + 1) * P], pt, AFT.Copy)

                held = {}

                def emit_pass(e, half, post_h1=None):
                    ts_range = (2 * half, 2 * half + 1)
                    if half == 0:
                        if e == 0:
                            win_t = pre_win
                            wout_t = pre_wout
                        else:
                            win_t = ws.tile([P, 2, 16, 2, P], F8,
                                            tag="win", bufs=1, name="win_t")
                            nc.sync.dma_start(win_t, win_d[e])
                            wout_t = ws.tile([P, 8, 4, 2, P], F8,
                                             tag="wot", bufs=1,
                                             name="wout_t")
                            nc.sync.dma_start(wout_t, wout_d[e])
                        held[e] = (win_t, wout_t)
                    else:
                        win_t, wout_t = held[e]
                    # h1 = x_res @ w_in  (fp8, S_H)
                    for ts in ts_range:
                        for hc in range(16):
                            tsl = slice(ts * D, (ts + 1) * D)
                            ps = psE.tile([P, D], F32, tag="mm")
                            for kp in range(2):
                                nc.tensor.matmul(
                                    ps, win_t[:, kp, hc, :, :],
                                    xrT8[:, 2 * kp:2 * kp + 2, tsl],
                                    start=(kp == 0), stop=(kp == 1),
                                    perf_mode=DR)
                            if (hc + ts) % 2 == 0:
                                nc.scalar.activation(
                                    h1T8[:, hc, tsl], ps, AFT.Identity,
                                    scale=S_H / (S_X * S_W),
                                    bias=bias_cp["bin"][:, e, hc:hc + 1])
                            else:
                                nc.vector.tensor_scalar(
                                    h1T8[:, hc, tsl], ps,
                                    S_H / (S_X * S_W),
                                    bias_cp["bin"][:, e, hc:hc + 1],
                                    op0=MUL, op1=ADD)
                    if post_h1 is not None:
                        post_h1()
                    # SwiGLU: s = silu(h@w1b + b1b) * (h@w1a + b1a)
                    sT8 = ac.tile([P, 16, T], F8, tag="sT8", bufs=1,
                                  name="sT8")
                    for pr in range(16):
                        if e == 0 and half == 0 and pr < 3:
                            w1_t = pre_w1[pr]
                        else:
                            w1_t = ws.tile([P, 2, 8, 2, P], F8, tag="w1s",
                                           bufs=4, name="w1_t")
                            nc.sync.dma_start(w1_t, w1_d[e, pr])
                        for ts in ts_range:
                            tsl = slice(ts * D, (ts + 1) * D)
                            pa = psE.tile([P, D], F32, tag="mm")
                            pb = psE.tile([P, D], F32, tag="mm")
                            for kp in range(8):
                                nc.tensor.matmul(
                                    pa, w1_t[:, 0, kp, :, :],
                                    h1T8[:, 2 * kp:2 * kp + 2, tsl],
                                    start=(kp == 0), stop=(kp == 7),
                                    perf_mode=DR)
                            for kp in range(8):
                                nc.tensor.matmul(
                                    pb, w1_t[:, 1, kp, :, :],
                                    h1T8[:, 2 * kp:2 * kp + 2, tsl],
                                    start=(kp == 0), stop=(kp == 7),
                                    perf_mode=DR)
                            sil = ev_.tile([P, D], BF16, tag="sil", bufs=4)
                            nc.scalar.activation(
                                sil, pb, AFT.Silu,
                                scale=1.0 / (S_H * S_W),
                                bias=bias_cp["b1b"][:, e, pr:pr + 1])
                            av8 = ev_.tile([P, D], F8, tag="av8", bufs=4)
                            if (pr + ts) % 2 == 0:
                                nc.scalar.activation(
                                    av8, pa, AFT.Identity,
                                    scale=S_S / (S_H * S_W),
                                    bias=bias_cp["b1a"][:, e, pr:pr + 1])
                            else:
                                nc.vector.tensor_scalar(
                                    av8, pa, S_S / (S_H * S_W),
                                    bias_cp["b1a"][:, e, pr:pr + 1],
                                    op0=MUL, op1=ADD)
                            ltsl = slice((ts - 2 * half) * D,
                                         (ts - 2 * half + 1) * D)
                            nc.vector.tensor_mul(sT8[:, pr, ltsl], av8, sil)
                    # o = s @ w2 + b2  (fp8, S_O)
                    oT8 = ac.tile([P, 16, T], F8, tag="oT8", bufs=1,
                                  name="oT8")
                    for og in range(4):
                        w2_t = ws.tile([P, 4, 8, 2, P], F8, tag="w2s",
                                       bufs=2, name="w2_t")
                        nc.sync.dma_start(w2_t, w2_d[e, og])
                        for oc4 in range(4):
                            oc = og * 4 + oc4
                            for ts in ts_range:
                                ltsl = slice((ts - 2 * half) * D,
                                             (ts - 2 * half + 1) * D)
                                ps = psE.tile([P, D], F32, tag="mm")
                                for kp in range(8):
                                    nc.tensor.matmul(
                                        ps, w2_t[:, oc4, kp, :, :],
                                        sT8[:, 2 * kp:2 * kp + 2, ltsl],
                                        start=(kp == 0), stop=(kp == 7),
                                        perf_mode=DR)
                                if (oc + ts) % 2 == 0:
                                    nc.scalar.activation(
                                        oT8[:, oc, ltsl], ps, AFT.Identity,
                                        scale=S_O / (S_S * S_W),
                                        bias=bias_cp["b2"][:, e, oc:oc + 1])
                                else:
                                    nc.vector.tensor_scalar(
                                        oT8[:, oc, ltsl], ps,
                                        S_O / (S_S * S_W),
                                        bias_cp["b2"][:, e, oc:oc + 1],
                                        op0=MUL, op1=ADD)
                    # gate broadcast [tokens] -> [P, D] per ts chunk
                    gb_tiles = {}
                    for ts in ts_range:
                        tsl = slice(ts * D, (ts + 1) * D)
                        pg = psB.tile([P, D], F32, tag="bc")
                        nc.tensor.matmul(pg, sel_sb[:, e, :], gatesT[:, tsl],
                                         start=True, stop=True)
                        gb = ev_.tile([P, D], BF16, tag="gb", bufs=4)
                        nc.scalar.activation(gb, pg, AFT.Copy)
                        gb_tiles[ts] = gb
                    # eo = o @ w_out + b_out; moeT += gate * eo
                    for dc in range(4):
                        for ts in ts_range:
                            tsl = slice(ts * D, (ts + 1) * D)
                            ltsl = slice((ts - 2 * half) * D,
                                         (ts - 2 * half + 1) * D)
                            ps = psE.tile([P, D], F32, tag="mm")
                            for kp in range(8):
                                nc.tensor.matmul(
                                    ps, wout_t[:, kp, dc, :, :],
                                    oT8[:, 2 * kp:2 * kp + 2, ltsl],
                                    start=(kp == 0), stop=(kp == 7),
                                    perf_mode=DR)
                            eo = ev_.tile([P, D], F32, tag="eo", bufs=3)
                            nc.scalar.activation(
                                eo, ps, AFT.Identity,
                                scale=1.0 / (S_O * S_W),
                                bias=bias_cp["bout"][:, e, dc:dc + 1])
                            t2 = ev_.tile([P, D], F32, tag="t2", bufs=3)
                            nc.vector.tensor_mul(t2, eo, gb_tiles[ts])
                            if (dc + ts) % 2 == 0:
                                nc.vector.tensor_add(moeT[:, dc, tsl],
                                                     moeT[:, dc, tsl], t2)
                            else:
                                nc.gpsimd.tensor_add(moeT[:, dc, tsl],
                                                     moeT[:, dc, tsl], t2)
                            if e == 1:
                                nc.sync.dma_start(
                                    rs_in[half][dc * P:(dc + 1) * P,
                                                (ts - 2 * half) * D:
                                                (ts - 2 * half + 1) * D],
                                    moeT[:, dc, tsl])
                    if e == 1:
                        nc.gpsimd.collective_compute(
                            "ReduceScatter", mybir.AluOpType.add,
                            ins=[rs_in[half][:]], outs=[rs_out[half][:]],
                            replica_groups=groups)
                        nc.sync.dma_start(
                            out_d[:, half * T:(half + 1) * T],
                            rs_out[half][:, :])

                emit_batch(0)
                emit_pass(0, 0, post_h1=lambda: emit_gates(0))
                emit_batch(1)
                emit_pass(0, 1, post_h1=lambda: emit_gates(1))
                emit_pass(1, 0)
                emit_pass(1, 1)
            s1pre.release()

            ws.release()

    _split_matmul_waits(nc)
    return nc


def _split_matmul_waits(nc):
    """walrus allows only one sync-wait per engine-instruction sync slot; move
    extra waits onto standalone InstEventSemaphore waits inserted before."""
    import concourse.mybir as mybir
    k = 0
    for bb in nc.main_func.blocks:
        il = list(bb.instructions)
        out = []
        changed = False
        for ins in il:
            si = getattr(ins, "sync_info", None)
            if si is not None and len(si.on_wait) > 1 \
                    and type(ins).__name__ != "InstEventSemaphore":
                waits = list(si.on_wait)
                keep, move = waits[-1], waits[:-1]
                for w in move:
                    nop = mybir.InstEventSemaphore(name=f"I-wsplit-{k}",
                                                   ins=[], outs=[])
                    k += 1
                    nop.engine = ins.engine
                    nop.sync_info = type(si)(on_wait=[w], on_update=[])
                    out.append(nop)
                ins.sync_info = type(si)(on_wait=[keep],
                                         on_update=list(si.on_update))
                changed = True
            out.append(ins)
        if changed:
            bb.instructions = out


def _q8w(w):
    """host fp8 cast with fixed 2^11 scale (clipped to TRN e4m3 max)."""
    return np.clip(np.asarray(w, np.float32) * S_W, -240.0, 240.0).astype(
        ml_dtypes.float8_e4m3)


def _prep_inputs(inputs, core):
    bf = ml_dtypes.bfloat16
    f32 = np.float32
    h = core
    x = np.asarray(inputs["x"], f32).reshape(N, D)
    xT = np.ascontiguousarray(x.T)                      # [512, 2048]
    g = np.asarray(inputs["g"], f32)
    bvec = np.asarray(inputs["b"], f32)
    caw = np.asarray(inputs["c_attn_w"], f32)
    cab = np.asarray(inputs["c_attn_b"], f32)
    wq = caw[:, h * 64:(h + 1) * 64]
    wk = caw[:, 512 + h * 64:512 + (h + 1) * 64]
    wv = caw[:, 1024 + h * 64:1024 + (h + 1) * 64]
    wqkv = np.concatenate([wq, wk, wv], axis=1)          # [512, 192]
    # RMSNorm additive b folded into qkv biases
    bq = bvec @ wq + cab[h * 64:(h + 1) * 64]
    bk = bvec @ wk + cab[512 + h * 64:512 + (h + 1) * 64]
    bv = bvec @ wv + cab[1024 + h * 64:1024 + (h + 1) * 64]
    kk = np.arange(4)[None, :, None] * P + np.arange(P)[:, None, None]
    qq = np.arange(D)[None, None, :]
    maskd = (kk <= qq).astype(f32)                       # [128, 4, 512]
    projb = np.asarray(inputs["c_proj_b"], f32)
    projb_col = np.ascontiguousarray(projb.reshape(4, P).T)  # [p, dc]
    c0 = 1.0 if core == 0 else 0.0
    xmask = np.zeros((D, N), f32)
    xmask[64 * core:64 * core + 64, :] = xT[64 * core:64 * core + 64, :]
    xmask += (projb * c0)[:, None]
    xmask = np.ascontiguousarray(
        xmask.reshape(4, P, N).transpose(1, 0, 2)).astype(bf)
    selb = np.zeros((E, EL, P), f32)
    selb[2 * core, 0, :] = 1.0
    selb[2 * core + 1, 1, :] = 1.0

    sl = slice(2 * core, 2 * core + 2)
    w_in = np.asarray(inputs["w_in"], f32)[sl]           # [2, 512, 2048]
    w1 = np.asarray(inputs["w1"], f32)[sl]               # [2, 2048, 4096]
    w2 = np.asarray(inputs["w2"], f32)[sl]               # [2, 2048, 2048]
    w_out = np.asarray(inputs["w_out"], f32)[sl]         # [2, 2048, 512]

    # DoubleRow lhsT layouts (pair index j adjacent to the 128-wide m dim)
    w_in8 = np.ascontiguousarray(
        _q8w(w_in).reshape(EL, 2, 2, P, 16, P)
        .transpose(0, 3, 1, 4, 2, 5))                    # [EL, p, kp, hc, j, m]
    w1a = _q8w(w1[:, :, :HD]).reshape(EL, 8, 2, P, 16, P)
    w1b = _q8w(w1[:, :, HD:]).reshape(EL, 8, 2, P, 16, P)
    w18 = np.stack([w1a, w1b], axis=2)                   # [EL, kp, ab, j, p, pr, m]
    w18 = np.ascontiguousarray(
        w18.transpose(0, 5, 4, 2, 1, 3, 6))              # [EL, pr, p, ab, kp, j, m]
    w28 = _q8w(w2).reshape(EL, 8, 2, P, 16, P) \
        .transpose(0, 4, 3, 1, 2, 5)                     # [EL, oc, p, kp, j, m]
    w28 = np.ascontiguousarray(
        w28.reshape(EL, 4, 4, P, 8, 2, P)
        .transpose(0, 1, 3, 2, 4, 5, 6))                 # [EL, og, p, ocl, kp, j, m]
    wout8 = np.ascontiguousarray(
        _q8w(w_out).reshape(EL, 8, 2, P, 4, P)
        .transpose(0, 3, 1, 4, 2, 5))                    # [EL, p, kp, dc, j, m]

    def bias_t(key, scale, w):
        b = np.asarray(inputs[key], f32)[sl] * scale     # [2, w*128]
        return np.ascontiguousarray(b.reshape(EL, w, P).transpose(2, 0, 1))

    mfeat = np.zeros((D,), f32)
    mfeat[64 * core:64 * core + 64] = 1.0
    smalls = np.concatenate([
        np.ascontiguousarray(mfeat.reshape(4, P).T),
        np.ascontiguousarray((projb * c0).reshape(4, P).T),
        np.broadcast_to(np.asarray(inputs["router_b"], f32), (P, E)),
        projb_col,
        projb_col * c0,
        np.full((P, 1), c0 / S_Y, f32),
        np.broadcast_to(bv, (P, HDIM)),
    ], axis=1).astype(f32)                               # [P, 97]
    bqa = np.zeros((HDIM, 4), f32)
    bqa[:, 0] = bq
    bqa[:, 1] = bk
    bqa[0, 3] = np.asarray(inputs["alpha"], f32)[h]
    wbig = np.concatenate([
        wqkv.reshape(4, P, 192).transpose(1, 0, 2).reshape(P, 768),
        maskd.reshape(P, 4 * D),
        np.asarray(inputs["router_w"], f32)
        .reshape(4, P, E).transpose(1, 0, 2).reshape(P, 4 * E),
    ], axis=1).astype(bf)                                # [P, 2880]
    b1 = bias_t("b1", 1.0, 32)
    ebias = np.concatenate([
        bias_t("b_in", S_H, 16),
        b1[:, :, :16] * S_S,
        b1[:, :, 16:],
        bias_t("b2", S_O, 16),
        bias_t("b_out", 1.0, 4),
    ], axis=2).astype(f32)                               # [P, 2, 68]
    return {
        "xtb": np.ascontiguousarray(
            xT.reshape(4, P, N).transpose(1, 0, 2)).astype(bf),
        "xmask": xmask,
        "smalls": smalls,
        "bqalpha": bqa,
        "wbig": wbig,
        "grow": np.concatenate(
            [g, np.full((HDIM,), np.asarray(inputs["alpha"], f32)[h])]
        ).reshape(1, D + HDIM).astype(bf),
        "selb": selb.astype(bf),
        "w_in8": w_in8, "w18": w18, "w28": w28, "wout8": wout8,
        "ebias": ebias,
    }


last_result = [None]


def kernel(**inputs):
    if "nc" not in _cache:
        _cache["nc"] = build_program()
    nc = _cache["nc"]
    in_maps = [_prep_inputs(inputs, c) for c in range(NCORES)]
    res = run_bass_kernel_spmd(nc, in_maps, core_ids=list(range(NCORES)))
    last_result[0] = res
    outT = np.concatenate(
        [np.asarray(res.results[c]["out"]).astype(np.float32)
         for c in range(NCORES)], axis=0)                # [512, 2048]
    return np.ascontiguousarray(outT.T).reshape(2, 1024, 512)


# revision 45
# speedup vs baseline: 1.0935x; 1.0009x over previous
"""MoE transformer block (QK-norm attention + top-8-of-16 MoE) on 8 trn2 cores.

v4: dense fp8 DoubleRow experts (as v2 baseline) with restructured
scheduling:
- batch-split expert pipeline: each expert runs a b0-pass (tokens 0-1023)
  then a b1-pass, so expert-0's b0 compute starts right after AllGather-0
  and fully hides AllGather-1 + proj-b1;
- the ReduceScatter is split per batch: RS(b0)'s input is complete after
  the last expert's b0-pass and it runs hidden under ~95us of b1 compute,
  leaving only RS(b1) (~18us) in the tail;
- attention-phase cost cuts: softmax denominator folded into an augmented
  v row (kills 24 denominator matmuls), all broadcast matmuls in bf16
  (1 cyc/row instead of 4), RMSNorm's g folded into the broadcast and its
  additive b folded into the qkv biases (host-side);
- SwiGLU's a-path and the moe combine adds alternate DVE/ACT/GpSimd to
  keep the vector engine off the critical path;
- startup DMAs split so RMSNorm starts after the first 0.5MB chunk.

Sharding: attention head-parallel (core c owns head c), experts
expert-parallel (core c owns experts 2c, 2c+1), output feature-parallel
(core c returns features [64c, 64c+64) for all tokens).

Everything feature-major ("T layout": features on partitions, tokens on
free). Scales (powers of two, exact): x_res*2^5, weights*2^11, h1*2^5,
s*2^5, o*2^7; descales folded into Act/DVE scale+bias immediates.
"""

import numpy as np
import ml_dtypes

import concourse.bass as bass
import concourse.mybir as mybir
from concourse.tile import TileContext
from concourse.masks import make_identity
from concourse.bass_utils import run_bass_kernel_spmd

BF16 = mybir.dt.bfloat16
F32 = mybir.dt.float32
F8 = mybir.dt.float8e4
AFT = mybir.ActivationFunctionType
MUL = mybir.AluOpType.mult
ADD = mybir.AluOpType.add
DR = mybir.MatmulPerfMode.DoubleRow

P = 128
D = 512          # embed dim
T = 1024         # tokens per batch
N = 2048         # total tokens
E = 16           # experts
EL = 2           # experts per core
HD = 2048        # expert hidden
HDIM = 64        # head dim
NCORES = 8

S_X = 2.0 ** 5
S_W = 2.0 ** 11
S_H = 2.0 ** 5
S_S = 2.0 ** 5
S_O = 2.0 ** 7
S_Y = 2.0 ** 5

_cache = {}


def build_program():
    nc = bass.Bass()
    dp_ = dict(isOutput=False)
    xtb_d = nc.declare_dram_parameter("xtb", [P, 4, N], BF16, **dp_)
    xm_d = nc.declare_dram_parameter("xmask", [P, 4, N], BF16, **dp_)
    sm_d = nc.declare_dram_parameter("smalls", [P, 97], F32, **dp_)
    bqa_d = nc.declare_dram_parameter("bqalpha", [HDIM, 4], F32, **dp_)
    wbig_d = nc.declare_dram_parameter("wbig", [P, 2880], BF16, **dp_)
    grow_d = nc.declare_dram_parameter("grow", [1, D + HDIM], BF16,
                                       **dp_)
    sel_d = nc.declare_dram_parameter("selb", [E, EL, P], BF16, **dp_)
    win_d = nc.declare_dram_parameter("w_in8", [EL, P, 2, 16, 2, P], F8, **dp_)
    w1_d = nc.declare_dram_parameter("w18", [EL, 16, P, 2, 8, 2, P], F8, **dp_)
    w2_d = nc.declare_dram_parameter("w28", [EL, 4, P, 4, 8, 2, P], F8, **dp_)
    wout_d = nc.declare_dram_parameter("wout8", [EL, P, 8, 4, 2, P], F8, **dp_)
    eb_d = nc.declare_dram_parameter("ebias", [P, EL, 68], F32, **dp_)
    out_d = nc.declare_dram_parameter("out", [HDIM, N], BF16, isOutput=True)

    groups = [list(range(NCORES))]

    with TileContext(nc, num_cores=NCORES) as tc:
        with (
            tc.tile_pool(name="const", bufs=1) as cp,
            tc.tile_pool(name="pp", bufs=1) as pp,
            tc.tile_pool(name="psE", bufs=4, space="PSUM") as psE,
            tc.tile_pool(name="psB", bufs=2, space="PSUM") as psB,
            tc.tile_pool(name="psS", bufs=2, space="PSUM") as psS,
            tc.tile_pool(name="dram", bufs=1, space="DRAM") as dp,
        ):
            # ---- constants / persistent ----
            ws = tc.alloc_tile_pool(name="wst", bufs=1)
            s1pre = tc.alloc_tile_pool(name="s1pre", bufs=1)
            xTb = s1pre.tile([P, 4, N], BF16, tag="xTb")
            nc.sync.dma_start(xTb[:, :, 0:D], xtb_d[:, :, 0:D])
            wbig = s1pre.tile([P, 2880], BF16, tag="wbig")
            nc.sync.dma_start(wbig[:, 0:768], wbig_d[:, 0:768])
            for cc in range(1, 4):
                sl = slice(cc * D, (cc + 1) * D)
                nc.sync.dma_start(xTb[:, :, sl], xtb_d[:, :, sl])
            ones128b = cp.tile([P, 1], BF16, tag="ones128b")
            nc.vector.memset(ones128b, 1.0)
            ones64b = cp.tile([HDIM, 1], BF16, tag="ones64b")
            nc.vector.memset(ones64b, 1.0)
            ones1rb = cp.tile([1, P], BF16, tag="ones1rb")
            nc.vector.memset(ones1rb, 1.0)
            sm = cp.tile([P, 97], F32, tag="sm")
            nc.sync.dma_start(sm, sm_d[:, :])
            mfeat_sb = sm[:, 0:4]
            pbc0_sb = sm[:, 4:8]
            rb_sb = sm[:, 8:24]
            projb_sb = sm[:, 24:28]
            c0_sb = sm[:, 32:33]
            vbias_sb = sm[:, 33:97]
            sel_sb = cp.tile([E, EL, P], BF16, tag="sel_sb")
            nc.sync.dma_start(sel_sb, sel_d[:, :, :])
            eps6 = cp.tile([1, 1], F32, tag="eps6")
            nc.vector.memset(eps6, 1e-6)
            grow = cp.tile([1, D + HDIM], BF16, tag="grow")
            nc.sync.dma_start(grow, grow_d[:, :])
            identb = cp.tile([P, P], BF16, tag="identb")
            make_identity(nc, identb)

            # persistent activations for the expert phase
            moeT = pp.tile([P, 4, N], BF16, tag="moeT")
            xrT8 = pp.tile([P, 4, N], F8, tag="xrT8")
            gatesT = pp.tile([E, N], BF16, tag="gatesT")
            h1T8 = pp.tile([P, 16, N], F8, tag="h1T8")

            ag_in = [dp.tile([HDIM, T], F8, name=f"ag_in{i}")
                     for i in range(2)]
            ag_out = [dp.tile([D, T], F8, addr_space="Shared",
                              name=f"ag_out{i}")
                      for i in range(2)]
            rs_in = [dp.tile([D, T], BF16, name=f"rs_in{i}")
                     for i in range(2)]
            rs_out = [dp.tile([HDIM, T], BF16, name=f"rs_out{i}")
                      for i in range(2)]

            with tc.tile_pool(name="s1", bufs=1) as s1:
                nc.sync.dma_start(wbig[:, 768:2880], wbig_d[:, 768:2880])
                bqa = s1.tile([HDIM, 4], F32, tag="bqa")
                nc.sync.dma_start(bqa, bqa_d[:, :])
                bq_sb = bqa[:, 0:3]
                alpha_sb = bqa[0:1, 3:4]
                # prefetch: expert biases + e0 weights (consumed ~90us later)
                ebt = pp.tile([P, EL, 68], F32, tag="ebias")
                nc.sync.dma_start(ebt, eb_d[:, :, :])
                bias_cp = {"bin": ebt[:, :, 0:16], "b1a": ebt[:, :, 16:32],
                           "b1b": ebt[:, :, 32:48], "b2": ebt[:, :, 48:64],
                           "bout": ebt[:, :, 64:68]}
                pre_win = ws.tile([P, 2, 16, 2, P], F8, tag="win", bufs=1)
                nc.sync.dma_start(pre_win, win_d[0])
                pre_wout = ws.tile([P, 8, 4, 2, P], F8, tag="wot", bufs=1)
                nc.sync.dma_start(pre_wout, wout_d[0])
                pre_w1 = []
                for pr in range(3):
                    t = ws.tile([P, 2, 8, 2, P], F8, tag="w1s", bufs=4)
                    nc.sync.dma_start(t, w1_d[0, pr])
                    pre_w1.append(t)
                # ---- RMSNorm + qkv + qk-norm + attention, batch-ordered:
                # all of batch b's chain runs before batch b+1 so AG(b)
                # issues early and b1 prep fills the AG0 window ----
                with tc.tile_pool(name="attp", bufs=1) as ap_, \
                     tc.tile_pool(name="ate", bufs=12) as ate:
                    xnT = ap_.tile([P, 4, N], BF16, tag="xnT")
                    qT = ap_.tile([HDIM, N], BF16, tag="qT")
                    kT = ap_.tile([HDIM, N], BF16, tag="kT")
                    v_aug = ap_.tile([P, 16, HDIM + 1], BF16, tag="v_aug")
                    nc.vector.memset(v_aug, 1.0)
                    qh = ap_.tile([HDIM, N], BF16, tag="qh")
                    kh = ap_.tile([HDIM, N], BF16, tag="kh")
                    yhT = ap_.tile([HDIM, N], F8, tag="yhT")
                    for b in range(2):
                        for cc in (2 * b, 2 * b + 1):
                            sl = slice(cc * D, (cc + 1) * D)
                            ps = psS.tile([1, D], F32, tag="ps_small")
                            for kc in range(4):
                                sq = ate.tile([P, D], BF16, tag="sq_t",
                                              bufs=3)
                                nc.scalar.activation(sq, xTb[:, kc, sl],
                                                     AFT.Square)
                                nc.tensor.matmul(ps, ones128b, sq,
                                                 start=(kc == 0),
                                                 stop=(kc == 3))
                            tmp = ate.tile([1, D], F32, tag="r_t", bufs=2)
                            nc.scalar.activation(tmp, ps, AFT.Sqrt,
                                                 scale=1.0 / D,
                                                 bias=eps6[0:1, 0:1])
                            rrow = ate.tile([1, D], BF16, tag="rrow",
                                            bufs=2)
                            with nc.allow_low_precision(
                                    reason="bf16 bcast row"):
                                nc.vector.reciprocal(rrow, tmp)
                            # xnT = xTb * bcast(rrow * g)
                            for kc in range(4):
                                pb = psB.tile([P, D], F32, tag="bc")
                                nc.tensor.matmul(
                                    pb, grow[0:1, kc * P:(kc + 1) * P],
                                    rrow[0:1, :], start=True, stop=True)
                                nc.vector.tensor_mul(xnT[:, kc, sl],
                                                     xTb[:, kc, sl], pb)
                            for wi, dst, bi in ((0, qT, 0), (1, kT, 1)):
                                ps2 = psS.tile([HDIM, D], F32,
                                               tag="ps_small")
                                for kc in range(4):
                                    nc.tensor.matmul(
                                        ps2,
                                        wbig[:, kc * 192 + wi * HDIM:
                                             kc * 192 + (wi + 1) * HDIM],
                                        xnT[:, kc, sl], start=(kc == 0),
                                        stop=(kc == 3))
                                nc.scalar.activation(
                                    dst[:, sl], ps2, AFT.Identity,
                                    bias=bq_sb[:, bi:bi + 1])
                            for tk in range(cc * 4, cc * 4 + 4):
                                ps3 = psS.tile([P, HDIM], F32,
                                               tag="ps_small")
                                for kc in range(4):
                                    nc.tensor.matmul(
                                        ps3, xnT[:, kc, tk * P:(tk + 1) * P],
                                        wbig[:, kc * 192 + 128:
                                             kc * 192 + 192],
                                        start=(kc == 0), stop=(kc == 3))
                                nc.vector.tensor_add(v_aug[:, tk, 0:HDIM],
                                                     ps3, vbias_sb)
                            # qk-norm for this chunk (alpha folded in the
                            # q bcast row)
                            for src_, dst, brow in (
                                    (qT, qh, grow[0:1, D:D + HDIM]),
                                    (kT, kh, ones1rb[0:1, 0:HDIM])):
                                sq = ate.tile([HDIM, D], BF16, tag="sqn",
                                              bufs=2)
                                nc.scalar.activation(sq, src_[:, sl],
                                                     AFT.Square)
                                ps4 = psS.tile([1, D], F32, tag="ps_small")
                                nc.tensor.matmul(ps4, ones64b, sq,
                                                 start=True, stop=True)
                                t = ate.tile([1, D], F32, tag="rn_t",
                                             bufs=2)
                                nc.scalar.activation(t, ps4, AFT.Sqrt)
                                nc.vector.tensor_scalar_add(t, t, 1e-5)
                                rn = ate.tile([1, D], BF16, tag="rn",
                                              bufs=2)
                                with nc.allow_low_precision(
                                        reason="bf16 row"):
                                    nc.vector.reciprocal(rn, t)
                                pb = psB.tile([HDIM, D], F32, tag="bc")
                                nc.tensor.matmul(pb, brow, rn[0:1, :],
                                                 start=True, stop=True)
                                nc.vector.tensor_mul(dst[:, sl],
                                                     src_[:, sl], pb)
                        # scoresT -> exp (masked diag) -> AV (denominator
                        # folded into v_aug's ones row) -> yhT -> AG(b)
                        for qc in range(2):
                            qsl = slice(b * T + qc * D, b * T + (qc + 1) * D)
                            nkc = 4 * qc + 4
                            ex_tiles = []
                            for kc in range(nkc):
                                ksl = slice(b * T + kc * P,
                                            b * T + (kc + 1) * P)
                                ps = psE.tile([P, D], F32, tag="mm")
                                nc.tensor.matmul(ps, kh[:, ksl], qh[:, qsl],
                                                 start=True, stop=True)
                                if kc >= 4 * qc:  # diagonal block: mask
                                    et = ate.tile([P, D], BF16, tag="exp_b",
                                                  bufs=4)
                                    nc.scalar.activation(et, ps, AFT.Exp)
                                    eb2 = ate.tile([P, D], BF16,
                                                   tag="exp_m", bufs=7)
                                    nc.vector.tensor_mul(
                                        eb2, et,
                                        wbig[:, 768 + (kc - 4 * qc) * D:
                                             768 + (kc - 4 * qc + 1) * D])
                                else:
                                    eb2 = ate.tile([P, D], BF16,
                                                   tag="exp_m", bufs=7)
                                    nc.scalar.activation(eb2, ps, AFT.Exp)
                                ex_tiles.append(eb2)
                            py = psS.tile([HDIM + 1, D], F32,
                                          tag="ps_small")
                            for kc in range(nkc):
                                nc.tensor.matmul(py, v_aug[:, b * 8 + kc, :],
                                                 ex_tiles[kc],
                                                 start=(kc == 0),
                                                 stop=(kc == nkc - 1))
                            dr = ate.tile([1, D], BF16, tag="dr", bufs=2)
                            with nc.allow_low_precision(
                                    reason="bf16 softmax denom"):
                                nc.vector.reciprocal(dr,
                                                     py[HDIM:HDIM + 1, :])
                            pb2 = psB.tile([HDIM, D], F32, tag="bc")
                            nc.tensor.matmul(pb2, ones1rb[0:1, 0:HDIM], dr,
                                             start=True, stop=True)
                            db = ate.tile([HDIM, D], BF16, tag="db", bufs=2)
                            nc.scalar.activation(db, pb2, AFT.Copy,
                                                 scale=S_Y)
                            nc.vector.tensor_mul(yhT[:, qsl],
                                                 py[0:HDIM, :], db)
                            nc.sync.dma_start(
                                ag_in[b][:, qc * D:(qc + 1) * D],
                                yhT[:, qsl])
                        nc.gpsimd.collective_compute(
                            "AllGather", mybir.AluOpType.bypass,
                            ins=[ag_in[b][:]], outs=[ag_out[b][:]],
                            replica_groups=groups)

                # ---- proj + x_res + router (per batch), interleaved with
                # expert passes so weight DMAs never queue behind the
                # AG1-blocked b1 input DMAs ----
                pass
            # (s1 stays open: xTb / qT / kT / v_aug / wbig / bqa)
            with tc.tile_pool(name="s2", bufs=1) as s2, \
                 tc.tile_pool(name="s2e", bufs=4) as s2e, \
                 tc.tile_pool(name="eact", bufs=1) as ac, \
                 tc.tile_pool(name="eev", bufs=6) as ev_:
                yT_sb = s2.tile([P, 4, N], F8, tag="yT_sb")
                routes = s2.tile([P, 16, E], F32, tag="routes")
                gates = routes
                gsum = s2.tile([P, 16], F32, tag="gsum")
                gates_bf = s2.tile([P, 16, E], BF16, tag="gates_bf")

                def emit_batch(b):
                    for ts in (2 * b, 2 * b + 1):
                        nc.sync.dma_start(
                            yT_sb[:, :, ts * D:(ts + 1) * D],
                            ag_out[b][:, (ts % 2) * D:(ts % 2 + 1) * D]
                            .rearrange("(kc p) n -> p kc n", p=P))
                    for ts in (2 * b, 2 * b + 1):
                        for dc in range(4):
                            tsl = slice(ts * D, (ts + 1) * D)
                            ps = psE.tile([P, D], F32, tag="mm")
                            for kc in range(4):
                                nc.tensor.matmul(
                                    ps,
                                    wbig[:, 2816 + kc * D + dc * P:
                                         2816 + kc * D + (dc + 1) * P],
                                    yT_sb[:, kc, tsl],
                                    start=(kc == 0), stop=(kc == 3))
                            # xmb = x*featmask + projb*c0 (replaces the
                            # host xmask tensor)
                            xmb = s2e.tile([P, D], BF16, tag="xmb", bufs=3)
                            nc.vector.tensor_scalar(
                                xmb, xTb[:, dc, tsl],
                                mfeat_sb[:, dc:dc + 1],
                                pbc0_sb[:, dc:dc + 1], op0=MUL, op1=ADD)
                            tmp = s2e.tile([P, D], F32, tag="yp_t", bufs=3)
                            nc.scalar.activation(
                                tmp, ps, AFT.Identity, scale=1.0 / S_Y,
                                bias=projb_sb[:, dc:dc + 1])
                            # x_res written in place over xTb
                            nc.vector.tensor_add(xTb[:, dc, tsl], tmp,
                                                 xTb[:, dc, tsl])
                            nc.scalar.activation(xrT8[:, dc, tsl],
                                                 xTb[:, dc, tsl],
                                                 AFT.Copy, scale=S_X)
                            # moeT init: yp*c0 + x feature slice
                            nc.vector.scalar_tensor_tensor(
                                moeT[:, dc, tsl], ps, c0_sb[:, 0:1],
                                xmb, op0=MUL, op1=ADD)
                def emit_gates(b):
                    # router for this batch -> normalized top-8 gates
                    for tk in range(8 * b, 8 * b + 8):
                        ps = psS.tile([P, E], F32, tag="ps_small")
                        for kc in range(4):
                            nc.tensor.matmul(
                                ps, xTb[:, kc, tk * P:(tk + 1) * P],
                                wbig[:, 2816 + kc * E:2816 + (kc + 1) * E],
                                start=(kc == 0), stop=(kc == 3))
                        nc.vector.tensor_add(routes[:, tk, :], ps, rb_sb)
                    nc.scalar.activation(routes[:, 8 * b:8 * b + 8, :],
                                         routes[:, 8 * b:8 * b + 8, :],
                                         AFT.Exp)
                    for g in range(8 * b, 8 * b + 8):
                        m8 = s2e.tile([P, 8], F32, tag="m8", bufs=2)
                        nc.vector.max(out=m8, in_=routes[:, g, :])
                        zap = s2e.tile([P, E], F32, tag="zap", bufs=2)
                        nc.vector.match_replace(out=zap, in_to_replace=m8,
                                                in_values=routes[:, g, :],
                                                imm_value=0)
                        nc.vector.tensor_sub(gates[:, g, :],
                                             routes[:, g, :], zap)
                    nc.vector.reduce_sum(gsum[:, 8 * b:8 * b + 8],
                                         gates[:, 8 * b:8 * b + 8, :],
                                         axis=mybir.AxisListType.X)
                    nc.vector.reciprocal(gsum[:, 8 * b:8 * b + 8],
                                         gsum[:, 8 * b:8 * b + 8])
                    for g in range(8 * b, 8 * b + 8):
                        nc.vector.tensor_scalar_mul(gates_bf[:, g, :],
                                                    gates[:, g, :],
                                                    gsum[:, g:g + 1])
                    for g in range(8 * b, 8 * b + 8):
                        pt = psS.tile([E, P], BF16, tag="ps_small")
                        nc.tensor.transpose(pt, gates_bf[:, g, :], identb)
                        nc.scalar.activation(
                            gatesT[:, g * P:(g + 1) * P], pt, AFT.Copy)

                held = {}

                def emit_pass(e, half, post_h1=None):
                    ts_range = (2 * half, 2 * half + 1)
                    if half == 0:
                        if e == 0:
                            win_t = pre_win
                            wout_t = pre_wout
                        else:
                            win_t = ws.tile([P, 2, 16, 2, P], F8,
                                            tag="win", bufs=1, name="win_t")
                            nc.sync.dma_start(win_t, win_d[e])
                            wout_t = ws.tile([P, 8, 4, 2, P], F8,
                                             tag="wot", bufs=1,
                                             name="wout_t")
                            nc.sync.dma_start(wout_t, wout_d[e])
                        held[e] = (win_t, wout_t)
                    else:
                        win_t, wout_t = held[e]
                    # h1 = x_res @ w_in  (fp8, S_H)
                    for ts in ts_range:
                        for hc in range(16):
                            tsl = slice(ts * D, (ts + 1) * D)
                            ps = psE.tile([P, D], F32, tag="mm")
                            for kp in range(2):
                                nc.tensor.matmul(
                                    ps, win_t[:, kp, hc, :, :],
                                    xrT8[:, 2 * kp:2 * kp + 2, tsl],
                                    start=(kp == 0), stop=(kp == 1),
                                    perf_mode=DR)
                            if (hc + ts) % 2 == 0:
                                nc.scalar.activation(
                                    h1T8[:, hc, tsl], ps, AFT.Identity,
                                    scale=S_H / (S_X * S_W),
                                    bias=bias_cp["bin"][:, e, hc:hc + 1])
                            else:
                                nc.vector.tensor_scalar(
                                    h1T8[:, hc, tsl], ps,
                                    S_H / (S_X * S_W),
                                    bias_cp["bin"][:, e, hc:hc + 1],
                                    op0=MUL, op1=ADD)
                    if post_h1 is not None:
                        post_h1()
                    # SwiGLU: s = silu(h@w1b + b1b) * (h@w1a + b1a)
                    sT8 = ac.tile([P, 16, T], F8, tag="sT8", bufs=1,
                                  name="sT8")
                    for pr in range(16):
                        if e == 0 and half == 0 and pr < 3:
                            w1_t = pre_w1[pr]
                        else:
                            w1_t = ws.tile([P, 2, 8, 2, P], F8, tag="w1s",
                                           bufs=4, name="w1_t")
                            nc.sync.dma_start(w1_t, w1_d[e, pr])
                        for ts in ts_range:
                            tsl = slice(ts * D, (ts + 1) * D)
                            pa = psE.tile([P, D], F32, tag="mm")
                            pb = psE.tile([P, D], F32, tag="mm")
                            for kp in range(8):
                                nc.tensor.matmul(
                                    pa, w1_t[:, 0, kp, :, :],
                                    h1T8[:, 2 * kp:2 * kp + 2, tsl],
                                    start=(kp == 0), stop=(kp == 7),
                                    perf_mode=DR)
                            for kp in range(8):
                                nc.tensor.matmul(
                                    pb, w1_t[:, 1, kp, :, :],
                                    h1T8[:, 2 * kp:2 * kp + 2, tsl],
                                    start=(kp == 0), stop=(kp == 7),
                                    perf_mode=DR)
                            sil = ev_.tile([P, D], BF16, tag="sil", bufs=4)
                            nc.scalar.activation(
                                sil, pb, AFT.Silu,
                                scale=1.0 / (S_H * S_W),
                                bias=bias_cp["b1b"][:, e, pr:pr + 1])
                            av8 = ev_.tile([P, D], F8, tag="av8", bufs=4)
                            if (pr + ts) % 4 != 3:
                                nc.scalar.activation(
                                    av8, pa, AFT.Identity,
                                    scale=S_S / (S_H * S_W),
                                    bias=bias_cp["b1a"][:, e, pr:pr + 1])
                            else:
                                nc.vector.tensor_scalar(
                                    av8, pa, S_S / (S_H * S_W),
                                    bias_cp["b1a"][:, e, pr:pr + 1],
                                    op0=MUL, op1=ADD)
                            ltsl = slice((ts - 2 * half) * D,
                                         (ts - 2 * half + 1) * D)
                            nc.vector.tensor_mul(sT8[:, pr, ltsl], av8, sil)
                    # o = s @ w2 + b2  (fp8, S_O)
                    oT8 = ac.tile([P, 16, T], F8, tag="oT8", bufs=1,
                                  name="oT8")
                    for og in range(4):
                        w2_t = ws.tile([P, 4, 8, 2, P], F8, tag="w2s",
                                       bufs=2, name="w2_t")
                        nc.sync.dma_start(w2_t, w2_d[e, og])
                        for oc4 in range(4):
                            oc = og * 4 + oc4
                            for ts in ts_range:
                                ltsl = slice((ts - 2 * half) * D,
                                             (ts - 2 * half + 1) * D)
                                ps = psE.tile([P, D], F32, tag="mm")
                                for kp in range(8):
                                    nc.tensor.matmul(
                                        ps, w2_t[:, oc4, kp, :, :],
                                        sT8[:, 2 * kp:2 * kp + 2, ltsl],
                                        start=(kp == 0), stop=(kp == 7),
                                        perf_mode=DR)
                                if (oc + ts) % 4 != 3:
                                    nc.vector.tensor_scalar(
                                        oT8[:, oc, ltsl], ps,
                                        S_O / (S_S * S_W),
                                        bias_cp["b2"][:, e, oc:oc + 1],
                                        op0=MUL, op1=ADD)
                                else:
                                    nc.scalar.activation(
                                        oT8[:, oc, ltsl], ps, AFT.Identity,
                                        scale=S_O / (S_S * S_W),
                                        bias=bias_cp["b2"][:, e, oc:oc + 1])
                    # gate broadcast [tokens] -> [P, D] per ts chunk
                    gb_tiles = {}
                    for ts in ts_range:
                        tsl = slice(ts * D, (ts + 1) * D)
                        pg = psB.tile([P, D], F32, tag="bc")
                        nc.tensor.matmul(pg, sel_sb[:, e, :], gatesT[:, tsl],
                                         start=True, stop=True)
                        gb = ev_.tile([P, D], BF16, tag="gb", bufs=4)
                        nc.scalar.activation(gb, pg, AFT.Copy)
                        gb_tiles[ts] = gb
                    # eo = o @ w_out + b_out; moeT += gate * eo
                    for dc in range(4):
                        for ts in ts_range:
                            tsl = slice(ts * D, (ts + 1) * D)
                            ltsl = slice((ts - 2 * half) * D,
                                         (ts - 2 * half + 1) * D)
                            ps = psE.tile([P, D], F32, tag="mm")
                            for kp in range(8):
                                nc.tensor.matmul(
                                    ps, wout_t[:, kp, dc, :, :],
                                    oT8[:, 2 * kp:2 * kp + 2, ltsl],
                                    start=(kp == 0), stop=(kp == 7),
                                    perf_mode=DR)
                            eo = ev_.tile([P, D], F32, tag="eo", bufs=3)
                            nc.scalar.activation(
                                eo, ps, AFT.Identity,
                                scale=1.0 / (S_O * S_W),
                                bias=bias_cp["bout"][:, e, dc:dc + 1])
                            t2 = ev_.tile([P, D], F32, tag="t2", bufs=3)
                            nc.vector.tensor_mul(t2, eo, gb_tiles[ts])
                            if (dc + ts) % 2 == 0:
                                nc.vector.tensor_add(moeT[:, dc, tsl],
                                                     moeT[:, dc, tsl], t2)
                            else:
                                nc.gpsimd.tensor_add(moeT[:, dc, tsl],
                                                     moeT[:, dc, tsl], t2)
                            if e == 1:
                                nc.sync.dma_start(
                                    rs_in[half][dc * P:(dc + 1) * P,
                                                (ts - 2 * half) * D:
                                                (ts - 2 * half + 1) * D],
                                    moeT[:, dc, tsl])
                    if e == 1:
                        nc.gpsimd.collective_compute(
                            "ReduceScatter", mybir.AluOpType.add,
                            ins=[rs_in[half][:]], outs=[rs_out[half][:]],
                            replica_groups=groups)
                        nc.sync.dma_start(
                            out_d[:, half * T:(half + 1) * T],
                            rs_out[half][:, :])

                emit_batch(0)
                emit_pass(0, 0, post_h1=lambda: emit_gates(0))
                emit_batch(1)
                emit_pass(0, 1, post_h1=lambda: emit_gates(1))
                emit_pass(1, 0)
                emit_pass(1, 1)
            s1pre.release()

            ws.release()

    _split_matmul_waits(nc)
    return nc


def _split_matmul_waits(nc):
    """walrus allows only one sync-wait per engine-instruction sync slot; move
    extra waits onto standalone InstEventSemaphore waits inserted before."""
    import concourse.mybir as mybir
    k = 0
    for bb in nc.main_func.blocks:
        il = list(bb.instructions)
        out = []
        changed = False
        for ins in il:
            si = getattr(ins, "sync_info", None)
            if si is not None and len(si.on_wait) > 1 \
                    and type(ins).__name__ != "InstEventSemaphore":
                waits = list(si.on_wait)
                keep, move = waits[-1], waits[:-1]
                for w in move:
                    nop = mybir.InstEventSemaphore(name=f"I-wsplit-{k}",
                                                   ins=[], outs=[])
                    k += 1
                    nop.engine = ins.engine
                    nop.sync_info = type(si)(on_wait=[w], on_update=[])
                    out.append(nop)
                ins.sync_info = type(si)(on_wait=[keep],
                                         on_update=list(si.on_update))
                changed = True
            out.append(ins)
        if changed:
            bb.instructions = out


def _q8w(w):
    """host fp8 cast with fixed 2^11 scale (clipped to TRN e4m3 max)."""
    return np.clip(np.asarray(w, np.float32) * S_W, -240.0, 240.0).astype(
        ml_dtypes.float8_e4m3)


def _prep_inputs(inputs, core):
    bf = ml_dtypes.bfloat16
    f32 = np.float32
    h = core
    x = np.asarray(inputs["x"], f32).reshape(N, D)
    xT = np.ascontiguousarray(x.T)                      # [512, 2048]
    g = np.asarray(inputs["g"], f32)
    bvec = np.asarray(inputs["b"], f32)
    caw = np.asarray(inputs["c_attn_w"], f32)
    cab = np.asarray(inputs["c_attn_b"], f32)
    wq = caw[:, h * 64:(h + 1) * 64]
    wk = caw[:, 512 + h * 64:512 + (h + 1) * 64]
    wv = caw[:, 1024 + h * 64:1024 + (h + 1) * 64]
    wqkv = np.concatenate([wq, wk, wv], axis=1)          # [512, 192]
    # RMSNorm additive b folded into qkv biases
    bq = bvec @ wq + cab[h * 64:(h + 1) * 64]
    bk = bvec @ wk + cab[512 + h * 64:512 + (h + 1) * 64]
    bv = bvec @ wv + cab[1024 + h * 64:1024 + (h + 1) * 64]
    kk = np.arange(4)[None, :, None] * P + np.arange(P)[:, None, None]
    qq = np.arange(D)[None, None, :]
    maskd = (kk <= qq).astype(f32)                       # [128, 4, 512]
    projb = np.asarray(inputs["c_proj_b"], f32)
    projb_col = np.ascontiguousarray(projb.reshape(4, P).T)  # [p, dc]
    c0 = 1.0 if core == 0 else 0.0
    xmask = np.zeros((D, N), f32)
    xmask[64 * core:64 * core + 64, :] = xT[64 * core:64 * core + 64, :]
    xmask += (projb * c0)[:, None]
    xmask = np.ascontiguousarray(
        xmask.reshape(4, P, N).transpose(1, 0, 2)).astype(bf)
    selb = np.zeros((E, EL, P), f32)
    selb[2 * core, 0, :] = 1.0
    selb[2 * core + 1, 1, :] = 1.0

    sl = slice(2 * core, 2 * core + 2)
    w_in = np.asarray(inputs["w_in"], f32)[sl]           # [2, 512, 2048]
    w1 = np.asarray(inputs["w1"], f32)[sl]               # [2, 2048, 4096]
    w2 = np.asarray(inputs["w2"], f32)[sl]               # [2, 2048, 2048]
    w_out = np.asarray(inputs["w_out"], f32)[sl]         # [2, 2048, 512]

    # DoubleRow lhsT layouts (pair index j adjacent to the 128-wide m dim)
    w_in8 = np.ascontiguousarray(
        _q8w(w_in).reshape(EL, 2, 2, P, 16, P)
        .transpose(0, 3, 1, 4, 2, 5))                    # [EL, p, kp, hc, j, m]
    w1a = _q8w(w1[:, :, :HD]).reshape(EL, 8, 2, P, 16, P)
    w1b = _q8w(w1[:, :, HD:]).reshape(EL, 8, 2, P, 16, P)
    w18 = np.stack([w1a, w1b], axis=2)                   # [EL, kp, ab, j, p, pr, m]
    w18 = np.ascontiguousarray(
        w18.transpose(0, 5, 4, 2, 1, 3, 6))              # [EL, pr, p, ab, kp, j, m]
    w28 = _q8w(w2).reshape(EL, 8, 2, P, 16, P) \
        .transpose(0, 4, 3, 1, 2, 5)                     # [EL, oc, p, kp, j, m]
    w28 = np.ascontiguousarray(
        w28.reshape(EL, 4, 4, P, 8, 2, P)
        .transpose(0, 1, 3, 2, 4, 5, 6))                 # [EL, og, p, ocl, kp, j, m]
    wout8 = np.ascontiguousarray(
        _q8w(w_out).reshape(EL, 8, 2, P, 4, P)
        .transpose(0, 3, 1, 4, 2, 5))                    # [EL, p, kp, dc, j, m]

    def bias_t(key, scale, w):
        b = np.asarray(inputs[key], f32)[sl] * scale     # [2, w*128]
        return np.ascontiguousarray(b.reshape(EL, w, P).transpose(2, 0, 1))

    mfeat = np.zeros((D,), f32)
    mfeat[64 * core:64 * core + 64] = 1.0
    smalls = np.concatenate([
        np.ascontiguousarray(mfeat.reshape(4, P).T),
        np.ascontiguousarray((projb * c0).reshape(4, P).T),
        np.broadcast_to(np.asarray(inputs["router_b"], f32), (P, E)),
        projb_col,
        projb_col * c0,
        np.full((P, 1), c0 / S_Y, f32),
        np.broadcast_to(bv, (P, HDIM)),
    ], axis=1).astype(f32)                               # [P, 97]
    bqa = np.zeros((HDIM, 4), f32)
    bqa[:, 0] = bq
    bqa[:, 1] = bk
    bqa[0, 3] = np.asarray(inputs["alpha"], f32)[h]
    wbig = np.concatenate([
        wqkv.reshape(4, P, 192).transpose(1, 0, 2).reshape(P, 768),
        maskd.reshape(P, 4 * D),
        np.asarray(inputs["router_w"], f32)
        .reshape(4, P, E).transpose(1, 0, 2).reshape(P, 4 * E),
    ], axis=1).astype(bf)                                # [P, 2880]
    b1 = bias_t("b1", 1.0, 32)
    ebias = np.concatenate([
        bias_t("b_in", S_H, 16),
        b1[:, :, :16] * S_S,
        b1[:, :, 16:],
        bias_t("b2", S_O, 16),
        bias_t("b_out", 1.0, 4),
    ], axis=2).astype(f32)                               # [P, 2, 68]
    return {
        "xtb": np.ascontiguousarray(
            xT.reshape(4, P, N).transpose(1, 0, 2)).astype(bf),
        "xmask": xmask,
        "smalls": smalls,
        "bqalpha": bqa,
        "wbig": wbig,
        "grow": np.concatenate(
            [g, np.full((HDIM,), np.asarray(inputs["alpha"], f32)[h])]
        ).reshape(1, D + HDIM).astype(bf),
        "selb": selb.astype(bf),
        "w_in8": w_in8, "w18": w18, "w28": w28, "wout8": wout8,
        "ebias": ebias,
    }


last_result = [None]


def kernel(**inputs):
    if "nc" not in _cache:
        _cache["nc"] = build_program()
    nc = _cache["nc"]
    in_maps = [_prep_inputs(inputs, c) for c in range(NCORES)]
    res = run_bass_kernel_spmd(nc, in_maps, core_ids=list(range(NCORES)))
    last_result[0] = res
    outT = np.concatenate(
        [np.asarray(res.results[c]["out"]).astype(np.float32)
         for c in range(NCORES)], axis=0)                # [512, 2048]
    return np.ascontiguousarray(outT.T).reshape(2, 1024, 512)


# revision 47
# speedup vs baseline: 1.0981x; 1.0042x over previous
"""MoE transformer block (QK-norm attention + top-8-of-16 MoE) on 8 trn2 cores.

v4: dense fp8 DoubleRow experts (as v2 baseline) with restructured
scheduling:
- batch-split expert pipeline: each expert runs a b0-pass (tokens 0-1023)
  then a b1-pass, so expert-0's b0 compute starts right after AllGather-0
  and fully hides AllGather-1 + proj-b1;
- the ReduceScatter is split per batch: RS(b0)'s input is complete after
  the last expert's b0-pass and it runs hidden under ~95us of b1 compute,
  leaving only RS(b1) (~18us) in the tail;
- attention-phase cost cuts: softmax denominator folded into an augmented
  v row (kills 24 denominator matmuls), all broadcast matmuls in bf16
  (1 cyc/row instead of 4), RMSNorm's g folded into the broadcast and its
  additive b folded into the qkv biases (host-side);
- SwiGLU's a-path and the moe combine adds alternate DVE/ACT/GpSimd to
  keep the vector engine off the critical path;
- startup DMAs split so RMSNorm starts after the first 0.5MB chunk.

Sharding: attention head-parallel (core c owns head c), experts
expert-parallel (core c owns experts 2c, 2c+1), output feature-parallel
(core c returns features [64c, 64c+64) for all tokens).

Everything feature-major ("T layout": features on partitions, tokens on
free). Scales (powers of two, exact): x_res*2^5, weights*2^11, h1*2^5,
s*2^5, o*2^7; descales folded into Act/DVE scale+bias immediates.
"""

import numpy as np
import ml_dtypes

import concourse.bass as bass
import concourse.mybir as mybir
from concourse.tile import TileContext
from concourse.masks import make_identity
from concourse.bass_utils import run_bass_kernel_spmd

BF16 = mybir.dt.bfloat16
F32 = mybir.dt.float32
F8 = mybir.dt.float8e4
AFT = mybir.ActivationFunctionType
MUL = mybir.AluOpType.mult
ADD = mybir.AluOpType.add
DR = mybir.MatmulPerfMode.DoubleRow

P = 128
D = 512          # embed dim
T = 1024         # tokens per batch
N = 2048         # total tokens
E = 16           # experts
EL = 2           # experts per core
HD = 2048        # expert hidden
HDIM = 64        # head dim
NCORES = 8

S_X = 2.0 ** 5
S_W = 2.0 ** 11
S_H = 2.0 ** 5
S_S = 2.0 ** 5
S_O = 2.0 ** 7
S_Y = 2.0 ** 5

_cache = {}


def build_program():
    nc = bass.Bass()
    dp_ = dict(isOutput=False)
    xtb_d = nc.declare_dram_parameter("xtb", [P, 4, N], BF16, **dp_)
    xm_d = nc.declare_dram_parameter("xmask", [P, 4, N], BF16, **dp_)
    sm_d = nc.declare_dram_parameter("smalls", [P, 97], F32, **dp_)
    bqa_d = nc.declare_dram_parameter("bqalpha", [HDIM, 4], F32, **dp_)
    wbig_d = nc.declare_dram_parameter("wbig", [P, 2880], BF16, **dp_)
    grow_d = nc.declare_dram_parameter("grow", [1, D + HDIM], BF16,
                                       **dp_)
    sel_d = nc.declare_dram_parameter("selb", [E, EL, P], BF16, **dp_)
    win_d = nc.declare_dram_parameter("w_in8", [EL, P, 2, 16, 2, P], F8, **dp_)
    w1_d = nc.declare_dram_parameter("w18", [EL, 16, P, 2, 8, 2, P], F8, **dp_)
    w2_d = nc.declare_dram_parameter("w28", [EL, 4, P, 4, 8, 2, P], F8, **dp_)
    wout_d = nc.declare_dram_parameter("wout8", [EL, P, 8, 4, 2, P], F8, **dp_)
    eb_d = nc.declare_dram_parameter("ebias", [P, EL, 68], F32, **dp_)
    out_d = nc.declare_dram_parameter("out", [HDIM, N], BF16, isOutput=True)

    groups = [list(range(NCORES))]

    with TileContext(nc, num_cores=NCORES) as tc:
        with (
            tc.tile_pool(name="const", bufs=1) as cp,
            tc.tile_pool(name="pp", bufs=1) as pp,
            tc.tile_pool(name="psE", bufs=4, space="PSUM") as psE,
            tc.tile_pool(name="psB", bufs=2, space="PSUM") as psB,
            tc.tile_pool(name="psS", bufs=2, space="PSUM") as psS,
            tc.tile_pool(name="dram", bufs=1, space="DRAM") as dp,
        ):
            # ---- constants / persistent ----
            ws = tc.alloc_tile_pool(name="wst", bufs=1)
            s1pre = tc.alloc_tile_pool(name="s1pre", bufs=1)
            xTb = s1pre.tile([P, 4, N], BF16, tag="xTb")
            nc.sync.dma_start(xTb[:, :, 0:D], xtb_d[:, :, 0:D])
            wbig = s1pre.tile([P, 2880], BF16, tag="wbig")
            nc.sync.dma_start(wbig[:, 0:768], wbig_d[:, 0:768])
            for cc in range(1, 4):
                sl = slice(cc * D, (cc + 1) * D)
                nc.sync.dma_start(xTb[:, :, sl], xtb_d[:, :, sl])
            ones128b = cp.tile([P, 1], BF16, tag="ones128b")
            nc.vector.memset(ones128b, 1.0)
            ones64b = cp.tile([HDIM, 1], BF16, tag="ones64b")
            nc.vector.memset(ones64b, 1.0)
            ones1rb = cp.tile([1, P], BF16, tag="ones1rb")
            nc.vector.memset(ones1rb, 1.0)
            sm = cp.tile([P, 97], F32, tag="sm")
            nc.sync.dma_start(sm, sm_d[:, :])
            mfeat_sb = sm[:, 0:4]
            pbc0_sb = sm[:, 4:8]
            rb_sb = sm[:, 8:24]
            projb_sb = sm[:, 24:28]
            c0_sb = sm[:, 32:33]
            vbias_sb = sm[:, 33:97]
            sel_sb = cp.tile([E, EL, P], BF16, tag="sel_sb")
            nc.sync.dma_start(sel_sb, sel_d[:, :, :])
            eps6 = cp.tile([1, 1], F32, tag="eps6")
            nc.vector.memset(eps6, 1e-6)
            grow = cp.tile([1, D + HDIM], BF16, tag="grow")
            nc.sync.dma_start(grow, grow_d[:, :])
            identb = cp.tile([P, P], BF16, tag="identb")
            make_identity(nc, identb)

            # persistent activations for the expert phase
            moeT = pp.tile([P, 4, N], BF16, tag="moeT")
            xrT8 = pp.tile([P, 4, N], F8, tag="xrT8")
            gatesT = pp.tile([E, N], BF16, tag="gatesT")
            h1T8 = pp.tile([P, 16, N], F8, tag="h1T8")

            ag_in = [dp.tile([HDIM, T], F8, name=f"ag_in{i}")
                     for i in range(2)]
            ag_out = [dp.tile([D, T], F8, addr_space="Shared",
                              name=f"ag_out{i}")
                      for i in range(2)]
            rs_in = [dp.tile([D, T], BF16, name=f"rs_in{i}")
                     for i in range(2)]
            rs_out = [dp.tile([HDIM, T], BF16, name=f"rs_out{i}")
                      for i in range(2)]

            with tc.tile_pool(name="s1", bufs=1) as s1:
                nc.sync.dma_start(wbig[:, 768:2880], wbig_d[:, 768:2880])
                bqa = s1.tile([HDIM, 4], F32, tag="bqa")
                nc.sync.dma_start(bqa, bqa_d[:, :])
                bq_sb = bqa[:, 0:3]
                alpha_sb = bqa[0:1, 3:4]
                # prefetch: expert biases + e0 weights (consumed ~90us later)
                ebt = pp.tile([P, EL, 68], F32, tag="ebias")
                nc.sync.dma_start(ebt, eb_d[:, :, :])
                bias_cp = {"bin": ebt[:, :, 0:16], "b1a": ebt[:, :, 16:32],
                           "b1b": ebt[:, :, 32:48], "b2": ebt[:, :, 48:64],
                           "bout": ebt[:, :, 64:68]}
                pre_win = ws.tile([P, 2, 16, 2, P], F8, tag="win", bufs=1)
                nc.sync.dma_start(pre_win, win_d[0])
                pre_wout = ws.tile([P, 8, 4, 2, P], F8, tag="wot", bufs=1)
                nc.sync.dma_start(pre_wout, wout_d[0])
                pre_w1 = []
                for pr in range(3):
                    t = ws.tile([P, 2, 8, 2, P], F8, tag="w1s", bufs=4)
                    nc.sync.dma_start(t, w1_d[0, pr])
                    pre_w1.append(t)
                # ---- RMSNorm + qkv + qk-norm + attention, batch-ordered:
                # all of batch b's chain runs before batch b+1 so AG(b)
                # issues early and b1 prep fills the AG0 window ----
                with tc.tile_pool(name="attp", bufs=1) as ap_, \
                     tc.tile_pool(name="ate", bufs=12) as ate:
                    xnT = ap_.tile([P, 4, N], BF16, tag="xnT")
                    qT = ap_.tile([HDIM, N], BF16, tag="qT")
                    kT = ap_.tile([HDIM, N], BF16, tag="kT")
                    v_aug = ap_.tile([P, 16, HDIM + 1], BF16, tag="v_aug")
                    nc.vector.memset(v_aug, 1.0)
                    qh = ap_.tile([HDIM, N], BF16, tag="qh")
                    kh = ap_.tile([HDIM, N], BF16, tag="kh")
                    yhT = ap_.tile([HDIM, N], F8, tag="yhT")
                    for b in range(2):
                        for cc in (2 * b, 2 * b + 1):
                            sl = slice(cc * D, (cc + 1) * D)
                            ps = psS.tile([1, D], F32, tag="ps_small")
                            for kc in range(4):
                                sq = ate.tile([P, D], BF16, tag="sq_t",
                                              bufs=3)
                                nc.scalar.activation(sq, xTb[:, kc, sl],
                                                     AFT.Square)
                                nc.tensor.matmul(ps, ones128b, sq,
                                                 start=(kc == 0),
                                                 stop=(kc == 3))
                            tmp = ate.tile([1, D], F32, tag="r_t", bufs=2)
                            nc.scalar.activation(tmp, ps, AFT.Sqrt,
                                                 scale=1.0 / D,
                                                 bias=eps6[0:1, 0:1])
                            rrow = ate.tile([1, D], BF16, tag="rrow",
                                            bufs=2)
                            with nc.allow_low_precision(
                                    reason="bf16 bcast row"):
                                nc.vector.reciprocal(rrow, tmp)
                            # xnT = xTb * bcast(rrow * g)
                            for kc in range(4):
                                pb = psB.tile([P, D], F32, tag="bc")
                                nc.tensor.matmul(
                                    pb, grow[0:1, kc * P:(kc + 1) * P],
                                    rrow[0:1, :], start=True, stop=True)
                                nc.vector.tensor_mul(xnT[:, kc, sl],
                                                     xTb[:, kc, sl], pb)
                            for wi, dst, bi in ((0, qT, 0), (1, kT, 1)):
                                ps2 = psS.tile([HDIM, D], F32,
                                               tag="ps_small")
                                for kc in range(4):
                                    nc.tensor.matmul(
                                        ps2,
                                        wbig[:, kc * 192 + wi * HDIM:
                                             kc * 192 + (wi + 1) * HDIM],
                                        xnT[:, kc, sl], start=(kc == 0),
                                        stop=(kc == 3))
                                nc.scalar.activation(
                                    dst[:, sl], ps2, AFT.Identity,
                                    bias=bq_sb[:, bi:bi + 1])
                            for tk in range(cc * 4, cc * 4 + 4):
                                ps3 = psS.tile([P, HDIM], F32,
                                               tag="ps_small")
                                for kc in range(4):
                                    nc.tensor.matmul(
                                        ps3, xnT[:, kc, tk * P:(tk + 1) * P],
                                        wbig[:, kc * 192 + 128:
                                             kc * 192 + 192],
                                        start=(kc == 0), stop=(kc == 3))
                                nc.vector.tensor_add(v_aug[:, tk, 0:HDIM],
                                                     ps3, vbias_sb)
                            # qk-norm for this chunk (alpha folded in the
                            # q bcast row)
                            for src_, dst, brow in (
                                    (qT, qh, grow[0:1, D:D + HDIM]),
                                    (kT, kh, ones1rb[0:1, 0:HDIM])):
                                sq = ate.tile([HDIM, D], BF16, tag="sqn",
                                              bufs=2)
                                nc.scalar.activation(sq, src_[:, sl],
                                                     AFT.Square)
                                ps4 = psS.tile([1, D], F32, tag="ps_small")
                                nc.tensor.matmul(ps4, ones64b, sq,
                                                 start=True, stop=True)
                                t = ate.tile([1, D], F32, tag="rn_t",
                                             bufs=2)
                                nc.scalar.activation(t, ps4, AFT.Sqrt)
                                nc.vector.tensor_scalar_add(t, t, 1e-5)
                                rn = ate.tile([1, D], BF16, tag="rn",
                                              bufs=2)
                                with nc.allow_low_precision(
                                        reason="bf16 row"):
                                    nc.vector.reciprocal(rn, t)
                                pb = psB.tile([HDIM, D], F32, tag="bc")
                                nc.tensor.matmul(pb, brow, rn[0:1, :],
                                                 start=True, stop=True)
                                nc.vector.tensor_mul(dst[:, sl],
                                                     src_[:, sl], pb)
                        # scoresT -> exp (masked diag) -> AV (denominator
                        # folded into v_aug's ones row) -> yhT -> AG(b)
                        for qc in range(2):
                            qsl = slice(b * T + qc * D, b * T + (qc + 1) * D)
                            nkc = 4 * qc + 4
                            ex_tiles = []
                            for kc in range(nkc):
                                ksl = slice(b * T + kc * P,
                                            b * T + (kc + 1) * P)
                                ps = psE.tile([P, D], F32, tag="mm")
                                nc.tensor.matmul(ps, kh[:, ksl], qh[:, qsl],
                                                 start=True, stop=True)
                                if kc >= 4 * qc:  # diagonal block: mask
                                    et = ate.tile([P, D], BF16, tag="exp_b",
                                                  bufs=4)
                                    nc.scalar.activation(et, ps, AFT.Exp)
                                    eb2 = ate.tile([P, D], BF16,
                                                   tag="exp_m", bufs=7)
                                    nc.vector.tensor_mul(
                                        eb2, et,
                                        wbig[:, 768 + (kc - 4 * qc) * D:
                                             768 + (kc - 4 * qc + 1) * D])
                                else:
                                    eb2 = ate.tile([P, D], BF16,
                                                   tag="exp_m", bufs=7)
                                    nc.scalar.activation(eb2, ps, AFT.Exp)
                                ex_tiles.append(eb2)
                            py = psS.tile([HDIM + 1, D], F32,
                                          tag="ps_small")
                            for kc in range(nkc):
                                nc.tensor.matmul(py, v_aug[:, b * 8 + kc, :],
                                                 ex_tiles[kc],
                                                 start=(kc == 0),
                                                 stop=(kc == nkc - 1))
                            dr = ate.tile([1, D], BF16, tag="dr", bufs=2)
                            with nc.allow_low_precision(
                                    reason="bf16 softmax denom"):
                                nc.vector.reciprocal(dr,
                                                     py[HDIM:HDIM + 1, :])
                            pb2 = psB.tile([HDIM, D], F32, tag="bc")
                            nc.tensor.matmul(pb2, ones1rb[0:1, 0:HDIM], dr,
                                             start=True, stop=True)
                            db = ate.tile([HDIM, D], BF16, tag="db", bufs=2)
                            nc.scalar.activation(db, pb2, AFT.Copy,
                                                 scale=S_Y)
                            nc.vector.tensor_mul(yhT[:, qsl],
                                                 py[0:HDIM, :], db)
                            nc.sync.dma_start(
                                ag_in[b][:, qc * D:(qc + 1) * D],
                                yhT[:, qsl])
                        nc.gpsimd.collective_compute(
                            "AllGather", mybir.AluOpType.bypass,
                            ins=[ag_in[b][:]], outs=[ag_out[b][:]],
                            replica_groups=groups)

                # ---- proj + x_res + router (per batch), interleaved with
                # expert passes so weight DMAs never queue behind the
                # AG1-blocked b1 input DMAs ----
                pass
            # (s1 stays open: xTb / qT / kT / v_aug / wbig / bqa)
            with tc.tile_pool(name="s2", bufs=1) as s2, \
                 tc.tile_pool(name="s2e", bufs=4) as s2e, \
                 tc.tile_pool(name="eact", bufs=1) as ac, \
                 tc.tile_pool(name="eev", bufs=6) as ev_:
                yT_sb = s2.tile([P, 4, N], F8, tag="yT_sb")
                routes = s2.tile([P, 16, E], F32, tag="routes")
                gates = routes
                gsum = s2.tile([P, 16], F32, tag="gsum")
                gates_bf = s2.tile([P, 16, E], BF16, tag="gates_bf")

                def emit_batch(b):
                    for ts in (2 * b, 2 * b + 1):
                        nc.sync.dma_start(
                            yT_sb[:, :, ts * D:(ts + 1) * D],
                            ag_out[b][:, (ts % 2) * D:(ts % 2 + 1) * D]
                            .rearrange("(kc p) n -> p kc n", p=P))
                    for ts in (2 * b, 2 * b + 1):
                        for dc in range(4):
                            tsl = slice(ts * D, (ts + 1) * D)
                            ps = psE.tile([P, D], F32, tag="mm")
                            for kc in range(4):
                                nc.tensor.matmul(
                                    ps,
                                    wbig[:, 2816 + kc * D + dc * P:
                                         2816 + kc * D + (dc + 1) * P],
                                    yT_sb[:, kc, tsl],
                                    start=(kc == 0), stop=(kc == 3))
                            # xmb = x*featmask + projb*c0 (replaces the
                            # host xmask tensor)
                            xmb = s2e.tile([P, D], BF16, tag="xmb", bufs=3)
                            nc.vector.tensor_scalar(
                                xmb, xTb[:, dc, tsl],
                                mfeat_sb[:, dc:dc + 1],
                                pbc0_sb[:, dc:dc + 1], op0=MUL, op1=ADD)
                            tmp = s2e.tile([P, D], F32, tag="yp_t", bufs=3)
                            nc.scalar.activation(
                                tmp, ps, AFT.Identity, scale=1.0 / S_Y,
                                bias=projb_sb[:, dc:dc + 1])
                            # x_res written in place over xTb
                            nc.vector.tensor_add(xTb[:, dc, tsl], tmp,
                                                 xTb[:, dc, tsl])
                            nc.scalar.activation(xrT8[:, dc, tsl],
                                                 xTb[:, dc, tsl],
                                                 AFT.Copy, scale=S_X)
                            # moeT init: yp*c0 + x feature slice
                            nc.vector.scalar_tensor_tensor(
                                moeT[:, dc, tsl], ps, c0_sb[:, 0:1],
                                xmb, op0=MUL, op1=ADD)
                def emit_gates(b):
                    # router for this batch -> normalized top-8 gates
                    for tk in range(8 * b, 8 * b + 8):
                        ps = psS.tile([P, E], F32, tag="ps_small")
                        for kc in range(4):
                            nc.tensor.matmul(
                                ps, xTb[:, kc, tk * P:(tk + 1) * P],
                                wbig[:, 2816 + kc * E:2816 + (kc + 1) * E],
                                start=(kc == 0), stop=(kc == 3))
                        nc.vector.tensor_add(routes[:, tk, :], ps, rb_sb)
                    nc.scalar.activation(routes[:, 8 * b:8 * b + 8, :],
                                         routes[:, 8 * b:8 * b + 8, :],
                                         AFT.Exp)
                    for g in range(8 * b, 8 * b + 8):
                        m8 = s2e.tile([P, 8], F32, tag="m8", bufs=2)
                        nc.vector.max(out=m8, in_=routes[:, g, :])
                        zap = s2e.tile([P, E], F32, tag="zap", bufs=2)
                        nc.vector.match_replace(out=zap, in_to_replace=m8,
                                                in_values=routes[:, g, :],
                                                imm_value=0)
                        nc.vector.tensor_sub(gates[:, g, :],
                                             routes[:, g, :], zap)
                    nc.vector.reduce_sum(gsum[:, 8 * b:8 * b + 8],
                                         gates[:, 8 * b:8 * b + 8, :],
                                         axis=mybir.AxisListType.X)
                    nc.vector.reciprocal(gsum[:, 8 * b:8 * b + 8],
                                         gsum[:, 8 * b:8 * b + 8])
                    for g in range(8 * b, 8 * b + 8):
                        nc.vector.tensor_scalar_mul(gates_bf[:, g, :],
                                                    gates[:, g, :],
                                                    gsum[:, g:g + 1])
                    for g in range(8 * b, 8 * b + 8):
                        pt = psS.tile([E, P], BF16, tag="ps_small")
                        nc.tensor.transpose(pt, gates_bf[:, g, :], identb)
                        nc.scalar.activation(
                            gatesT[:, g * P:(g + 1) * P], pt, AFT.Copy)

                held = {}

                def emit_pass(e, half, post_h1=None):
                    ts_range = (2 * half, 2 * half + 1)
                    if half == 0:
                        if e == 0:
                            win_t = pre_win
                            wout_t = pre_wout
                        else:
                            win_t = ws.tile([P, 2, 16, 2, P], F8,
                                            tag="win", bufs=1, name="win_t")
                            nc.sync.dma_start(win_t, win_d[e])
                            wout_t = ws.tile([P, 8, 4, 2, P], F8,
                                             tag="wot", bufs=1,
                                             name="wout_t")
                            nc.sync.dma_start(wout_t, wout_d[e])
                        held[e] = (win_t, wout_t)
                    else:
                        win_t, wout_t = held[e]
                    # h1 = x_res @ w_in  (fp8, S_H)
                    for ts in ts_range:
                        for hc in range(16):
                            tsl = slice(ts * D, (ts + 1) * D)
                            ps = psE.tile([P, D], F32, tag="mm")
                            for kp in range(2):
                                nc.tensor.matmul(
                                    ps, win_t[:, kp, hc, :, :],
                                    xrT8[:, 2 * kp:2 * kp + 2, tsl],
                                    start=(kp == 0), stop=(kp == 1),
                                    perf_mode=DR)
                            if (hc + ts) % 2 == 0:
                                nc.scalar.activation(
                                    h1T8[:, hc, tsl], ps, AFT.Identity,
                                    scale=S_H / (S_X * S_W),
                                    bias=bias_cp["bin"][:, e, hc:hc + 1])
                            else:
                                nc.vector.tensor_scalar(
                                    h1T8[:, hc, tsl], ps,
                                    S_H / (S_X * S_W),
                                    bias_cp["bin"][:, e, hc:hc + 1],
                                    op0=MUL, op1=ADD)
                    if post_h1 is not None:
                        post_h1()
                    # SwiGLU: s = silu(h@w1b + b1b) * (h@w1a + b1a)
                    sT8 = ac.tile([P, 16, T], F8, tag="sT8", bufs=1,
                                  name="sT8")
                    for pr in range(16):
                        if e == 0 and half == 0 and pr < 3:
                            w1_t = pre_w1[pr]
                        else:
                            w1_t = ws.tile([P, 2, 8, 2, P], F8, tag="w1s",
                                           bufs=4, name="w1_t")
                            nc.sync.dma_start(w1_t, w1_d[e, pr])
                        for ts in ts_range:
                            tsl = slice(ts * D, (ts + 1) * D)
                            pa = psE.tile([P, D], F32, tag="mm")
                            pb = psE.tile([P, D], F32, tag="mm")
                            for kp in range(8):
                                nc.tensor.matmul(
                                    pa, w1_t[:, 0, kp, :, :],
                                    h1T8[:, 2 * kp:2 * kp + 2, tsl],
                                    start=(kp == 0), stop=(kp == 7),
                                    perf_mode=DR)
                            for kp in range(8):
                                nc.tensor.matmul(
                                    pb, w1_t[:, 1, kp, :, :],
                                    h1T8[:, 2 * kp:2 * kp + 2, tsl],
                                    start=(kp == 0), stop=(kp == 7),
                                    perf_mode=DR)
                            sil = ev_.tile([P, D], BF16, tag="sil", bufs=4)
                            nc.scalar.activation(
                                sil, pb, AFT.Silu,
                                scale=1.0 / (S_H * S_W),
                                bias=bias_cp["b1b"][:, e, pr:pr + 1])
                            av8 = ev_.tile([P, D], F8, tag="av8", bufs=4)
                            if (pr + ts) % 4 != 3:
                                nc.scalar.activation(
                                    av8, pa, AFT.Identity,
                                    scale=S_S / (S_H * S_W),
                                    bias=bias_cp["b1a"][:, e, pr:pr + 1])
                            else:
                                nc.vector.tensor_scalar(
                                    av8, pa, S_S / (S_H * S_W),
                                    bias_cp["b1a"][:, e, pr:pr + 1],
                                    op0=MUL, op1=ADD)
                            ltsl = slice((ts - 2 * half) * D,
                                         (ts - 2 * half + 1) * D)
                            nc.vector.tensor_mul(sT8[:, pr, ltsl], av8, sil)
                    # o = s @ w2 + b2  (fp8, S_O)
                    oT8 = ac.tile([P, 16, T], F8, tag="oT8", bufs=1,
                                  name="oT8")
                    for og in range(4):
                        w2_t = ws.tile([P, 4, 8, 2, P], F8, tag="w2s",
                                       bufs=2, name="w2_t")
                        nc.sync.dma_start(w2_t, w2_d[e, og])
                        for oc4 in range(4):
                            oc = og * 4 + oc4
                            for ts in ts_range:
                                ltsl = slice((ts - 2 * half) * D,
                                             (ts - 2 * half + 1) * D)
                                ps = psE.tile([P, D], F32, tag="mm")
                                for kp in range(8):
                                    nc.tensor.matmul(
                                        ps, w2_t[:, oc4, kp, :, :],
                                        sT8[:, 2 * kp:2 * kp + 2, ltsl],
                                        start=(kp == 0), stop=(kp == 7),
                                        perf_mode=DR)
                                if (oc + ts) % 4 != 3:
                                    nc.vector.tensor_scalar(
                                        oT8[:, oc, ltsl], ps,
                                        S_O / (S_S * S_W),
                                        bias_cp["b2"][:, e, oc:oc + 1],
                                        op0=MUL, op1=ADD)
                                else:
                                    nc.scalar.activation(
                                        oT8[:, oc, ltsl], ps, AFT.Identity,
                                        scale=S_O / (S_S * S_W),
                                        bias=bias_cp["b2"][:, e, oc:oc + 1])
                    # gate broadcast [tokens] -> [P, D] per ts chunk
                    gb_tiles = {}
                    for ts in ts_range:
                        tsl = slice(ts * D, (ts + 1) * D)
                        pg = psB.tile([P, D], F32, tag="bc")
                        nc.tensor.matmul(pg, sel_sb[:, e, :], gatesT[:, tsl],
                                         start=True, stop=True)
                        gb = ev_.tile([P, D], BF16, tag="gb", bufs=4)
                        nc.scalar.activation(gb, pg, AFT.Copy,
                                             scale=1.0 / (S_O * S_W))
                        gb_tiles[ts] = gb
                    # eo = o @ w_out + b_out; moeT += gate * eo
                    for dc in range(4):
                        for ts in ts_range:
                            tsl = slice(ts * D, (ts + 1) * D)
                            ltsl = slice((ts - 2 * half) * D,
                                         (ts - 2 * half + 1) * D)
                            ps = psE.tile([P, D], F32, tag="mm")
                            for kp in range(8):
                                nc.tensor.matmul(
                                    ps, wout_t[:, kp, dc, :, :],
                                    oT8[:, 2 * kp:2 * kp + 2, ltsl],
                                    start=(kp == 0), stop=(kp == 7),
                                    perf_mode=DR)
                            t2 = ev_.tile([P, D], F32, tag="t2", bufs=3)
                            nc.vector.scalar_tensor_tensor(
                                t2, ps, bias_cp["bout"][:, e, dc:dc + 1],
                                gb_tiles[ts], op0=ADD, op1=MUL)
                            if (dc + ts) % 2 == 0:
                                nc.vector.tensor_add(moeT[:, dc, tsl],
                                                     moeT[:, dc, tsl], t2)
                            else:
                                nc.gpsimd.tensor_add(moeT[:, dc, tsl],
                                                     moeT[:, dc, tsl], t2)
                            if e == 1:
                                nc.sync.dma_start(
                                    rs_in[half][dc * P:(dc + 1) * P,
                                                (ts - 2 * half) * D:
                                                (ts - 2 * half + 1) * D],
                                    moeT[:, dc, tsl])
                    if e == 1:
                        nc.gpsimd.collective_compute(
                            "ReduceScatter", mybir.AluOpType.add,
                            ins=[rs_in[half][:]], outs=[rs_out[half][:]],
                            replica_groups=groups)
                        nc.sync.dma_start(
                            out_d[:, half * T:(half + 1) * T],
                            rs_out[half][:, :])

                emit_batch(0)
                emit_pass(0, 0, post_h1=lambda: emit_gates(0))
                emit_batch(1)
                emit_pass(0, 1, post_h1=lambda: emit_gates(1))
                emit_pass(1, 0)
                emit_pass(1, 1)
            s1pre.release()

            ws.release()

    _split_matmul_waits(nc)
    return nc


def _split_matmul_waits(nc):
    """walrus allows only one sync-wait per engine-instruction sync slot; move
    extra waits onto standalone InstEventSemaphore waits inserted before."""
    import concourse.mybir as mybir
    k = 0
    for bb in nc.main_func.blocks:
        il = list(bb.instructions)
        out = []
        changed = False
        for ins in il:
            si = getattr(ins, "sync_info", None)
            if si is not None and len(si.on_wait) > 1 \
                    and type(ins).__name__ != "InstEventSemaphore":
                waits = list(si.on_wait)
                keep, move = waits[-1], waits[:-1]
                for w in move:
                    nop = mybir.InstEventSemaphore(name=f"I-wsplit-{k}",
                                                   ins=[], outs=[])
                    k += 1
                    nop.engine = ins.engine
                    nop.sync_info = type(si)(on_wait=[w], on_update=[])
                    out.append(nop)
                ins.sync_info = type(si)(on_wait=[keep],
                                         on_update=list(si.on_update))
                changed = True
            out.append(ins)
        if changed:
            bb.instructions = out


def _q8w(w):
    """host fp8 cast with fixed 2^11 scale (clipped to TRN e4m3 max)."""
    return np.clip(np.asarray(w, np.float32) * S_W, -240.0, 240.0).astype(
        ml_dtypes.float8_e4m3)


def _prep_inputs(inputs, core):
    bf = ml_dtypes.bfloat16
    f32 = np.float32
    h = core
    x = np.asarray(inputs["x"], f32).reshape(N, D)
    xT = np.ascontiguousarray(x.T)                      # [512, 2048]
    g = np.asarray(inputs["g"], f32)
    bvec = np.asarray(inputs["b"], f32)
    caw = np.asarray(inputs["c_attn_w"], f32)
    cab = np.asarray(inputs["c_attn_b"], f32)
    wq = caw[:, h * 64:(h + 1) * 64]
    wk = caw[:, 512 + h * 64:512 + (h + 1) * 64]
    wv = caw[:, 1024 + h * 64:1024 + (h + 1) * 64]
    wqkv = np.concatenate([wq, wk, wv], axis=1)          # [512, 192]
    # RMSNorm additive b folded into qkv biases
    bq = bvec @ wq + cab[h * 64:(h + 1) * 64]
    bk = bvec @ wk + cab[512 + h * 64:512 + (h + 1) * 64]
    bv = bvec @ wv + cab[1024 + h * 64:1024 + (h + 1) * 64]
    kk = np.arange(4)[None, :, None] * P + np.arange(P)[:, None, None]
    qq = np.arange(D)[None, None, :]
    maskd = (kk <= qq).astype(f32)                       # [128, 4, 512]
    projb = np.asarray(inputs["c_proj_b"], f32)
    projb_col = np.ascontiguousarray(projb.reshape(4, P).T)  # [p, dc]
    c0 = 1.0 if core == 0 else 0.0
    xmask = np.zeros((D, N), f32)
    xmask[64 * core:64 * core + 64, :] = xT[64 * core:64 * core + 64, :]
    xmask += (projb * c0)[:, None]
    xmask = np.ascontiguousarray(
        xmask.reshape(4, P, N).transpose(1, 0, 2)).astype(bf)
    selb = np.zeros((E, EL, P), f32)
    selb[2 * core, 0, :] = 1.0
    selb[2 * core + 1, 1, :] = 1.0

    sl = slice(2 * core, 2 * core + 2)
    w_in = np.asarray(inputs["w_in"], f32)[sl]           # [2, 512, 2048]
    w1 = np.asarray(inputs["w1"], f32)[sl]               # [2, 2048, 4096]
    w2 = np.asarray(inputs["w2"], f32)[sl]               # [2, 2048, 2048]
    w_out = np.asarray(inputs["w_out"], f32)[sl]         # [2, 2048, 512]

    # DoubleRow lhsT layouts (pair index j adjacent to the 128-wide m dim)
    w_in8 = np.ascontiguousarray(
        _q8w(w_in).reshape(EL, 2, 2, P, 16, P)
        .transpose(0, 3, 1, 4, 2, 5))                    # [EL, p, kp, hc, j, m]
    w1a = _q8w(w1[:, :, :HD]).reshape(EL, 8, 2, P, 16, P)
    w1b = _q8w(w1[:, :, HD:]).reshape(EL, 8, 2, P, 16, P)
    w18 = np.stack([w1a, w1b], axis=2)                   # [EL, kp, ab, j, p, pr, m]
    w18 = np.ascontiguousarray(
        w18.transpose(0, 5, 4, 2, 1, 3, 6))              # [EL, pr, p, ab, kp, j, m]
    w28 = _q8w(w2).reshape(EL, 8, 2, P, 16, P) \
        .transpose(0, 4, 3, 1, 2, 5)                     # [EL, oc, p, kp, j, m]
    w28 = np.ascontiguousarray(
        w28.reshape(EL, 4, 4, P, 8, 2, P)
        .transpose(0, 1, 3, 2, 4, 5, 6))                 # [EL, og, p, ocl, kp, j, m]
    wout8 = np.ascontiguousarray(
        _q8w(w_out).reshape(EL, 8, 2, P, 4, P)
        .transpose(0, 3, 1, 4, 2, 5))                    # [EL, p, kp, dc, j, m]

    def bias_t(key, scale, w):
        b = np.asarray(inputs[key], f32)[sl] * scale     # [2, w*128]
        return np.ascontiguousarray(b.reshape(EL, w, P).transpose(2, 0, 1))

    mfeat = np.zeros((D,), f32)
    mfeat[64 * core:64 * core + 64] = 1.0
    smalls = np.concatenate([
        np.ascontiguousarray(mfeat.reshape(4, P).T),
        np.ascontiguousarray((projb * c0).reshape(4, P).T),
        np.broadcast_to(np.asarray(inputs["router_b"], f32), (P, E)),
        projb_col,
        projb_col * c0,
        np.full((P, 1), c0 / S_Y, f32),
        np.broadcast_to(bv, (P, HDIM)),
    ], axis=1).astype(f32)                               # [P, 97]
    bqa = np.zeros((HDIM, 4), f32)
    bqa[:, 0] = bq
    bqa[:, 1] = bk
    bqa[0, 3] = np.asarray(inputs["alpha"], f32)[h]
    wbig = np.concatenate([
        wqkv.reshape(4, P, 192).transpose(1, 0, 2).reshape(P, 768),
        maskd.reshape(P, 4 * D),
        np.asarray(inputs["router_w"], f32)
        .reshape(4, P, E).transpose(1, 0, 2).reshape(P, 4 * E),
    ], axis=1).astype(bf)                                # [P, 2880]
    b1 = bias_t("b1", 1.0, 32)
    ebias = np.concatenate([
        bias_t("b_in", S_H, 16),
        b1[:, :, :16] * S_S,
        b1[:, :, 16:],
        bias_t("b2", S_O, 16),
        bias_t("b_out", S_O * S_W, 4),
    ], axis=2).astype(f32)                               # [P, 2, 68]
    return {
        "xtb": np.ascontiguousarray(
            xT.reshape(4, P, N).transpose(1, 0, 2)).astype(bf),
        "xmask": xmask,
        "smalls": smalls,
        "bqalpha": bqa,
        "wbig": wbig,
        "grow": np.concatenate(
            [g, np.full((HDIM,), np.asarray(inputs["alpha"], f32)[h])]
        ).reshape(1, D + HDIM).astype(bf),
        "selb": selb.astype(bf),
        "w_in8": w_in8, "w18": w18, "w28": w28, "wout8": wout8,
        "ebias": ebias,
    }


last_result = [None]


def kernel(**inputs):
    if "nc" not in _cache:
        _cache["nc"] = build_program()
    nc = _cache["nc"]
    in_maps = [_prep_inputs(inputs, c) for c in range(NCORES)]
    res = run_bass_kernel_spmd(nc, in_maps, core_ids=list(range(NCORES)))
    last_result[0] = res
    outT = np.concatenate(
        [np.asarray(res.results[c]["out"]).astype(np.float32)
         for c in range(NCORES)], axis=0)                # [512, 2048]
    return np.ascontiguousarray(outT.T).reshape(2, 1024, 512)


# revision 49
# speedup vs baseline: 1.1017x; 1.0033x over previous
"""MoE transformer block (QK-norm attention + top-8-of-16 MoE) on 8 trn2 cores.

v4: dense fp8 DoubleRow experts (as v2 baseline) with restructured
scheduling:
- batch-split expert pipeline: each expert runs a b0-pass (tokens 0-1023)
  then a b1-pass, so expert-0's b0 compute starts right after AllGather-0
  and fully hides AllGather-1 + proj-b1;
- the ReduceScatter is split per batch: RS(b0)'s input is complete after
  the last expert's b0-pass and it runs hidden under ~95us of b1 compute,
  leaving only RS(b1) (~18us) in the tail;
- attention-phase cost cuts: softmax denominator folded into an augmented
  v row (kills 24 denominator matmuls), all broadcast matmuls in bf16
  (1 cyc/row instead of 4), RMSNorm's g folded into the broadcast and its
  additive b folded into the qkv biases (host-side);
- SwiGLU's a-path and the moe combine adds alternate DVE/ACT/GpSimd to
  keep the vector engine off the critical path;
- startup DMAs split so RMSNorm starts after the first 0.5MB chunk.

Sharding: attention head-parallel (core c owns head c), experts
expert-parallel (core c owns experts 2c, 2c+1), output feature-parallel
(core c returns features [64c, 64c+64) for all tokens).

Everything feature-major ("T layout": features on partitions, tokens on
free). Scales (powers of two, exact): x_res*2^5, weights*2^11, h1*2^5,
s*2^5, o*2^7; descales folded into Act/DVE scale+bias immediates.
"""

import numpy as np
import ml_dtypes

import concourse.bass as bass
import concourse.mybir as mybir
from concourse.tile import TileContext
from concourse.masks import make_identity
from concourse.bass_utils import run_bass_kernel_spmd

BF16 = mybir.dt.bfloat16
F32 = mybir.dt.float32
F8 = mybir.dt.float8e4
AFT = mybir.ActivationFunctionType
MUL = mybir.AluOpType.mult
ADD = mybir.AluOpType.add
DR = mybir.MatmulPerfMode.DoubleRow

P = 128
D = 512          # embed dim
T = 1024         # tokens per batch
N = 2048         # total tokens
E = 16           # experts
EL = 2           # experts per core
HD = 2048        # expert hidden
HDIM = 64        # head dim
NCORES = 8

S_X = 2.0 ** 5
S_W = 2.0 ** 11
S_H = 2.0 ** 5
S_S = 2.0 ** 5
S_O = 2.0 ** 7
S_Y = 2.0 ** 5

_cache = {}


def build_program():
    nc = bass.Bass()
    dp_ = dict(isOutput=False)
    xtb_d = nc.declare_dram_parameter("xtb", [P, 4, N], BF16, **dp_)
    xm_d = nc.declare_dram_parameter("xmask", [P, 4, N], BF16, **dp_)
    sm_d = nc.declare_dram_parameter("smalls", [P, 97], F32, **dp_)
    bqa_d = nc.declare_dram_parameter("bqalpha", [HDIM, 4], F32, **dp_)
    wbig_d = nc.declare_dram_parameter("wbig", [P, 2880], BF16, **dp_)
    grow_d = nc.declare_dram_parameter("grow", [1, D + HDIM], BF16,
                                       **dp_)
    sel_d = nc.declare_dram_parameter("selb", [E, EL, P], BF16, **dp_)
    win_d = nc.declare_dram_parameter("w_in8", [EL, P, 2, 16, 2, P], F8, **dp_)
    w1_d = nc.declare_dram_parameter("w18", [EL, 16, P, 2, 8, 2, P], F8, **dp_)
    w2_d = nc.declare_dram_parameter("w28", [EL, 4, P, 4, 8, 2, P], F8, **dp_)
    wout_d = nc.declare_dram_parameter("wout8", [EL, P, 8, 4, 2, P], F8, **dp_)
    eb_d = nc.declare_dram_parameter("ebias", [P, EL, 68], F32, **dp_)
    out_d = nc.declare_dram_parameter("out", [HDIM, N], BF16, isOutput=True)

    groups = [list(range(NCORES))]

    with TileContext(nc, num_cores=NCORES) as tc:
        with (
            tc.tile_pool(name="const", bufs=1) as cp,
            tc.tile_pool(name="pp", bufs=1) as pp,
            tc.tile_pool(name="psE", bufs=4, space="PSUM") as psE,
            tc.tile_pool(name="psB", bufs=2, space="PSUM") as psB,
            tc.tile_pool(name="psS", bufs=2, space="PSUM") as psS,
            tc.tile_pool(name="dram", bufs=1, space="DRAM") as dp,
        ):
            # ---- constants / persistent ----
            ws = tc.alloc_tile_pool(name="wst", bufs=1)
            s1pre = tc.alloc_tile_pool(name="s1pre", bufs=1)
            xTb = s1pre.tile([P, 4, N], BF16, tag="xTb")
            nc.sync.dma_start(xTb[:, :, 0:D], xtb_d[:, :, 0:D])
            wbig = s1pre.tile([P, 2880], BF16, tag="wbig")
            nc.sync.dma_start(wbig[:, 0:768], wbig_d[:, 0:768])
            for cc in range(1, 4):
                sl = slice(cc * D, (cc + 1) * D)
                nc.sync.dma_start(xTb[:, :, sl], xtb_d[:, :, sl])
            ones128b = cp.tile([P, 1], BF16, tag="ones128b")
            nc.vector.memset(ones128b, 1.0)
            ones64b = cp.tile([HDIM, 1], BF16, tag="ones64b")
            nc.vector.memset(ones64b, 1.0)
            ones1rb = cp.tile([1, P], BF16, tag="ones1rb")
            nc.vector.memset(ones1rb, 1.0)
            sm = cp.tile([P, 97], F32, tag="sm")
            nc.sync.dma_start(sm, sm_d[:, :])
            mfeat_sb = sm[:, 0:4]
            pbc0_sb = sm[:, 4:8]
            rb_sb = sm[:, 8:24]
            projb_sb = sm[:, 24:28]
            c0_sb = sm[:, 32:33]
            vbias_sb = sm[:, 33:97]
            sel_sb = cp.tile([E, EL, P], BF16, tag="sel_sb")
            nc.sync.dma_start(sel_sb, sel_d[:, :, :])
            eps6 = cp.tile([1, 1], F32, tag="eps6")
            nc.vector.memset(eps6, 1e-6)
            grow = cp.tile([1, D + HDIM], BF16, tag="grow")
            nc.sync.dma_start(grow, grow_d[:, :])
            identb = cp.tile([P, P], BF16, tag="identb")
            make_identity(nc, identb)

            # persistent activations for the expert phase
            moeT = pp.tile([P, 4, N], BF16, tag="moeT")
            xrT8 = pp.tile([P, 4, N], F8, tag="xrT8")
            gatesT = pp.tile([E, N], BF16, tag="gatesT")
            h1T8 = pp.tile([P, 16, N], F8, tag="h1T8")

            ag_in = [dp.tile([HDIM, T], F8, name=f"ag_in{i}")
                     for i in range(2)]
            ag_out = [dp.tile([D, T], F8, addr_space="Shared",
                              name=f"ag_out{i}")
                      for i in range(2)]
            rs_in = [dp.tile([D, T], BF16, name=f"rs_in{i}")
                     for i in range(2)]
            rs_out = [dp.tile([HDIM, T], BF16, name=f"rs_out{i}")
                      for i in range(2)]

            with tc.tile_pool(name="s1", bufs=1) as s1:
                nc.sync.dma_start(wbig[:, 768:2880], wbig_d[:, 768:2880])
                bqa = s1.tile([HDIM, 4], F32, tag="bqa")
                nc.sync.dma_start(bqa, bqa_d[:, :])
                bq_sb = bqa[:, 0:3]
                alpha_sb = bqa[0:1, 3:4]
                # prefetch: expert biases + e0 weights (consumed ~90us later)
                ebt = pp.tile([P, EL, 68], F32, tag="ebias")
                nc.sync.dma_start(ebt, eb_d[:, :, :])
                bias_cp = {"bin": ebt[:, :, 0:16], "b1a": ebt[:, :, 16:32],
                           "b1b": ebt[:, :, 32:48], "b2": ebt[:, :, 48:64],
                           "bout": ebt[:, :, 64:68]}
                pre_win = ws.tile([P, 2, 16, 2, P], F8, tag="win", bufs=1)
                nc.sync.dma_start(pre_win, win_d[0])
                pre_wout = ws.tile([P, 8, 4, 2, P], F8, tag="wot", bufs=1)
                nc.sync.dma_start(pre_wout, wout_d[0])
                pre_w1 = []
                for pr in range(3):
                    t = ws.tile([P, 2, 8, 2, P], F8, tag="w1s", bufs=4)
                    nc.sync.dma_start(t, w1_d[0, pr])
                    pre_w1.append(t)
                # ---- RMSNorm + qkv + qk-norm + attention, batch-ordered:
                # all of batch b's chain runs before batch b+1 so AG(b)
                # issues early and b1 prep fills the AG0 window ----
                with tc.tile_pool(name="attp", bufs=1) as ap_, \
                     tc.tile_pool(name="ate", bufs=12) as ate:
                    xnT = ap_.tile([P, 4, N], BF16, tag="xnT")
                    qT = ap_.tile([HDIM, N], BF16, tag="qT")
                    kT = ap_.tile([HDIM, N], BF16, tag="kT")
                    v_aug = ap_.tile([P, 16, HDIM + 1], BF16, tag="v_aug")
                    nc.vector.memset(v_aug, 1.0)
                    qh = ap_.tile([HDIM, N], BF16, tag="qh")
                    kh = ap_.tile([HDIM, N], BF16, tag="kh")
                    yhT = ap_.tile([HDIM, N], F8, tag="yhT")
                    for b in range(2):
                        for cc in (2 * b, 2 * b + 1):
                            sl = slice(cc * D, (cc + 1) * D)
                            ps = psS.tile([1, D], F32, tag="ps_small")
                            for kc in range(4):
                                sq = ate.tile([P, D], BF16, tag="sq_t",
                                              bufs=3)
                                nc.scalar.activation(sq, xTb[:, kc, sl],
                                                     AFT.Square)
                                nc.tensor.matmul(ps, ones128b, sq,
                                                 start=(kc == 0),
                                                 stop=(kc == 3))
                            tmp = ate.tile([1, D], F32, tag="r_t", bufs=2)
                            nc.scalar.activation(tmp, ps, AFT.Sqrt,
                                                 scale=1.0 / D,
                                                 bias=eps6[0:1, 0:1])
                            rrow = ate.tile([1, D], BF16, tag="rrow",
                                            bufs=2)
                            with nc.allow_low_precision(
                                    reason="bf16 bcast row"):
                                nc.vector.reciprocal(rrow, tmp)
                            # xnT = xTb * bcast(rrow * g)
                            for kc in range(4):
                                pb = psB.tile([P, D], F32, tag="bc")
                                nc.tensor.matmul(
                                    pb, grow[0:1, kc * P:(kc + 1) * P],
                                    rrow[0:1, :], start=True, stop=True)
                                nc.vector.tensor_mul(xnT[:, kc, sl],
                                                     xTb[:, kc, sl], pb)
                            for wi, dst, bi in ((0, qT, 0), (1, kT, 1)):
                                ps2 = psS.tile([HDIM, D], F32,
                                               tag="ps_small")
                                for kc in range(4):
                                    nc.tensor.matmul(
                                        ps2,
                                        wbig[:, kc * 192 + wi * HDIM:
                                             kc * 192 + (wi + 1) * HDIM],
                                        xnT[:, kc, sl], start=(kc == 0),
                                        stop=(kc == 3))
                                nc.scalar.activation(
                                    dst[:, sl], ps2, AFT.Identity,
                                    bias=bq_sb[:, bi:bi + 1])
                            for tk in range(cc * 4, cc * 4 + 4):
                                ps3 = psS.tile([P, HDIM], F32,
                                               tag="ps_small")
                                for kc in range(4):
                                    nc.tensor.matmul(
                                        ps3, xnT[:, kc, tk * P:(tk + 1) * P],
                                        wbig[:, kc * 192 + 128:
                                             kc * 192 + 192],
                                        start=(kc == 0), stop=(kc == 3))
                                nc.vector.tensor_add(v_aug[:, tk, 0:HDIM],
                                                     ps3, vbias_sb)
                            # qk-norm for this chunk (alpha folded in the
                            # q bcast row)
                            for src_, dst, brow in (
                                    (qT, qh, grow[0:1, D:D + HDIM]),
                                    (kT, kh, ones1rb[0:1, 0:HDIM])):
                                sq = ate.tile([HDIM, D], BF16, tag="sqn",
                                              bufs=2)
                                nc.scalar.activation(sq, src_[:, sl],
                                                     AFT.Square)
                                ps4 = psS.tile([1, D], F32, tag="ps_small")
                                nc.tensor.matmul(ps4, ones64b, sq,
                                                 start=True, stop=True)
                                t = ate.tile([1, D], F32, tag="rn_t",
                                             bufs=2)
                                nc.scalar.activation(t, ps4, AFT.Sqrt)
                                nc.vector.tensor_scalar_add(t, t, 1e-5)
                                rn = ate.tile([1, D], BF16, tag="rn",
                                              bufs=2)
                                with nc.allow_low_precision(
                                        reason="bf16 row"):
                                    nc.vector.reciprocal(rn, t)
                                pb = psB.tile([HDIM, D], F32, tag="bc")
                                nc.tensor.matmul(pb, brow, rn[0:1, :],
                                                 start=True, stop=True)
                                nc.vector.tensor_mul(dst[:, sl],
                                                     src_[:, sl], pb)
                        # scoresT -> exp (masked diag) -> AV (denominator
                        # folded into v_aug's ones row) -> yhT -> AG(b)
                        for qc in range(2):
                            qsl = slice(b * T + qc * D, b * T + (qc + 1) * D)
                            nkc = 4 * qc + 4
                            ex_tiles = []
                            for kc in range(nkc):
                                ksl = slice(b * T + kc * P,
                                            b * T + (kc + 1) * P)
                                ps = psE.tile([P, D], F32, tag="mm")
                                nc.tensor.matmul(ps, kh[:, ksl], qh[:, qsl],
                                                 start=True, stop=True)
                                if kc >= 4 * qc:  # diagonal block: mask
                                    et = ate.tile([P, D], BF16, tag="exp_b",
                                                  bufs=4)
                                    nc.scalar.activation(et, ps, AFT.Exp)
                                    eb2 = ate.tile([P, D], BF16,
                                                   tag="exp_m", bufs=7)
                                    nc.vector.tensor_mul(
                                        eb2, et,
                                        wbig[:, 768 + (kc - 4 * qc) * D:
                                             768 + (kc - 4 * qc + 1) * D])
                                else:
                                    eb2 = ate.tile([P, D], BF16,
                                                   tag="exp_m", bufs=7)
                                    nc.scalar.activation(eb2, ps, AFT.Exp)
                                ex_tiles.append(eb2)
                            py = psS.tile([HDIM + 1, D], F32,
                                          tag="ps_small")
                            for kc in range(nkc):
                                nc.tensor.matmul(py, v_aug[:, b * 8 + kc, :],
                                                 ex_tiles[kc],
                                                 start=(kc == 0),
                                                 stop=(kc == nkc - 1))
                            dr = ate.tile([1, D], BF16, tag="dr", bufs=2)
                            with nc.allow_low_precision(
                                    reason="bf16 softmax denom"):
                                nc.vector.reciprocal(dr,
                                                     py[HDIM:HDIM + 1, :])
                            pb2 = psB.tile([HDIM, D], F32, tag="bc")
                            nc.tensor.matmul(pb2, ones1rb[0:1, 0:HDIM], dr,
                                             start=True, stop=True)
                            db = ate.tile([HDIM, D], BF16, tag="db", bufs=2)
                            nc.scalar.activation(db, pb2, AFT.Copy,
                                                 scale=S_Y)
                            nc.vector.tensor_mul(yhT[:, qsl],
                                                 py[0:HDIM, :], db)
                            nc.sync.dma_start(
                                ag_in[b][:, qc * D:(qc + 1) * D],
                                yhT[:, qsl])
                        nc.gpsimd.collective_compute(
                            "AllGather", mybir.AluOpType.bypass,
                            ins=[ag_in[b][:]], outs=[ag_out[b][:]],
                            replica_groups=groups)

                # ---- proj + x_res + router (per batch), interleaved with
                # expert passes so weight DMAs never queue behind the
                # AG1-blocked b1 input DMAs ----
                pass
            # (s1 stays open: xTb / qT / kT / v_aug / wbig / bqa)
            with tc.tile_pool(name="s2", bufs=1) as s2, \
                 tc.tile_pool(name="s2e", bufs=4) as s2e, \
                 tc.tile_pool(name="eact", bufs=1) as ac, \
                 tc.tile_pool(name="eev", bufs=6) as ev_:
                yT_sb = s2.tile([P, 4, N], F8, tag="yT_sb")
                routes = s2.tile([P, 16, E], F32, tag="routes")
                gates = routes
                gsum = s2.tile([P, 16], F32, tag="gsum")
                gates_bf = s2.tile([P, 16, E], BF16, tag="gates_bf")

                def emit_batch(b):
                    for ts in (2 * b, 2 * b + 1):
                        nc.sync.dma_start(
                            yT_sb[:, :, ts * D:(ts + 1) * D],
                            ag_out[b][:, (ts % 2) * D:(ts % 2 + 1) * D]
                            .rearrange("(kc p) n -> p kc n", p=P))
                    for ts in (2 * b, 2 * b + 1):
                        for dc in range(4):
                            tsl = slice(ts * D, (ts + 1) * D)
                            ps = psE.tile([P, D], F32, tag="mm")
                            for kc in range(4):
                                nc.tensor.matmul(
                                    ps,
                                    wbig[:, 2816 + kc * D + dc * P:
                                         2816 + kc * D + (dc + 1) * P],
                                    yT_sb[:, kc, tsl],
                                    start=(kc == 0), stop=(kc == 3))
                            # xmb = x*featmask + projb*c0 (replaces the
                            # host xmask tensor)
                            xmb = s2e.tile([P, D], BF16, tag="xmb", bufs=3)
                            nc.vector.tensor_scalar(
                                xmb, xTb[:, dc, tsl],
                                mfeat_sb[:, dc:dc + 1],
                                pbc0_sb[:, dc:dc + 1], op0=MUL, op1=ADD)
                            tmp = s2e.tile([P, D], F32, tag="yp_t", bufs=3)
                            nc.scalar.activation(
                                tmp, ps, AFT.Identity, scale=1.0 / S_Y,
                                bias=projb_sb[:, dc:dc + 1])
                            # x_res written in place over xTb
                            nc.vector.tensor_add(xTb[:, dc, tsl], tmp,
                                                 xTb[:, dc, tsl])
                            nc.scalar.activation(xrT8[:, dc, tsl],
                                                 xTb[:, dc, tsl],
                                                 AFT.Copy, scale=S_X)
                            # moeT init: yp*c0 + x feature slice
                            nc.vector.scalar_tensor_tensor(
                                moeT[:, dc, tsl], ps, c0_sb[:, 0:1],
                                xmb, op0=MUL, op1=ADD)
                def emit_gates(b):
                    # router for this batch -> normalized top-8 gates
                    for tk in range(8 * b, 8 * b + 8):
                        ps = psS.tile([P, E], F32, tag="ps_small")
                        for kc in range(4):
                            nc.tensor.matmul(
                                ps, xTb[:, kc, tk * P:(tk + 1) * P],
                                wbig[:, 2816 + kc * E:2816 + (kc + 1) * E],
                                start=(kc == 0), stop=(kc == 3))
                        nc.vector.tensor_add(routes[:, tk, :], ps, rb_sb)
                    nc.scalar.activation(routes[:, 8 * b:8 * b + 8, :],
                                         routes[:, 8 * b:8 * b + 8, :],
                                         AFT.Exp)
                    for g in range(8 * b, 8 * b + 8):
                        m8 = s2e.tile([P, 8], F32, tag="m8", bufs=2)
                        nc.vector.max(out=m8, in_=routes[:, g, :])
                        zap = s2e.tile([P, E], F32, tag="zap", bufs=2)
                        nc.vector.match_replace(out=zap, in_to_replace=m8,
                                                in_values=routes[:, g, :],
                                                imm_value=0)
                        nc.vector.tensor_sub(gates[:, g, :],
                                             routes[:, g, :], zap)
                    nc.vector.reduce_sum(gsum[:, 8 * b:8 * b + 8],
                                         gates[:, 8 * b:8 * b + 8, :],
                                         axis=mybir.AxisListType.X)
                    nc.vector.reciprocal(gsum[:, 8 * b:8 * b + 8],
                                         gsum[:, 8 * b:8 * b + 8])
                    for g in range(8 * b, 8 * b + 8):
                        nc.vector.tensor_scalar_mul(gates_bf[:, g, :],
                                                    gates[:, g, :],
                                                    gsum[:, g:g + 1])
                    for g in range(8 * b, 8 * b + 8):
                        pt = psS.tile([E, P], BF16, tag="ps_small")
                        nc.tensor.transpose(pt, gates_bf[:, g, :], identb)
                        nc.scalar.activation(
                            gatesT[:, g * P:(g + 1) * P], pt, AFT.Copy)

                held = {}

                def emit_pass(e, half, post_h1=None, post_sw=None):
                    ts_range = (2 * half, 2 * half + 1)
                    if half == 0:
                        if e == 0:
                            win_t = pre_win
                            wout_t = pre_wout
                        else:
                            win_t = ws.tile([P, 2, 16, 2, P], F8,
                                            tag="win", bufs=1, name="win_t")
                            nc.sync.dma_start(win_t, win_d[e])
                            wout_t = ws.tile([P, 8, 4, 2, P], F8,
                                             tag="wot", bufs=1,
                                             name="wout_t")
                            nc.sync.dma_start(wout_t, wout_d[e])
                        held[e] = (win_t, wout_t)
                    else:
                        win_t, wout_t = held[e]
                    # h1 = x_res @ w_in  (fp8, S_H)
                    for ts in ts_range:
                        for hc in range(16):
                            tsl = slice(ts * D, (ts + 1) * D)
                            ps = psE.tile([P, D], F32, tag="mm")
                            for kp in range(2):
                                nc.tensor.matmul(
                                    ps, win_t[:, kp, hc, :, :],
                                    xrT8[:, 2 * kp:2 * kp + 2, tsl],
                                    start=(kp == 0), stop=(kp == 1),
                                    perf_mode=DR)
                            if (hc + ts) % 2 == 0:
                                nc.scalar.activation(
                                    h1T8[:, hc, tsl], ps, AFT.Identity,
                                    scale=S_H / (S_X * S_W),
                                    bias=bias_cp["bin"][:, e, hc:hc + 1])
                            else:
                                nc.vector.tensor_scalar(
                                    h1T8[:, hc, tsl], ps,
                                    S_H / (S_X * S_W),
                                    bias_cp["bin"][:, e, hc:hc + 1],
                                    op0=MUL, op1=ADD)
                    if post_h1 is not None:
                        post_h1()
                    # SwiGLU: s = silu(h@w1b + b1b) * (h@w1a + b1a)
                    sT8 = ac.tile([P, 16, T], F8, tag="sT8", bufs=1,
                                  name="sT8")
                    for pr in range(16):
                        if e == 0 and half == 0 and pr < 3:
                            w1_t = pre_w1[pr]
                        else:
                            w1_t = ws.tile([P, 2, 8, 2, P], F8, tag="w1s",
                                           bufs=4, name="w1_t")
                            nc.sync.dma_start(w1_t, w1_d[e, pr])
                        for ts in ts_range:
                            tsl = slice(ts * D, (ts + 1) * D)
                            pa = psE.tile([P, D], F32, tag="mm")
                            pb = psE.tile([P, D], F32, tag="mm")
                            for kp in range(8):
                                nc.tensor.matmul(
                                    pa, w1_t[:, 0, kp, :, :],
                                    h1T8[:, 2 * kp:2 * kp + 2, tsl],
                                    start=(kp == 0), stop=(kp == 7),
                                    perf_mode=DR)
                            for kp in range(8):
                                nc.tensor.matmul(
                                    pb, w1_t[:, 1, kp, :, :],
                                    h1T8[:, 2 * kp:2 * kp + 2, tsl],
                                    start=(kp == 0), stop=(kp == 7),
                                    perf_mode=DR)
                            sil = ev_.tile([P, D], BF16, tag="sil", bufs=4)
                            nc.scalar.activation(
                                sil, pb, AFT.Silu,
                                scale=1.0 / (S_H * S_W),
                                bias=bias_cp["b1b"][:, e, pr:pr + 1])
                            av8 = ev_.tile([P, D], F8, tag="av8", bufs=4)
                            if (pr + ts) % 4 != 3:
                                nc.scalar.activation(
                                    av8, pa, AFT.Identity,
                                    scale=S_S / (S_H * S_W),
                                    bias=bias_cp["b1a"][:, e, pr:pr + 1])
                            else:
                                nc.vector.tensor_scalar(
                                    av8, pa, S_S / (S_H * S_W),
                                    bias_cp["b1a"][:, e, pr:pr + 1],
                                    op0=MUL, op1=ADD)
                            ltsl = slice((ts - 2 * half) * D,
                                         (ts - 2 * half + 1) * D)
                            nc.vector.tensor_mul(sT8[:, pr, ltsl], av8, sil)
                    if post_sw is not None:
                        post_sw()
                    # o = s @ w2 + b2  (fp8, S_O)
                    oT8 = ac.tile([P, 16, T], F8, tag="oT8", bufs=1,
                                  name="oT8")
                    for og in range(4):
                        w2_t = ws.tile([P, 4, 8, 2, P], F8, tag="w2s",
                                       bufs=2, name="w2_t")
                        nc.sync.dma_start(w2_t, w2_d[e, og])
                        for oc4 in range(4):
                            oc = og * 4 + oc4
                            for ts in ts_range:
                                ltsl = slice((ts - 2 * half) * D,
                                             (ts - 2 * half + 1) * D)
                                ps = psE.tile([P, D], F32, tag="mm")
                                for kp in range(8):
                                    nc.tensor.matmul(
                                        ps, w2_t[:, oc4, kp, :, :],
                                        sT8[:, 2 * kp:2 * kp + 2, ltsl],
                                        start=(kp == 0), stop=(kp == 7),
                                        perf_mode=DR)
                                if (oc + ts) % 4 != 3:
                                    nc.vector.tensor_scalar(
                                        oT8[:, oc, ltsl], ps,
                                        S_O / (S_S * S_W),
                                        bias_cp["b2"][:, e, oc:oc + 1],
                                        op0=MUL, op1=ADD)
                                else:
                                    nc.scalar.activation(
                                        oT8[:, oc, ltsl], ps, AFT.Identity,
                                        scale=S_O / (S_S * S_W),
                                        bias=bias_cp["b2"][:, e, oc:oc + 1])
                    # gate broadcast [tokens] -> [P, D] per ts chunk
                    gb_tiles = {}
                    for ts in ts_range:
                        tsl = slice(ts * D, (ts + 1) * D)
                        pg = psB.tile([P, D], F32, tag="bc")
                        nc.tensor.matmul(pg, sel_sb[:, e, :], gatesT[:, tsl],
                                         start=True, stop=True)
                        gb = ev_.tile([P, D], BF16, tag="gb", bufs=4)
                        nc.scalar.activation(gb, pg, AFT.Copy,
                                             scale=1.0 / (S_O * S_W))
                        gb_tiles[ts] = gb
                    # eo = o @ w_out + b_out; moeT += gate * eo
                    for dc in range(4):
                        for ts in ts_range:
                            tsl = slice(ts * D, (ts + 1) * D)
                            ltsl = slice((ts - 2 * half) * D,
                                         (ts - 2 * half + 1) * D)
                            ps = psE.tile([P, D], F32, tag="mm")
                            for kp in range(8):
                                nc.tensor.matmul(
                                    ps, wout_t[:, kp, dc, :, :],
                                    oT8[:, 2 * kp:2 * kp + 2, ltsl],
                                    start=(kp == 0), stop=(kp == 7),
                                    perf_mode=DR)
                            t2 = ev_.tile([P, D], F32, tag="t2", bufs=3)
                            nc.vector.scalar_tensor_tensor(
                                t2, ps, bias_cp["bout"][:, e, dc:dc + 1],
                                gb_tiles[ts], op0=ADD, op1=MUL)
                            if (dc + ts) % 2 == 0:
                                nc.vector.tensor_add(moeT[:, dc, tsl],
                                                     moeT[:, dc, tsl], t2)
                            else:
                                nc.gpsimd.tensor_add(moeT[:, dc, tsl],
                                                     moeT[:, dc, tsl], t2)
                            if e == 1:
                                nc.sync.dma_start(
                                    rs_in[half][dc * P:(dc + 1) * P,
                                                (ts - 2 * half) * D:
                                                (ts - 2 * half + 1) * D],
                                    moeT[:, dc, tsl])
                    if e == 1:
                        nc.gpsimd.collective_compute(
                            "ReduceScatter", mybir.AluOpType.add,
                            ins=[rs_in[half][:]], outs=[rs_out[half][:]],
                            replica_groups=groups)
                        nc.sync.dma_start(
                            out_d[:, half * T:(half + 1) * T],
                            rs_out[half][:, :])

                emit_batch(0)
                emit_pass(0, 0, post_h1=lambda: emit_gates(0),
                          post_sw=lambda: emit_batch(1))
                emit_pass(0, 1, post_h1=lambda: emit_gates(1))
                emit_pass(1, 0)
                emit_pass(1, 1)
            s1pre.release()

            ws.release()

    _split_matmul_waits(nc)
    return nc


def _split_matmul_waits(nc):
    """walrus allows only one sync-wait per engine-instruction sync slot; move
    extra waits onto standalone InstEventSemaphore waits inserted before."""
    import concourse.mybir as mybir
    k = 0
    for bb in nc.main_func.blocks:
        il = list(bb.instructions)
        out = []
        changed = False
        for ins in il:
            si = getattr(ins, "sync_info", None)
            if si is not None and len(si.on_wait) > 1 \
                    and type(ins).__name__ != "InstEventSemaphore":
                waits = list(si.on_wait)
                keep, move = waits[-1], waits[:-1]
                for w in move:
                    nop = mybir.InstEventSemaphore(name=f"I-wsplit-{k}",
                                                   ins=[], outs=[])
                    k += 1
                    nop.engine = ins.engine
                    nop.sync_info = type(si)(on_wait=[w], on_update=[])
                    out.append(nop)
                ins.sync_info = type(si)(on_wait=[keep],
                                         on_update=list(si.on_update))
                changed = True
            out.append(ins)
        if changed:
            bb.instructions = out


def _q8w(w):
    """host fp8 cast with fixed 2^11 scale (clipped to TRN e4m3 max)."""
    return np.clip(np.asarray(w, np.float32) * S_W, -240.0, 240.0).astype(
        ml_dtypes.float8_e4m3)


def _prep_inputs(inputs, core):
    bf = ml_dtypes.bfloat16
    f32 = np.float32
    h = core
    x = np.asarray(inputs["x"], f32).reshape(N, D)
    xT = np.ascontiguousarray(x.T)                      # [512, 2048]
    g = np.asarray(inputs["g"], f32)
    bvec = np.asarray(inputs["b"], f32)
    caw = np.asarray(inputs["c_attn_w"], f32)
    cab = np.asarray(inputs["c_attn_b"], f32)
    wq = caw[:, h * 64:(h + 1) * 64]
    wk = caw[:, 512 + h * 64:512 + (h + 1) * 64]
    wv = caw[:, 1024 + h * 64:1024 + (h + 1) * 64]
    wqkv = np.concatenate([wq, wk, wv], axis=1)          # [512, 192]
    # RMSNorm additive b folded into qkv biases
    bq = bvec @ wq + cab[h * 64:(h + 1) * 64]
    bk = bvec @ wk + cab[512 + h * 64:512 + (h + 1) * 64]
    bv = bvec @ wv + cab[1024 + h * 64:1024 + (h + 1) * 64]
    kk = np.arange(4)[None, :, None] * P + np.arange(P)[:, None, None]
    qq = np.arange(D)[None, None, :]
    maskd = (kk <= qq).astype(f32)                       # [128, 4, 512]
    projb = np.asarray(inputs["c_proj_b"], f32)
    projb_col = np.ascontiguousarray(projb.reshape(4, P).T)  # [p, dc]
    c0 = 1.0 if core == 0 else 0.0
    xmask = np.zeros((D, N), f32)
    xmask[64 * core:64 * core + 64, :] = xT[64 * core:64 * core + 64, :]
    xmask += (projb * c0)[:, None]
    xmask = np.ascontiguousarray(
        xmask.reshape(4, P, N).transpose(1, 0, 2)).astype(bf)
    selb = np.zeros((E, EL, P), f32)
    selb[2 * core, 0, :] = 1.0
    selb[2 * core + 1, 1, :] = 1.0

    sl = slice(2 * core, 2 * core + 2)
    w_in = np.asarray(inputs["w_in"], f32)[sl]           # [2, 512, 2048]
    w1 = np.asarray(inputs["w1"], f32)[sl]               # [2, 2048, 4096]
    w2 = np.asarray(inputs["w2"], f32)[sl]               # [2, 2048, 2048]
    w_out = np.asarray(inputs["w_out"], f32)[sl]         # [2, 2048, 512]

    # DoubleRow lhsT layouts (pair index j adjacent to the 128-wide m dim)
    w_in8 = np.ascontiguousarray(
        _q8w(w_in).reshape(EL, 2, 2, P, 16, P)
        .transpose(0, 3, 1, 4, 2, 5))                    # [EL, p, kp, hc, j, m]
    w1a = _q8w(w1[:, :, :HD]).reshape(EL, 8, 2, P, 16, P)
    w1b = _q8w(w1[:, :, HD:]).reshape(EL, 8, 2, P, 16, P)
    w18 = np.stack([w1a, w1b], axis=2)                   # [EL, kp, ab, j, p, pr, m]
    w18 = np.ascontiguousarray(
        w18.transpose(0, 5, 4, 2, 1, 3, 6))              # [EL, pr, p, ab, kp, j, m]
    w28 = _q8w(w2).reshape(EL, 8, 2, P, 16, P) \
        .transpose(0, 4, 3, 1, 2, 5)                     # [EL, oc, p, kp, j, m]
    w28 = np.ascontiguousarray(
        w28.reshape(EL, 4, 4, P, 8, 2, P)
        .transpose(0, 1, 3, 2, 4, 5, 6))                 # [EL, og, p, ocl, kp, j, m]
    wout8 = np.ascontiguousarray(
        _q8w(w_out).reshape(EL, 8, 2, P, 4, P)
        .transpose(0, 3, 1, 4, 2, 5))                    # [EL, p, kp, dc, j, m]

    def bias_t(key, scale, w):
        b = np.asarray(inputs[key], f32)[sl] * scale     # [2, w*128]
        return np.ascontiguousarray(b.reshape(EL, w, P).transpose(2, 0, 1))

    mfeat = np.zeros((D,), f32)
    mfeat[64 * core:64 * core + 64] = 1.0
    smalls = np.concatenate([
        np.ascontiguousarray(mfeat.reshape(4, P).T),
        np.ascontiguousarray((projb * c0).reshape(4, P).T),
        np.broadcast_to(np.asarray(inputs["router_b"], f32), (P, E)),
        projb_col,
        projb_col * c0,
        np.full((P, 1), c0 / S_Y, f32),
        np.broadcast_to(bv, (P, HDIM)),
    ], axis=1).astype(f32)                               # [P, 97]
    bqa = np.zeros((HDIM, 4), f32)
    bqa[:, 0] = bq
    bqa[:, 1] = bk
    bqa[0, 3] = np.asarray(inputs["alpha"], f32)[h]
    wbig = np.concatenate([
        wqkv.reshape(4, P, 192).transpose(1, 0, 2).reshape(P, 768),
        maskd.reshape(P, 4 * D),
        np.asarray(inputs["router_w"], f32)
        .reshape(4, P, E).transpose(1, 0, 2).reshape(P, 4 * E),
    ], axis=1).astype(bf)                                # [P, 2880]
    b1 = bias_t("b1", 1.0, 32)
    ebias = np.concatenate([
        bias_t("b_in", S_H, 16),
        b1[:, :, :16] * S_S,
        b1[:, :, 16:],
        bias_t("b2", S_O, 16),
        bias_t("b_out", S_O * S_W, 4),
    ], axis=2).astype(f32)                               # [P, 2, 68]
    return {
        "xtb": np.ascontiguousarray(
            xT.reshape(4, P, N).transpose(1, 0, 2)).astype(bf),
        "xmask": xmask,
        "smalls": smalls,
        "bqalpha": bqa,
        "wbig": wbig,
        "grow": np.concatenate(
            [g, np.full((HDIM,), np.asarray(inputs["alpha"], f32)[h])]
        ).reshape(1, D + HDIM).astype(bf),
        "selb": selb.astype(bf),
        "w_in8": w_in8, "w18": w18, "w28": w28, "wout8": wout8,
        "ebias": ebias,
    }


last_result = [None]


def kernel(**inputs):
    if "nc" not in _cache:
        _cache["nc"] = build_program()
    nc = _cache["nc"]
    in_maps = [_prep_inputs(inputs, c) for c in range(NCORES)]
    res = run_bass_kernel_spmd(nc, in_maps, core_ids=list(range(NCORES)))
    last_result[0] = res
    outT = np.concatenate(
        [np.asarray(res.results[c]["out"]).astype(np.float32)
         for c in range(NCORES)], axis=0)                # [512, 2048]
    return np.ascontiguousarray(outT.T).reshape(2, 1024, 512)


# revision 55
# speedup vs baseline: 1.1026x; 1.0008x over previous
"""MoE transformer block (QK-norm attention + top-8-of-16 MoE) on 8 trn2 cores.

v4: dense fp8 DoubleRow experts (as v2 baseline) with restructured
scheduling:
- batch-split expert pipeline: each expert runs a b0-pass (tokens 0-1023)
  then a b1-pass, so expert-0's b0 compute starts right after AllGather-0
  and fully hides AllGather-1 + proj-b1;
- the ReduceScatter is split per batch: RS(b0)'s input is complete after
  the last expert's b0-pass and it runs hidden under ~95us of b1 compute,
  leaving only RS(b1) (~18us) in the tail;
- attention-phase cost cuts: softmax denominator folded into an augmented
  v row (kills 24 denominator matmuls), all broadcast matmuls in bf16
  (1 cyc/row instead of 4), RMSNorm's g folded into the broadcast and its
  additive b folded into the qkv biases (host-side);
- SwiGLU's a-path and the moe combine adds alternate DVE/ACT/GpSimd to
  keep the vector engine off the critical path;
- startup DMAs split so RMSNorm starts after the first 0.5MB chunk.

Sharding: attention head-parallel (core c owns head c), experts
expert-parallel (core c owns experts 2c, 2c+1), output feature-parallel
(core c returns features [64c, 64c+64) for all tokens).

Everything feature-major ("T layout": features on partitions, tokens on
free). Scales (powers of two, exact): x_res*2^5, weights*2^11, h1*2^5,
s*2^5, o*2^7; descales folded into Act/DVE scale+bias immediates.
"""

import numpy as np
import ml_dtypes

import concourse.bass as bass
import concourse.mybir as mybir
from concourse.tile import TileContext
from concourse.masks import make_identity
from concourse.bass_utils import run_bass_kernel_spmd

BF16 = mybir.dt.bfloat16
F32 = mybir.dt.float32
F8 = mybir.dt.float8e4
AFT = mybir.ActivationFunctionType
MUL = mybir.AluOpType.mult
ADD = mybir.AluOpType.add
DR = mybir.MatmulPerfMode.DoubleRow

P = 128
D = 512          # embed dim
T = 1024         # tokens per batch
N = 2048         # total tokens
E = 16           # experts
EL = 2           # experts per core
HD = 2048        # expert hidden
HDIM = 64        # head dim
NCORES = 8

S_X = 2.0 ** 5
S_W = 2.0 ** 11
S_H = 2.0 ** 5
S_S = 2.0 ** 5
S_O = 2.0 ** 7
S_Y = 2.0 ** 5

_cache = {}


def build_program():
    nc = bass.Bass()
    dp_ = dict(isOutput=False)
    xtb_d = nc.declare_dram_parameter("xtb", [P, 4, N], BF16, **dp_)
    xm_d = nc.declare_dram_parameter("xmask", [P, 4, N], BF16, **dp_)
    sm_d = nc.declare_dram_parameter("smalls", [P, 97], F32, **dp_)
    bqa_d = nc.declare_dram_parameter("bqalpha", [HDIM, 4], F32, **dp_)
    wbig_d = nc.declare_dram_parameter("wbig", [P, 2880], BF16, **dp_)
    grow_d = nc.declare_dram_parameter("grow", [1, D + HDIM], BF16,
                                       **dp_)
    sel_d = nc.declare_dram_parameter("selb", [E, EL, P], BF16, **dp_)
    win_d = nc.declare_dram_parameter("w_in8", [EL, P, 2, 16, 2, P], F8, **dp_)
    w1_d = nc.declare_dram_parameter("w18", [EL, 16, P, 2, 8, 2, P], F8, **dp_)
    w2_d = nc.declare_dram_parameter("w28", [EL, 4, P, 4, 8, 2, P], F8, **dp_)
    wout_d = nc.declare_dram_parameter("wout8", [EL, P, 8, 4, 2, P], F8, **dp_)
    eb_d = nc.declare_dram_parameter("ebias", [P, EL, 68], F32, **dp_)
    out_d = nc.declare_dram_parameter("out", [HDIM, N], BF16, isOutput=True)

    groups = [list(range(NCORES))]

    with TileContext(nc, num_cores=NCORES) as tc:
        with (
            tc.tile_pool(name="const", bufs=1) as cp,
            tc.tile_pool(name="pp", bufs=1) as pp,
            tc.tile_pool(name="psE", bufs=4, space="PSUM") as psE,
            tc.tile_pool(name="psB", bufs=2, space="PSUM") as psB,
            tc.tile_pool(name="psS", bufs=2, space="PSUM") as psS,
            tc.tile_pool(name="dram", bufs=1, space="DRAM") as dp,
        ):
            # ---- constants / persistent ----
            ws = tc.alloc_tile_pool(name="wst", bufs=1)
            s1pre = tc.alloc_tile_pool(name="s1pre", bufs=1)
            xTb = s1pre.tile([P, 4, N], BF16, tag="xTb")
            nc.sync.dma_start(xTb[:, :, 0:D], xtb_d[:, :, 0:D])
            wbig = s1pre.tile([P, 2880], BF16, tag="wbig")
            nc.sync.dma_start(wbig[:, 0:768], wbig_d[:, 0:768])
            for cc in range(1, 4):
                sl = slice(cc * D, (cc + 1) * D)
                nc.sync.dma_start(xTb[:, :, sl], xtb_d[:, :, sl])
            ones128b = cp.tile([P, 1], BF16, tag="ones128b")
            nc.vector.memset(ones128b, 1.0)
            ones64b = cp.tile([HDIM, 1], BF16, tag="ones64b")
            nc.vector.memset(ones64b, 1.0)
            ones1rb = cp.tile([1, P], BF16, tag="ones1rb")
            nc.vector.memset(ones1rb, 1.0)
            sm = cp.tile([P, 97], F32, tag="sm")
            nc.sync.dma_start(sm, sm_d[:, :])
            mfeat_sb = sm[:, 0:4]
            pbc0_sb = sm[:, 4:8]
            rb_sb = sm[:, 8:24]
            projb_sb = sm[:, 24:28]
            c0_sb = sm[:, 32:33]
            vbias_sb = sm[:, 33:97]
            sel_sb = cp.tile([E, EL, P], BF16, tag="sel_sb")
            nc.sync.dma_start(sel_sb, sel_d[:, :, :])
            eps6 = cp.tile([1, 1], F32, tag="eps6")
            nc.vector.memset(eps6, 1e-6)
            grow = cp.tile([1, D + HDIM], BF16, tag="grow")
            nc.sync.dma_start(grow, grow_d[:, :])
            identb = cp.tile([P, P], BF16, tag="identb")
            make_identity(nc, identb)

            # persistent activations for the expert phase
            moeT = pp.tile([P, 4, N], BF16, tag="moeT")
            xrT8 = pp.tile([P, 4, N], F8, tag="xrT8")
            gatesT = pp.tile([E, N], BF16, tag="gatesT")
            h1T8 = pp.tile([P, 16, N], F8, tag="h1T8")

            ag_in = [dp.tile([HDIM, T], F8, name=f"ag_in{i}")
                     for i in range(2)]
            ag_out = [dp.tile([D, T], F8, addr_space="Shared",
                              name=f"ag_out{i}")
                      for i in range(2)]
            rs_in = [dp.tile([D, T], BF16, name=f"rs_in{i}")
                     for i in range(2)]
            rs_out = [dp.tile([HDIM, T], BF16, name=f"rs_out{i}")
                      for i in range(2)]

            with tc.tile_pool(name="s1", bufs=1) as s1:
                nc.sync.dma_start(wbig[:, 768:2880], wbig_d[:, 768:2880])
                bqa = s1.tile([HDIM, 4], F32, tag="bqa")
                nc.sync.dma_start(bqa, bqa_d[:, :])
                bq_sb = bqa[:, 0:3]
                alpha_sb = bqa[0:1, 3:4]
                # prefetch: expert biases + e0 weights (consumed ~90us later)
                ebt = pp.tile([P, EL, 68], F32, tag="ebias")
                nc.sync.dma_start(ebt, eb_d[:, :, :])
                bias_cp = {"bin": ebt[:, :, 0:16], "b1a": ebt[:, :, 16:32],
                           "b1b": ebt[:, :, 32:48], "b2": ebt[:, :, 48:64],
                           "bout": ebt[:, :, 64:68]}
                pre_win = ws.tile([P, 2, 16, 2, P], F8, tag="win", bufs=1)
                nc.sync.dma_start(pre_win, win_d[0])
                pre_wout = ws.tile([P, 8, 4, 2, P], F8, tag="wot", bufs=1)
                nc.sync.dma_start(pre_wout, wout_d[0])
                pre_w1 = []
                for pr in range(3):
                    t = ws.tile([P, 2, 8, 2, P], F8, tag="w1s", bufs=4)
                    nc.sync.dma_start(t, w1_d[0, pr])
                    pre_w1.append(t)
                # ---- RMSNorm + qkv + qk-norm + attention, batch-ordered:
                # all of batch b's chain runs before batch b+1 so AG(b)
                # issues early and b1 prep fills the AG0 window ----
                with tc.tile_pool(name="attp", bufs=1) as ap_, \
                     tc.tile_pool(name="ate", bufs=12) as ate:
                    xnT = ap_.tile([P, 4, N], BF16, tag="xnT")
                    qT = ap_.tile([HDIM, N], BF16, tag="qT")
                    kT = ap_.tile([HDIM, N], BF16, tag="kT")
                    v_aug = ap_.tile([P, 16, HDIM + 1], BF16, tag="v_aug")
                    nc.vector.memset(v_aug, 1.0)
                    qh = ap_.tile([HDIM, N], BF16, tag="qh")
                    kh = ap_.tile([HDIM, N], BF16, tag="kh")
                    yhT = ap_.tile([HDIM, N], F8, tag="yhT")
                    for b in range(2):
                        for cc in (2 * b, 2 * b + 1):
                            sl = slice(cc * D, (cc + 1) * D)
                            ps = psS.tile([1, D], F32, tag="ps_small")
                            for kc in range(4):
                                sq = ate.tile([P, D], BF16, tag="sq_t",
                                              bufs=3)
                                nc.scalar.activation(sq, xTb[:, kc, sl],
                                                     AFT.Square)
                                nc.tensor.matmul(ps, ones128b, sq,
                                                 start=(kc == 0),
                                                 stop=(kc == 3))
                            tmp = ate.tile([1, D], F32, tag="r_t", bufs=2)
                            nc.scalar.activation(tmp, ps, AFT.Sqrt,
                                                 scale=1.0 / D,
                                                 bias=eps6[0:1, 0:1])
                            rrow = ate.tile([1, D], BF16, tag="rrow",
                                            bufs=2)
                            with nc.allow_low_precision(
                                    reason="bf16 bcast row"):
                                nc.vector.reciprocal(rrow, tmp)
                            # xnT = xTb * bcast(rrow * g)
                            for kc in range(4):
                                pb = psB.tile([P, D], F32, tag="bc")
                                nc.tensor.matmul(
                                    pb, grow[0:1, kc * P:(kc + 1) * P],
                                    rrow[0:1, :], start=True, stop=True)
                                nc.vector.tensor_mul(xnT[:, kc, sl],
                                                     xTb[:, kc, sl], pb)
                            for wi, dst, bi in ((0, qT, 0), (1, kT, 1)):
                                ps2 = psS.tile([HDIM, D], F32,
                                               tag="ps_small")
                                for kc in range(4):
                                    nc.tensor.matmul(
                                        ps2,
                                        wbig[:, kc * 192 + wi * HDIM:
                                             kc * 192 + (wi + 1) * HDIM],
                                        xnT[:, kc, sl], start=(kc == 0),
                                        stop=(kc == 3))
                                nc.scalar.activation(
                                    dst[:, sl], ps2, AFT.Identity,
                                    bias=bq_sb[:, bi:bi + 1])
                            for tk in range(cc * 4, cc * 4 + 4):
                                ps3 = psS.tile([P, HDIM], F32,
                                               tag="ps_small")
                                for kc in range(4):
                                    nc.tensor.matmul(
                                        ps3, xnT[:, kc, tk * P:(tk + 1) * P],
                                        wbig[:, kc * 192 + 128:
                                             kc * 192 + 192],
                                        start=(kc == 0), stop=(kc == 3))
                                nc.vector.tensor_add(v_aug[:, tk, 0:HDIM],
                                                     ps3, vbias_sb)
                            # qk-norm for this chunk (alpha folded in the
                            # q bcast row)
                            for src_, dst, brow in (
                                    (qT, qh, grow[0:1, D:D + HDIM]),
                                    (kT, kh, ones1rb[0:1, 0:HDIM])):
                                sq = ate.tile([HDIM, D], BF16, tag="sqn",
                                              bufs=2)
                                nc.scalar.activation(sq, src_[:, sl],
                                                     AFT.Square)
                                ps4 = psS.tile([1, D], F32, tag="ps_small")
                                nc.tensor.matmul(ps4, ones64b, sq,
                                                 start=True, stop=True)
                                t = ate.tile([1, D], F32, tag="rn_t",
                                             bufs=2)
                                nc.scalar.activation(t, ps4, AFT.Sqrt)
                                nc.vector.tensor_scalar_add(t, t, 1e-5)
                                rn = ate.tile([1, D], BF16, tag="rn",
                                              bufs=2)
                                with nc.allow_low_precision(
                                        reason="bf16 row"):
                                    nc.vector.reciprocal(rn, t)
                                pb = psB.tile([HDIM, D], F32, tag="bc")
                                nc.tensor.matmul(pb, brow, rn[0:1, :],
                                                 start=True, stop=True)
                                nc.vector.tensor_mul(dst[:, sl],
                                                     src_[:, sl], pb)
                        # scoresT -> exp (masked diag) -> AV (denominator
                        # folded into v_aug's ones row) -> yhT -> AG(b)
                        for qc in range(2):
                            qsl = slice(b * T + qc * D, b * T + (qc + 1) * D)
                            nkc = 4 * qc + 4
                            ex_tiles = []
                            for kc in range(nkc):
                                ksl = slice(b * T + kc * P,
                                            b * T + (kc + 1) * P)
                                ps = psE.tile([P, D], F32, tag="mm")
                                nc.tensor.matmul(ps, kh[:, ksl], qh[:, qsl],
                                                 start=True, stop=True)
                                if kc >= 4 * qc:  # diagonal block: mask
                                    et = ate.tile([P, D], BF16, tag="exp_b",
                                                  bufs=4)
                                    nc.scalar.activation(et, ps, AFT.Exp)
                                    eb2 = ate.tile([P, D], BF16,
                                                   tag="exp_m", bufs=7)
                                    nc.vector.tensor_mul(
                                        eb2, et,
                                        wbig[:, 768 + (kc - 4 * qc) * D:
                                             768 + (kc - 4 * qc + 1) * D])
                                else:
                                    eb2 = ate.tile([P, D], BF16,
                                                   tag="exp_m", bufs=7)
                                    nc.scalar.activation(eb2, ps, AFT.Exp)
                                ex_tiles.append(eb2)
                            py = psS.tile([HDIM + 1, D], F32,
                                          tag="ps_small")
                            for kc in range(nkc):
                                nc.tensor.matmul(py, v_aug[:, b * 8 + kc, :],
                                                 ex_tiles[kc],
                                                 start=(kc == 0),
                                                 stop=(kc == nkc - 1))
                            dr = ate.tile([1, D], BF16, tag="dr", bufs=2)
                            with nc.allow_low_precision(
                                    reason="bf16 softmax denom"):
                                nc.vector.reciprocal(dr,
                                                     py[HDIM:HDIM + 1, :])
                            pb2 = psB.tile([HDIM, D], F32, tag="bc")
                            nc.tensor.matmul(pb2, ones1rb[0:1, 0:HDIM], dr,
                                             start=True, stop=True)
                            db = ate.tile([HDIM, D], BF16, tag="db", bufs=2)
                            nc.scalar.activation(db, pb2, AFT.Copy,
                                                 scale=S_Y)
                            nc.vector.tensor_mul(yhT[:, qsl],
                                                 py[0:HDIM, :], db)
                            nc.sync.dma_start(
                                ag_in[b][:, qc * D:(qc + 1) * D],
                                yhT[:, qsl])
                        nc.gpsimd.collective_compute(
                            "AllGather", mybir.AluOpType.bypass,
                            ins=[ag_in[b][:]], outs=[ag_out[b][:]],
                            replica_groups=groups)

                # ---- proj + x_res + router (per batch), interleaved with
                # expert passes so weight DMAs never queue behind the
                # AG1-blocked b1 input DMAs ----
                pass
            # (s1 stays open: xTb / qT / kT / v_aug / wbig / bqa)
            with tc.tile_pool(name="s2", bufs=1) as s2, \
                 tc.tile_pool(name="s2e", bufs=4) as s2e, \
                 tc.tile_pool(name="eact", bufs=1) as ac, \
                 tc.tile_pool(name="eev", bufs=6) as ev_:
                yT_sb = s2.tile([P, 4, N], F8, tag="yT_sb")
                routes = s2.tile([P, 16, E], F32, tag="routes")
                gates = routes
                gsum = s2.tile([P, 16], F32, tag="gsum")
                gates_bf = s2.tile([P, 16, E], BF16, tag="gates_bf")

                def emit_batch(b):
                    for ts in (2 * b, 2 * b + 1):
                        nc.sync.dma_start(
                            yT_sb[:, :, ts * D:(ts + 1) * D],
                            ag_out[b][:, (ts % 2) * D:(ts % 2 + 1) * D]
                            .rearrange("(kc p) n -> p kc n", p=P))
                    for ts in (2 * b, 2 * b + 1):
                        for dc in range(4):
                            tsl = slice(ts * D, (ts + 1) * D)
                            ps = psE.tile([P, D], F32, tag="mm")
                            for kc in range(4):
                                nc.tensor.matmul(
                                    ps,
                                    wbig[:, 2816 + kc * D + dc * P:
                                         2816 + kc * D + (dc + 1) * P],
                                    yT_sb[:, kc, tsl],
                                    start=(kc == 0), stop=(kc == 3))
                            # xmb = x*featmask + projb*c0 (replaces the
                            # host xmask tensor)
                            xmb = s2e.tile([P, D], BF16, tag="xmb", bufs=3)
                            nc.vector.tensor_scalar(
                                xmb, xTb[:, dc, tsl],
                                mfeat_sb[:, dc:dc + 1],
                                pbc0_sb[:, dc:dc + 1], op0=MUL, op1=ADD)
                            tmp = s2e.tile([P, D], F32, tag="yp_t", bufs=3)
                            nc.scalar.activation(
                                tmp, ps, AFT.Identity, scale=1.0 / S_Y,
                                bias=projb_sb[:, dc:dc + 1])
                            # x_res written in place over xTb
                            nc.vector.tensor_add(xTb[:, dc, tsl], tmp,
                                                 xTb[:, dc, tsl])
                            nc.scalar.activation(xrT8[:, dc, tsl],
                                                 xTb[:, dc, tsl],
                                                 AFT.Copy, scale=S_X)
                            # moeT init: yp*c0 + x feature slice
                            nc.vector.scalar_tensor_tensor(
                                moeT[:, dc, tsl], ps, c0_sb[:, 0:1],
                                xmb, op0=MUL, op1=ADD)
                def emit_gates(b):
                    # router for this batch -> normalized top-8 gates
                    for tk in range(8 * b, 8 * b + 8):
                        ps = psS.tile([P, E], F32, tag="ps_small")
                        for kc in range(4):
                            nc.tensor.matmul(
                                ps, xTb[:, kc, tk * P:(tk + 1) * P],
                                wbig[:, 2816 + kc * E:2816 + (kc + 1) * E],
                                start=(kc == 0), stop=(kc == 3))
                        nc.vector.tensor_add(routes[:, tk, :], ps, rb_sb)
                    nc.scalar.activation(routes[:, 8 * b:8 * b + 8, :],
                                         routes[:, 8 * b:8 * b + 8, :],
                                         AFT.Exp)
                    for g in range(8 * b, 8 * b + 8):
                        m8 = s2e.tile([P, 8], F32, tag="m8", bufs=2)
                        nc.vector.max(out=m8, in_=routes[:, g, :])
                        zap = s2e.tile([P, E], F32, tag="zap", bufs=2)
                        nc.vector.match_replace(out=zap, in_to_replace=m8,
                                                in_values=routes[:, g, :],
                                                imm_value=0)
                        nc.vector.tensor_sub(gates[:, g, :],
                                             routes[:, g, :], zap)
                    nc.vector.reduce_sum(gsum[:, 8 * b:8 * b + 8],
                                         gates[:, 8 * b:8 * b + 8, :],
                                         axis=mybir.AxisListType.X)
                    nc.vector.reciprocal(gsum[:, 8 * b:8 * b + 8],
                                         gsum[:, 8 * b:8 * b + 8])
                    for g in range(8 * b, 8 * b + 8):
                        nc.vector.tensor_scalar_mul(gates_bf[:, g, :],
                                                    gates[:, g, :],
                                                    gsum[:, g:g + 1])
                    for g in range(8 * b, 8 * b + 8):
                        pt = psS.tile([E, P], BF16, tag="ps_small")
                        nc.tensor.transpose(pt, gates_bf[:, g, :], identb)
                        nc.scalar.activation(
                            gatesT[:, g * P:(g + 1) * P], pt, AFT.Copy)

                held = {}

                def emit_pass(e, half, post_h1=None, post_sw=None):
                    ts_range = (2 * half, 2 * half + 1)
                    if half == 0:
                        if e == 0:
                            win_t = pre_win
                            wout_t = pre_wout
                        else:
                            win_t = ws.tile([P, 2, 16, 2, P], F8,
                                            tag="win", bufs=1, name="win_t")
                            nc.sync.dma_start(win_t, win_d[e])
                            wout_t = ws.tile([P, 8, 4, 2, P], F8,
                                             tag="wot", bufs=1,
                                             name="wout_t")
                            nc.sync.dma_start(wout_t, wout_d[e])
                        held[e] = (win_t, wout_t)
                    else:
                        win_t, wout_t = held[e]
                    # h1 = x_res @ w_in  (fp8, S_H)
                    for ts in ts_range:
                        for hc in range(16):
                            tsl = slice(ts * D, (ts + 1) * D)
                            ps = psE.tile([P, D], F32, tag="mm")
                            for kp in range(2):
                                nc.tensor.matmul(
                                    ps, win_t[:, kp, hc, :, :],
                                    xrT8[:, 2 * kp:2 * kp + 2, tsl],
                                    start=(kp == 0), stop=(kp == 1),
                                    perf_mode=DR)
                            if (hc + ts) % 2 == 0:
                                nc.scalar.activation(
                                    h1T8[:, hc, tsl], ps, AFT.Identity,
                                    scale=S_H / (S_X * S_W),
                                    bias=bias_cp["bin"][:, e, hc:hc + 1])
                            else:
                                nc.vector.tensor_scalar(
                                    h1T8[:, hc, tsl], ps,
                                    S_H / (S_X * S_W),
                                    bias_cp["bin"][:, e, hc:hc + 1],
                                    op0=MUL, op1=ADD)
                    if post_h1 is not None:
                        post_h1()
                    # SwiGLU: s = silu(h@w1b + b1b) * (h@w1a + b1a)
                    sT8 = ac.tile([P, 16, T], F8, tag="sT8", bufs=1,
                                  name="sT8")
                    for pr in range(16):
                        if e == 0 and half == 0 and pr < 3:
                            w1_t = pre_w1[pr]
                        else:
                            w1_t = ws.tile([P, 2, 8, 2, P], F8, tag="w1s",
                                           bufs=4, name="w1_t")
                            nc.sync.dma_start(w1_t, w1_d[e, pr])
                        for ts in ts_range:
                            tsl = slice(ts * D, (ts + 1) * D)
                            pa = psE.tile([P, D], F32, tag="mm")
                            pb = psE.tile([P, D], F32, tag="mm")
                            for kp in range(8):
                                nc.tensor.matmul(
                                    pa, w1_t[:, 0, kp, :, :],
                                    h1T8[:, 2 * kp:2 * kp + 2, tsl],
                                    start=(kp == 0), stop=(kp == 7),
                                    perf_mode=DR)
                            for kp in range(8):
                                nc.tensor.matmul(
                                    pb, w1_t[:, 1, kp, :, :],
                                    h1T8[:, 2 * kp:2 * kp + 2, tsl],
                                    start=(kp == 0), stop=(kp == 7),
                                    perf_mode=DR)
                            sil = ev_.tile([P, D], BF16, tag="sil", bufs=4)
                            nc.scalar.activation(
                                sil, pb, AFT.Silu,
                                scale=1.0 / (S_H * S_W),
                                bias=bias_cp["b1b"][:, e, pr:pr + 1])
                            av8 = ev_.tile([P, D], F8, tag="av8", bufs=4)
                            if (pr + ts) % 4 != 3:
                                nc.scalar.activation(
                                    av8, pa, AFT.Identity,
                                    scale=S_S / (S_H * S_W),
                                    bias=bias_cp["b1a"][:, e, pr:pr + 1])
                            else:
                                nc.vector.tensor_scalar(
                                    av8, pa, S_S / (S_H * S_W),
                                    bias_cp["b1a"][:, e, pr:pr + 1],
                                    op0=MUL, op1=ADD)
                            ltsl = slice((ts - 2 * half) * D,
                                         (ts - 2 * half + 1) * D)
                            nc.vector.tensor_mul(sT8[:, pr, ltsl], av8, sil)
                    if post_sw is not None:
                        post_sw()
                    # o = s @ w2 + b2  (fp8, S_O)
                    oT8 = ac.tile([P, 16, T], F8, tag="oT8", bufs=1,
                                  name="oT8")
                    for og in range(4):
                        w2_t = ws.tile([P, 4, 8, 2, P], F8, tag="w2s",
                                       bufs=2, name="w2_t")
                        nc.sync.dma_start(w2_t, w2_d[e, og])
                        for oc4 in range(4):
                            oc = og * 4 + oc4
                            for ts in ts_range:
                                ltsl = slice((ts - 2 * half) * D,
                                             (ts - 2 * half + 1) * D)
                                ps = psE.tile([P, D], F32, tag="mm")
                                for kp in range(8):
                                    nc.tensor.matmul(
                                        ps, w2_t[:, oc4, kp, :, :],
                                        sT8[:, 2 * kp:2 * kp + 2, ltsl],
                                        start=(kp == 0), stop=(kp == 7),
                                        perf_mode=DR)
                                if (oc + ts) % 4 != 3:
                                    nc.vector.tensor_scalar(
                                        oT8[:, oc, ltsl], ps,
                                        S_O / (S_S * S_W),
                                        bias_cp["b2"][:, e, oc:oc + 1],
                                        op0=MUL, op1=ADD)
                                else:
                                    nc.scalar.activation(
                                        oT8[:, oc, ltsl], ps, AFT.Identity,
                                        scale=S_O / (S_S * S_W),
                                        bias=bias_cp["b2"][:, e, oc:oc + 1])
                    # gate broadcast [tokens] -> [P, D] per ts chunk
                    gb_tiles = {}
                    for ts in ts_range:
                        tsl = slice(ts * D, (ts + 1) * D)
                        pg = psB.tile([P, D], F32, tag="bc")
                        nc.tensor.matmul(pg, sel_sb[:, e, :], gatesT[:, tsl],
                                         start=True, stop=True)
                        gb = ev_.tile([P, D], BF16, tag="gb", bufs=4)
                        nc.scalar.activation(gb, pg, AFT.Copy,
                                             scale=1.0 / (S_O * S_W))
                        gb_tiles[ts] = gb
                    # eo = o @ w_out + b_out; moeT += gate * eo
                    for dc in range(4):
                        for ts in ts_range:
                            tsl = slice(ts * D, (ts + 1) * D)
                            ltsl = slice((ts - 2 * half) * D,
                                         (ts - 2 * half + 1) * D)
                            ps = psE.tile([P, D], F32, tag="mm")
                            for kp in range(8):
                                nc.tensor.matmul(
                                    ps, wout_t[:, kp, dc, :, :],
                                    oT8[:, 2 * kp:2 * kp + 2, ltsl],
                                    start=(kp == 0), stop=(kp == 7),
                                    perf_mode=DR)
                            t2 = ev_.tile([P, D], F32, tag="t2", bufs=3)
                            nc.vector.scalar_tensor_tensor(
                                t2, ps, bias_cp["bout"][:, e, dc:dc + 1],
                                gb_tiles[ts], op0=ADD, op1=MUL)
                            if (dc + ts) % 2 == 0 or \
                                    (e, half, dc) == (1, 1, 3):
                                nc.vector.tensor_add(moeT[:, dc, tsl],
                                                     moeT[:, dc, tsl], t2)
                            else:
                                nc.gpsimd.tensor_add(moeT[:, dc, tsl],
                                                     moeT[:, dc, tsl], t2)
                            if e == 1:
                                nc.sync.dma_start(
                                    rs_in[half][dc * P:(dc + 1) * P,
                                                (ts - 2 * half) * D:
                                                (ts - 2 * half + 1) * D],
                                    moeT[:, dc, tsl])
                    if e == 1:
                        nc.gpsimd.collective_compute(
                            "ReduceScatter", mybir.AluOpType.add,
                            ins=[rs_in[half][:]], outs=[rs_out[half][:]],
                            replica_groups=groups)
                        nc.sync.dma_start(
                            out_d[:, half * T:(half + 1) * T],
                            rs_out[half][:, :])

                emit_batch(0)
                emit_pass(0, 0, post_h1=lambda: emit_gates(0),
                          post_sw=lambda: emit_batch(1))
                emit_pass(0, 1, post_h1=lambda: emit_gates(1))
                emit_pass(1, 0)
                emit_pass(1, 1)
            s1pre.release()

            ws.release()

    _split_matmul_waits(nc)
    return nc


def _split_matmul_waits(nc):
    """walrus allows only one sync-wait per engine-instruction sync slot; move
    extra waits onto standalone InstEventSemaphore waits inserted before."""
    import concourse.mybir as mybir
    k = 0
    for bb in nc.main_func.blocks:
        il = list(bb.instructions)
        out = []
        changed = False
        for ins in il:
            si = getattr(ins, "sync_info", None)
            if si is not None and len(si.on_wait) > 1 \
                    and type(ins).__name__ != "InstEventSemaphore":
                waits = list(si.on_wait)
                keep, move = waits[-1], waits[:-1]
                for w in move:
                    nop = mybir.InstEventSemaphore(name=f"I-wsplit-{k}",
                                                   ins=[], outs=[])
                    k += 1
                    nop.engine = ins.engine
                    nop.sync_info = type(si)(on_wait=[w], on_update=[])
                    out.append(nop)
                ins.sync_info = type(si)(on_wait=[keep],
                                         on_update=list(si.on_update))
                changed = True
            out.append(ins)
        if changed:
            bb.instructions = out


def _q8w(w):
    """host fp8 cast with fixed 2^11 scale (clipped to TRN e4m3 max)."""
    return np.clip(np.asarray(w, np.float32) * S_W, -240.0, 240.0).astype(
        ml_dtypes.float8_e4m3)


def _prep_inputs(inputs, core):
    bf = ml_dtypes.bfloat16
    f32 = np.float32
    h = core
    x = np.asarray(inputs["x"], f32).reshape(N, D)
    xT = np.ascontiguousarray(x.T)                      # [512, 2048]
    g = np.asarray(inputs["g"], f32)
    bvec = np.asarray(inputs["b"], f32)
    caw = np.asarray(inputs["c_attn_w"], f32)
    cab = np.asarray(inputs["c_attn_b"], f32)
    wq = caw[:, h * 64:(h + 1) * 64]
    wk = caw[:, 512 + h * 64:512 + (h + 1) * 64]
    wv = caw[:, 1024 + h * 64:1024 + (h + 1) * 64]
    wqkv = np.concatenate([wq, wk, wv], axis=1)          # [512, 192]
    # RMSNorm additive b folded into qkv biases
    bq = bvec @ wq + cab[h * 64:(h + 1) * 64]
    bk = bvec @ wk + cab[512 + h * 64:512 + (h + 1) * 64]
    bv = bvec @ wv + cab[1024 + h * 64:1024 + (h + 1) * 64]
    kk = np.arange(4)[None, :, None] * P + np.arange(P)[:, None, None]
    qq = np.arange(D)[None, None, :]
    maskd = (kk <= qq).astype(f32)                       # [128, 4, 512]
    projb = np.asarray(inputs["c_proj_b"], f32)
    projb_col = np.ascontiguousarray(projb.reshape(4, P).T)  # [p, dc]
    c0 = 1.0 if core == 0 else 0.0
    xmask = np.zeros((D, N), f32)
    xmask[64 * core:64 * core + 64, :] = xT[64 * core:64 * core + 64, :]
    xmask += (projb * c0)[:, None]
    xmask = np.ascontiguousarray(
        xmask.reshape(4, P, N).transpose(1, 0, 2)).astype(bf)
    selb = np.zeros((E, EL, P), f32)
    selb[2 * core, 0, :] = 1.0
    selb[2 * core + 1, 1, :] = 1.0

    sl = slice(2 * core, 2 * core + 2)
    w_in = np.asarray(inputs["w_in"], f32)[sl]           # [2, 512, 2048]
    w1 = np.asarray(inputs["w1"], f32)[sl]               # [2, 2048, 4096]
    w2 = np.asarray(inputs["w2"], f32)[sl]               # [2, 2048, 2048]
    w_out = np.asarray(inputs["w_out"], f32)[sl]         # [2, 2048, 512]

    # DoubleRow lhsT layouts (pair index j adjacent to the 128-wide m dim)
    w_in8 = np.ascontiguousarray(
        _q8w(w_in).reshape(EL, 2, 2, P, 16, P)
        .transpose(0, 3, 1, 4, 2, 5))                    # [EL, p, kp, hc, j, m]
    w1a = _q8w(w1[:, :, :HD]).reshape(EL, 8, 2, P, 16, P)
    w1b = _q8w(w1[:, :, HD:]).reshape(EL, 8, 2, P, 16, P)
    w18 = np.stack([w1a, w1b], axis=2)                   # [EL, kp, ab, j, p, pr, m]
    w18 = np.ascontiguousarray(
        w18.transpose(0, 5, 4, 2, 1, 3, 6))              # [EL, pr, p, ab, kp, j, m]
    w28 = _q8w(w2).reshape(EL, 8, 2, P, 16, P) \
        .transpose(0, 4, 3, 1, 2, 5)                     # [EL, oc, p, kp, j, m]
    w28 = np.ascontiguousarray(
        w28.reshape(EL, 4, 4, P, 8, 2, P)
        .transpose(0, 1, 3, 2, 4, 5, 6))                 # [EL, og, p, ocl, kp, j, m]
    wout8 = np.ascontiguousarray(
        _q8w(w_out).reshape(EL, 8, 2, P, 4, P)
        .transpose(0, 3, 1, 4, 2, 5))                    # [EL, p, kp, dc, j, m]

    def bias_t(key, scale, w):
        b = np.asarray(inputs[key], f32)[sl] * scale     # [2, w*128]
        return np.ascontiguousarray(b.reshape(EL, w, P).transpose(2, 0, 1))

    mfeat = np.zeros((D,), f32)
    mfeat[64 * core:64 * core + 64] = 1.0
    smalls = np.concatenate([
        np.ascontiguousarray(mfeat.reshape(4, P).T),
        np.ascontiguousarray((projb * c0).reshape(4, P).T),
        np.broadcast_to(np.asarray(inputs["router_b"], f32), (P, E)),
        projb_col,
        projb_col * c0,
        np.full((P, 1), c0 / S_Y, f32),
        np.broadcast_to(bv, (P, HDIM)),
    ], axis=1).astype(f32)                               # [P, 97]
    bqa = np.zeros((HDIM, 4), f32)
    bqa[:, 0] = bq
    bqa[:, 1] = bk
    bqa[0, 3] = np.asarray(inputs["alpha"], f32)[h]
    wbig = np.concatenate([
        wqkv.reshape(4, P, 192).transpose(1, 0, 2).reshape(P, 768),
        maskd.reshape(P, 4 * D),
        np.asarray(inputs["router_w"], f32)
        .reshape(4, P, E).transpose(1, 0, 2).reshape(P, 4 * E),
    ], axis=1).astype(bf)                                # [P, 2880]
    b1 = bias_t("b1", 1.0, 32)
    ebias = np.concatenate([
        bias_t("b_in", S_H, 16),
        b1[:, :, :16] * S_S,
        b1[:, :, 16:],
        bias_t("b2", S_O, 16),
        bias_t("b_out", S_O * S_W, 4),
    ], axis=2).astype(f32)                               # [P, 2, 68]
    return {
        "xtb": np.ascontiguousarray(
            xT.reshape(4, P, N).transpose(1, 0, 2)).astype(bf),
        "xmask": xmask,
        "smalls": smalls,
        "bqalpha": bqa,
        "wbig": wbig,
        "grow": np.concatenate(
            [g, np.full((HDIM,), np.asarray(inputs["alpha"], f32)[h])]
        ).reshape(1, D + HDIM).astype(bf),
        "selb": selb.astype(bf),
        "w_in8": w_in8, "w18": w18, "w28": w28, "wout8": wout8,
        "ebias": ebias,
    }


last_result = [None]


def kernel(**inputs):
    if "nc" not in _cache:
        _cache["nc"] = build_program()
    nc = _cache["nc"]
    in_maps = [_prep_inputs(inputs, c) for c in range(NCORES)]
    res = run_bass_kernel_spmd(nc, in_maps, core_ids=list(range(NCORES)))
    last_result[0] = res
    outT = np.concatenate(
        [np.asarray(res.results[c]["out"]).astype(np.float32)
         for c in range(NCORES)], axis=0)                # [512, 2048]
    return np.ascontiguousarray(outT.T).reshape(2, 1024, 512)
